# revision 17
# baseline (speedup 1.0000x reference)
"""Trainium2 Bass kernel for nn_Encoder_Postnet (ragged_sequence).

Computation (reference):
    idx   = sequential aligner scan over (align_phone, text_phone)   [B,T]
    out   = enc[idx] + pitch @ w_pitch + b_pitch + emb_beats[beats]
            + (enc[idx] + pe) @ w_pos + b_pos

Key algebraic restructure: the duration-expansion gather commutes with the
E x E linear, so
    out[t] = encG[idx_t] + (pe @ w_pos)[t] + pitch[t]*w_pitch + beats[t]*demb + bias
with encG = enc @ (I + w_pos) computed once per batch row ([P,E] not [T,E]),
collapsing the big [B*T,E]@[E,E] matmul 8x and making the kernel memory-bound.

Sharding: pure data parallel, 2 batch rows per core across 8 cores.

Device pipeline per core:
  phase A: encG = enc_row @ (I+w_pos) via PE (fp32), staged to DRAM scratch.
  phase B: duration-expand encG rows to tokens, add (pe@w_pos) tile (DVE),
           add pitch/beats/bias terms via a tiny K=3 PE matmul into PSUM,
           DVE add, DMA out.

The aligner scan is index metadata ([B,T] int32); it is computed on host with
a run-compressed O(B*P) algorithm (exactly equivalent to the reference scan).
Two device programs exist:
  - fast path: when idx == arange(T)//8 for every row (the uniform duration-8
    expansion this model produces), the expansion is a static step-0
    replication access pattern on a plain HWDGE DMA - no gather instruction.
  - general path: arbitrary idx, via per-128-token indirect DMA row gathers
    (production-shaped offset [128,1] DynamicAP descriptors).
"""

import sys

for _p in ("/opt/trn_rl_repo",):
    if _p not in sys.path:
        sys.path.insert(0, _p)

import numpy as np

B, P, T, E = 16, 1024, 8192, 256
NCORES = 8
RPC = B // NCORES          # batch rows per core
CHUNK = 2048               # tokens per expansion chunk (fast path)
NCHUNK = T // CHUNK        # 4
NB = CHUNK // 128          # 16 token-blocks per chunk
NGRP = T // 128            # 64 groups of 128 tokens per row
DUR = T // P               # uniform duration of the fast path (8)

FORCE_GENERAL = False      # test hook: force the arbitrary-idx path
_CACHE = {}


# --------------------------------------------------------------------------
# Host: aligner index computation (exact replica of the reference recurrence)
# --------------------------------------------------------------------------
def compute_idx(align, text):
    """idx[b,0]=0; idx[b,j] = idx[b,j-1] if align[b,j]==text[b,idx[b,j-1]]
    else min(idx[b,j-1]+1, P-1).   Vectorized over batch via segment starts:
    the pointer advances i->i+1 at s_{i+1} = first j >= s_i+1 with
    align[j] != text[i]; within a run of align values equal to text[i] the
    first mismatch is the run end."""
    align = np.asarray(align)
    text = np.asarray(text)
    Bn, Tn = align.shape
    Pn = text.shape[1]
    diff = align[:, 1:] != align[:, :-1]                       # [B, T-1]
    c = np.full((Bn, Tn), Tn, np.int64)
    c[:, :-1] = np.where(diff, np.arange(1, Tn)[None, :], Tn)
    re = np.flip(np.minimum.accumulate(np.flip(c, axis=1), axis=1), axis=1)

    s = np.full((Bn, Pn), Tn, np.int64)
    s[:, 0] = 0
    cur = np.zeros(Bn, np.int64)
    arB = np.arange(Bn)
    for i in range(Pn - 1):
        j0 = cur + 1
        active = j0 < Tn
        j0c = np.minimum(j0, Tn - 1)
        eq = (align[arB, j0c] == text[:, i]) & active
        nxt = np.where(active, np.where(eq, re[arB, j0c], j0), Tn)
        s[:, i + 1] = nxt
        cur = nxt
    idx = np.empty((Bn, Tn), np.int32)
    pos = np.arange(Tn)
    for b in range(Bn):
        idx[b] = (np.searchsorted(s[b], pos, side="right") - 1).astype(np.int32)
    return idx


def _positional_encoding_f64(t, e):
    pos = np.arange(t, dtype=np.float64)[:, None]
    div = np.exp(np.arange(0, e, 2, dtype=np.float64) * (-np.log(10000.0) / e))
    ang = pos * div[None, :]
    return np.stack([np.sin(ang), np.cos(ang)], axis=-1).reshape(t, e)


# --------------------------------------------------------------------------
# Device programs
# --------------------------------------------------------------------------
def _common_prelude(fast):
    import concourse.tile as tile
    from concourse import bacc, mybir
    from concourse._compat import get_trn_type

    f32 = mybir.dt.float32
    nc = bacc.Bacc(get_trn_type() or "TRN2", target_bir_lowering=False, debug=False)
    enc_t = nc.declare_dram_parameter("enc_t", [RPC, E, P], f32, isOutput=False)
    g_mat = nc.declare_dram_parameter("g_mat", [E, E], f32, isOutput=False)
    pe_w = nc.declare_dram_parameter("pe_w", [128, NGRP, E], f32, isOutput=False)
    p3 = nc.declare_dram_parameter("p3", [RPC, 3, T], f32, isOutput=False)
    w3 = nc.declare_dram_parameter("w3", [3, E], f32, isOutput=False)
    if not fast:
        idxo = nc.declare_dram_parameter(
            "idxo", [RPC, 128, NGRP], mybir.dt.int32, isOutput=False
        )
    else:
        idxo = None
    out = nc.declare_dram_parameter("out", [RPC, T, E], f32, isOutput=True)
    encg = nc.dram_tensor("encg", [RPC, P, E], f32)
    return nc, tile, mybir, (enc_t, g_mat, pe_w, p3, w3, idxo, out, encg)


def _emit_phase_a(nc, pools, tensors, f32):
    (enc_t, g_mat, pe_w, p3, w3, idxo, out, encg) = tensors
    const, encT_pool, psum_pool, eg_pool = pools
    g0 = const.tile([128, E], f32, tag="g0")
    g1 = const.tile([128, E], f32, tag="g1")
    nc.sync.dma_start(g0[:], g_mat[0:128, :])
    nc.sync.dma_start(g1[:], g_mat[128:256, :])
    w3_sb = const.tile([3, E], f32, tag="w3")
    nc.sync.dma_start(w3_sb[:], w3[:, :])
    p3_sb = []
    for r in range(RPC):
        p3t = const.tile([3, T], f32, tag=f"p3_{r}")
        nc.sync.dma_start(p3t[:], p3[r])
        p3_sb.append(p3t)

    for r in range(RPC):
        et0 = encT_pool.tile([128, P], f32, tag="et0")
        et1 = encT_pool.tile([128, P], f32, tag="et1")
        nc.sync.dma_start(et0[:], enc_t[r, 0:128, :])
        nc.sync.dma_start(et1[:], enc_t[r, 128:256, :])
        ps = psum_pool.tile([128, 8 * E], f32, tag="ps")
        for m in range(8):
            nc.tensor.matmul(
                ps[:, m * E:(m + 1) * E],
                lhsT=et0[:, m * 128:(m + 1) * 128],
                rhs=g0[:],
                start=True, stop=False,
            )
            nc.tensor.matmul(
                ps[:, m * E:(m + 1) * E],
                lhsT=et1[:, m * 128:(m + 1) * 128],
                rhs=g1[:],
                start=False, stop=True,
            )
        eg = eg_pool.tile([128, 8 * E], f32, tag="eg")
        nc.vector.tensor_copy(eg[:], ps[:])
        nc.sync.dma_start(
            encg[r].rearrange("(m p) e -> p m e", p=128),
            eg[:].rearrange("q (m e) -> q m e", e=E),
        )
    return w3_sb, p3_sb


def build_nc_fast():
    """Uniform duration-8 path.

    Per 128-token group, everything lands in one PSUM accumulation group of
    TWO bf16 matmuls (bf16 hi/lo splitting keeps ~17 mantissa bits):
        lhsT [19, 128] = [S one-hot(16); beats; pitch_hi; pitch_lo]
        rhs_hi [19, E] = [encG_hi rows(16); demb_hi; w_pitch_hi; w_pitch_hi]
        rhs_lo [19, E] = [encG_lo rows(16); demb_lo; w_pitch_lo; 0]
    so psum = encG[t//8] + beats*demb + pitch*w_pitch.  One DVE add folds in
    the f32 (pe@w_pos + bias) tile, then DMA out."""
    from contextlib import ExitStack
    import concourse.tile as tile
    from concourse import bacc, mybir
    from concourse._compat import get_trn_type

    f32 = mybir.dt.float32
    bf16 = mybir.dt.bfloat16
    mult = mybir.AluOpType.mult
    add = mybir.AluOpType.add
    NSUP = 8                # token groups per super-chunk
    NW = 128 // DUR         # encG rows per group (16)
    KC = NW + 3             # combo contraction dim (19)

    nc = bacc.Bacc(get_trn_type() or "TRN2", target_bir_lowering=False, debug=False)
    enc_t = nc.declare_dram_parameter("enc_t", [RPC, E, P], f32, isOutput=False)
    g_mat = nc.declare_dram_parameter("g_mat", [E, E], f32, isOutput=False)
    pe_w = nc.declare_dram_parameter("pe_w", [128, NGRP, E], f32, isOutput=False)
    l3 = nc.declare_dram_parameter("l3", [RPC, KC, T], bf16, isOutput=False)
    wc_hi = nc.declare_dram_parameter("wc_hi", [3, NSUP, E], bf16, isOutput=False)
    wc_lo = nc.declare_dram_parameter("wc_lo", [3, NSUP, E], bf16, isOutput=False)
    out = nc.declare_dram_parameter("out", [RPC, T, E], f32, isOutput=True)
    encg_hi = nc.dram_tensor("encg_hi", [RPC, P, E], bf16)
    encg_lo = nc.dram_tensor("encg_lo", [RPC, P, E], bf16)

    with tile.TileContext(nc) as tc, ExitStack() as ctx:
        const = ctx.enter_context(tc.tile_pool(name="const", bufs=1))
        pe_pool = ctx.enter_context(tc.tile_pool(name="pe", bufs=2))
        out_pool = ctx.enter_context(tc.tile_pool(name="outp", bufs=3))
        eg16_pool = ctx.enter_context(tc.tile_pool(name="eg16", bufs=3))

        g0 = const.tile([128, E], f32, tag="g0")
        g1 = const.tile([128, E], f32, tag="g1")
        nc.sync.dma_start(g0[:], g_mat[0:128, :])
        nc.sync.dma_start(g1[:], g_mat[128:256, :])
        l3_sb = []
        for r in range(RPC):
            l3t = const.tile([KC, T], bf16, tag=f"l3_{r}")
            nc.sync.dma_start(l3t[:], l3[r])
            l3_sb.append(l3t)

        # ---- phase A: encG = enc @ (I+w_pos), split to bf16 hi/lo in DRAM ----
        with tc.tile_pool(name="psumA", bufs=2, space="PSUM") as psum_a, \
             tc.tile_pool(name="encT", bufs=2) as encT_pool, \
             tc.tile_pool(name="egA", bufs=2) as eg_pool:
            for r in range(RPC):
                et0 = encT_pool.tile([128, P], f32, tag="et0")
                et1 = encT_pool.tile([128, P], f32, tag="et1")
                nc.sync.dma_start(et0[:], enc_t[r, 0:128, :])
                nc.sync.dma_start(et1[:], enc_t[r, 128:256, :])
                ps = psum_a.tile([128, 8 * E], f32, tag="psA")
                for m in range(8):
                    nc.tensor.matmul(
                        ps[:, m * E:(m + 1) * E],
                        lhsT=et0[:, m * 128:(m + 1) * 128],
                        rhs=g0[:], start=True, stop=False,
                    )
                    nc.tensor.matmul(
                        ps[:, m * E:(m + 1) * E],
                        lhsT=et1[:, m * 128:(m + 1) * 128],
                        rhs=g1[:], start=False, stop=True,
                    )
                eg_hi = eg_pool.tile([128, 8 * E], bf16, tag="egh")
                eg_lo = eg_pool.tile([128, 8 * E], bf16, tag="egl")
                nc.vector.tensor_copy(eg_hi[:], ps[:])
                nc.vector.scalar_tensor_tensor(
                    out=eg_lo[:], in0=eg_hi[:], scalar=-1.0, in1=ps[:],
                    op0=mult, op1=add,
                )
                for dst, src in ((encg_hi, eg_hi), (encg_lo, eg_lo)):
                    nc.sync.dma_start(
                        dst[r].rearrange("(m p) e -> p m e", p=128),
                        src[:].rearrange("q (m e) -> q m e", e=E),
                    )

        # ---- phase B ----
        with tc.tile_pool(name="psumB", bufs=8, space="PSUM") as psum_b:
            for s in range(T // (NSUP * 128)):
                pe_t = pe_pool.tile([128, NSUP, E], f32, tag="pe")
                nc.sync.dma_start(pe_t[:], pe_w[:, s * NSUP:(s + 1) * NSUP, :])
                for r in range(RPC):
                    # combo rhs tile: rows 0..15 encG rows (partition w holds
                    # row s*128+j*16+w at block j), rows 16..18 constants
                    egh = eg16_pool.tile([KC, NSUP, E], bf16, tag="egh16")
                    egl = eg16_pool.tile([KC, NSUP, E], bf16, tag="egl16")
                    for t16, src, wc in ((egh, encg_hi, wc_hi), (egl, encg_lo, wc_lo)):
                        nc.sync.dma_start(
                            t16[0:NW, :, :],
                            src[r, s * 128:(s + 1) * 128, :].rearrange(
                                "(g w) e -> w g e", w=NW
                            ),
                        )
                        nc.sync.dma_start(t16[NW:KC, :, :], wc[:, :, :])
                    ot = out_pool.tile([128, NSUP, E], f32, tag="ot")
                    for j in range(NSUP):
                        g = s * NSUP + j
                        ps = psum_b.tile([128, E], f32, tag="ps")
                        nc.tensor.matmul(
                            ps[:],
                            lhsT=l3_sb[r][:, g * 128:(g + 1) * 128],
                            rhs=egh[:, j, :],
                            start=True, stop=False,
                        )
                        nc.tensor.matmul(
                            ps[:],
                            lhsT=l3_sb[r][:, g * 128:(g + 1) * 128],
                            rhs=egl[:, j, :],
                            start=False, stop=True,
                        )
                        nc.vector.tensor_add(ot[:, j, :], ps[:], pe_t[:, j, :])
                    nc.sync.dma_start(
                        out[r, s * NSUP * 128:(s + 1) * NSUP * 128, :].rearrange(
                            "(n p) e -> p n e", p=128
                        ),
                        ot[:],
                    )
    nc.compile()
    return nc


def build_nc_general():
    """Arbitrary-idx path: per-128-token indirect row gathers."""
    import concourse.bass as bass
    from contextlib import ExitStack

    nc, tile, mybir, tensors = _common_prelude(fast=False)
    (enc_t, g_mat, pe_w, p3, w3, idxo, out, encg) = tensors
    f32 = mybir.dt.float32
    i32 = mybir.dt.int32

    with tile.TileContext(nc) as tc, ExitStack() as ctx:
        const = ctx.enter_context(tc.tile_pool(name="const", bufs=1))
        encT_pool = ctx.enter_context(tc.tile_pool(name="encT", bufs=2))
        psum_pool = ctx.enter_context(tc.tile_pool(name="psum", bufs=2, space="PSUM"))
        eg_pool = ctx.enter_context(tc.tile_pool(name="eg", bufs=2))
        pe_pool = ctx.enter_context(tc.tile_pool(name="pe", bufs=2))
        gath_pool = ctx.enter_context(tc.tile_pool(name="gath", bufs=3))

        w3_sb, p3_sb = _emit_phase_a(
            nc, (const, encT_pool, psum_pool, eg_pool), tensors, f32
        )
        ixo_sb = []
        for r in range(RPC):
            ixt = const.tile([128, NGRP], i32, tag=f"ixo_{r}")
            nc.sync.dma_start(ixt[:], idxo[r])
            ixo_sb.append(ixt)

        encg_flat = encg[:].rearrange("r p e -> (r p) e")
        NSUP = 8                # token groups per super-chunk
        for s in range(T // (NSUP * 128)):
            pe_t = pe_pool.tile([128, NSUP, E], f32, tag="pe")
            nc.sync.dma_start(
                pe_t[:], pe_w[:, s * NSUP:(s + 1) * NSUP, :]
            )
            for r in range(RPC):
                gt = gath_pool.tile([128, NSUP, E], f32, tag="gt")
                for g in range(NSUP):
                    gi = s * NSUP + g
                    nc.gpsimd.indirect_dma_start(
                        out=gt[:, g, :],
                        out_offset=None,
                        in_=encg_flat,
                        in_offset=bass.IndirectOffsetOnAxis(
                            ap=ixo_sb[r][:, gi:gi + 1], axis=0
                        ),
                    )
                nc.vector.tensor_add(gt[:], gt[:], pe_t[:])
                ps = psum_pool.tile([128, 8 * E], f32, tag="ps")
                for g in range(NSUP):
                    gi = s * NSUP + g
                    nc.tensor.matmul(
                        ps[:, g * E:(g + 1) * E],
                        lhsT=p3_sb[r][:, gi * 128:(gi + 1) * 128],
                        rhs=w3_sb[:],
                        start=True, stop=True,
                    )
                nc.vector.tensor_add(
                    gt[:], gt[:], ps[:].rearrange("q (n e) -> q n e", e=E)
                )
                nc.sync.dma_start(
                    out[r, s * NSUP * 128:(s + 1) * NSUP * 128, :].rearrange(
                        "(n p) e -> p n e", p=128
                    ),
                    gt[:],
                )
    nc.compile()
    return nc


def get_nc(fast):
    key = "nc_fast" if fast else "nc_gen"
    if key not in _CACHE:
        _CACHE[key] = build_nc_fast() if fast else build_nc_general()
    return _CACHE[key]


# --------------------------------------------------------------------------
# Host wrapper
# --------------------------------------------------------------------------
def make_in_maps(encoder_out, align_phone, text_phone, pitch, beats,
                 w_pitch, b_pitch, emb_beats, w_pos, b_pos):
    encoder_out = np.asarray(encoder_out, np.float32)
    pitch = np.asarray(pitch, np.float32)
    beats = np.asarray(beats)
    w_pitch = np.asarray(w_pitch, np.float32)
    w_pos = np.asarray(w_pos, np.float32)

    idx = compute_idx(np.asarray(align_phone), np.asarray(text_phone))
    fast = bool(np.all(idx == (np.arange(T, dtype=np.int32) // DUR)[None, :]))
    if FORCE_GENERAL:
        fast = False

    g_mat = (np.eye(E, dtype=np.float64) + w_pos.astype(np.float64)).astype(np.float32)
    pe = _positional_encoding_f64(T, E)
    pe_proj = pe @ w_pos.astype(np.float64)                          # [T, E]
    bias = (np.asarray(emb_beats[0], np.float64)
            + np.asarray(b_pitch, np.float64)
            + np.asarray(b_pos, np.float64))
    demb = np.asarray(emb_beats[1], np.float64) - np.asarray(emb_beats[0], np.float64)

    def bf16_split(x):
        import ml_dtypes
        hi = np.asarray(x, np.float32).astype(ml_dtypes.bfloat16)
        lo = (np.asarray(x, np.float32) - hi.astype(np.float32)).astype(
            ml_dtypes.bfloat16
        )
        return hi, lo

    if fast:
        NSUP, NW, KC = 8, 128 // DUR, 128 // DUR + 3
        pe_tot = (pe_proj + bias[None, :]).astype(np.float32)
        pe_wl = np.ascontiguousarray(pe_tot.reshape(NGRP, 128, E).swapaxes(0, 1))
        demb_hi, demb_lo = bf16_split(demb)
        wp_hi, wp_lo = bf16_split(w_pitch[0])
        import ml_dtypes
        wc_hi = np.broadcast_to(
            np.stack([demb_hi, wp_hi, wp_hi])[:, None, :], (3, NSUP, E)
        ).astype(ml_dtypes.bfloat16).copy()
        wc_lo = np.broadcast_to(
            np.stack([demb_lo, wp_lo, np.zeros(E, ml_dtypes.bfloat16)])[:, None, :],
            (3, NSUP, E),
        ).astype(ml_dtypes.bfloat16).copy()
        s_onehot = (np.arange(NW)[:, None]
                    == (np.arange(128) // DUR)[None, :]).astype(np.float32)
    else:
        w3 = np.stack([w_pitch[0].astype(np.float64), demb, bias]).astype(np.float32)
        pe_wl = np.ascontiguousarray(
            pe_proj.astype(np.float32).reshape(NGRP, 128, E).swapaxes(0, 1)
        )

    in_maps = []
    for core in range(NCORES):
        rows = range(core * RPC, (core + 1) * RPC)
        enc_t = np.ascontiguousarray(
            encoder_out[core * RPC:(core + 1) * RPC].transpose(0, 2, 1)
        )
        m = {"enc_t": enc_t, "g_mat": g_mat, "pe_w": pe_wl}
        if fast:
            import ml_dtypes
            l3 = np.zeros((RPC, KC, T), ml_dtypes.bfloat16)
            for j, b in enumerate(rows):
                l3[j, 0:NW] = np.tile(s_onehot, (1, NGRP)).astype(ml_dtypes.bfloat16)
                l3[j, NW] = beats[b, :, 0].astype(ml_dtypes.bfloat16)
                p_hi, p_lo = bf16_split(pitch[b, :, 0])
                l3[j, NW + 1] = p_hi
                l3[j, NW + 2] = p_lo
            m["l3"] = l3
            m["wc_hi"] = wc_hi
            m["wc_lo"] = wc_lo
        else:
            p3 = np.empty((RPC, 3, T), np.float32)
            idxo = np.empty((RPC, 128, NGRP), np.int32)
            for j, b in enumerate(rows):
                p3[j, 0] = pitch[b, :, 0]
                p3[j, 1] = beats[b, :, 0].astype(np.float32)
                p3[j, 2] = 1.0
                idxo[j] = idx[b].reshape(NGRP, 128).T + j * P
            m["p3"] = p3
            m["w3"] = w3
            m["idxo"] = idxo
        in_maps.append(m)
    return fast, in_maps


def kernel(**inputs):
    from concourse.bass_utils import run_bass_kernel_spmd

    fast, in_maps = make_in_maps(**inputs)
    nc = get_nc(fast)
    res = run_bass_kernel_spmd(nc, in_maps, core_ids=list(range(NCORES)))
    out = np.concatenate([res.results[i]["out"] for i in range(NCORES)], axis=0)
    return np.ascontiguousarray(out.astype(np.float32))


# revision 23
# speedup vs baseline: 1.7374x; 1.7374x over previous
"""Trainium2 Bass kernel for nn_Encoder_Postnet (ragged_sequence).

Computation (reference):
    idx   = sequential aligner scan over (align_phone, text_phone)   [B,T]
    out   = enc[idx] + pitch @ w_pitch + b_pitch + emb_beats[beats]
            + (enc[idx] + pe) @ w_pos + b_pos

Key algebraic restructure: the duration-expansion gather commutes with the
E x E linear, so
    out[t] = encG[idx_t] + (pe@w_pos + bias)[t] + pitch[t]*w_pitch + beats[t]*demb
with encG = enc @ (I + w_pos) computed once per batch row ([P,E] not [T,E]),
collapsing the big [B*T,E]@[E,E] matmul 8x and making the kernel memory-bound.

Sharding: pure data parallel, 2 batch rows per core across 8 cores.

Fast path (the uniform duration-8 expansion this model's inputs produce,
idx == arange(T)//8 for every row):
  phase A: encG = enc_row @ (I+w_pos) on PE (fp32); the result stays resident
           in SBUF split into bf16 hi/lo pairs (hi+lo keeps ~17 mantissa
           bits, and bf16 matmuls run 4x faster than fp32 on the PE).
  phase B: per 128-token group, one PSUM accumulation group of 4 bf16
           matmuls: S_j-one-hot expansion of encG rows (hi+lo) + identity
           matmuls adding the (pe@w_pos + bias) tile (hi+lo).  Then the
           pitch rank-1 term rides the DVE op that drains PSUM->SBUF
           (scalar_tensor_tensor), and the beats term runs on the otherwise
           idle GPSIMD.  The only DMA traffic is inputs-once + outputs-once.

General path (arbitrary idx): per-128-token indirect-DMA row gathers
(production-shaped offset [128,1] DynamicAP descriptors) + K=3 stream matmul.

The aligner scan itself is index metadata ([B,T] int32); it is computed on
host with a run-compressed O(B*P) algorithm exactly equivalent to the
reference recurrence, then consumed either as a uniformity proof (fast path)
or as gather offsets (general path).
"""

import sys

for _p in ("/opt/trn_rl_repo",):
    if _p not in sys.path:
        sys.path.insert(0, _p)

import numpy as np

B, P, T, E = 16, 1024, 8192, 256
NCORES = 8
RPC = B // NCORES          # batch rows per core
NGRP = T // 128            # 64 groups of 128 tokens per row
NSUP = 8                   # groups per super-chunk
DUR = T // P               # uniform duration of the fast path (8)
NW = 128 // DUR            # encG rows per group (16)

FORCE_GENERAL = False      # test hook: force the arbitrary-idx path
_CACHE = {}


# --------------------------------------------------------------------------
# Host: aligner index computation (exact replica of the reference recurrence)
# --------------------------------------------------------------------------
def compute_idx(align, text):
    """idx[b,0]=0; idx[b,j] = idx[b,j-1] if align[b,j]==text[b,idx[b,j-1]]
    else min(idx[b,j-1]+1, P-1).   Vectorized over batch via segment starts:
    the pointer advances i->i+1 at s_{i+1} = first j >= s_i+1 with
    align[j] != text[i]; within a run of align values equal to text[i] the
    first mismatch is the run end."""
    align = np.asarray(align)
    text = np.asarray(text)
    Bn, Tn = align.shape
    Pn = text.shape[1]
    diff = align[:, 1:] != align[:, :-1]                       # [B, T-1]
    c = np.full((Bn, Tn), Tn, np.int64)
    c[:, :-1] = np.where(diff, np.arange(1, Tn)[None, :], Tn)
    re = np.flip(np.minimum.accumulate(np.flip(c, axis=1), axis=1), axis=1)

    s = np.full((Bn, Pn), Tn, np.int64)
    s[:, 0] = 0
    cur = np.zeros(Bn, np.int64)
    arB = np.arange(Bn)
    for i in range(Pn - 1):
        j0 = cur + 1
        active = j0 < Tn
        j0c = np.minimum(j0, Tn - 1)
        eq = (align[arB, j0c] == text[:, i]) & active
        nxt = np.where(active, np.where(eq, re[arB, j0c], j0), Tn)
        s[:, i + 1] = nxt
        cur = nxt
    idx = np.empty((Bn, Tn), np.int32)
    pos = np.arange(Tn)
    for b in range(Bn):
        idx[b] = (np.searchsorted(s[b], pos, side="right") - 1).astype(np.int32)
    return idx


def _positional_encoding_f64(t, e):
    pos = np.arange(t, dtype=np.float64)[:, None]
    div = np.exp(np.arange(0, e, 2, dtype=np.float64) * (-np.log(10000.0) / e))
    ang = pos * div[None, :]
    return np.stack([np.sin(ang), np.cos(ang)], axis=-1).reshape(t, e)


def _bf16_split(x):
    import ml_dtypes
    x = np.asarray(x, np.float32)
    hi = x.astype(ml_dtypes.bfloat16)
    lo = (x - hi.astype(np.float32)).astype(ml_dtypes.bfloat16)
    return hi, lo


# --------------------------------------------------------------------------
# Device programs
# --------------------------------------------------------------------------
def build_nc_fast():
    from contextlib import ExitStack
    import concourse.tile as tile
    from concourse import bacc, mybir
    from concourse._compat import get_trn_type

    f32 = mybir.dt.float32
    bf16 = mybir.dt.bfloat16
    mult = mybir.AluOpType.mult
    add = mybir.AluOpType.add

    nc = bacc.Bacc(get_trn_type() or "TRN2", target_bir_lowering=False, debug=False)
    enc_hi = nc.declare_dram_parameter("enc_hi", [RPC, E, P], bf16, isOutput=False)
    enc_lo = nc.declare_dram_parameter("enc_lo", [RPC, E, P], bf16, isOutput=False)
    g_hi = nc.declare_dram_parameter("g_hi", [E, E], bf16, isOutput=False)
    g_lo = nc.declare_dram_parameter("g_lo", [E, E], bf16, isOutput=False)
    pe_hi = nc.declare_dram_parameter("pe_hi", [128, NGRP, E], bf16, isOutput=False)
    pe_lo = nc.declare_dram_parameter("pe_lo", [128, NGRP, E], bf16, isOutput=False)
    sj_d = nc.declare_dram_parameter("sj", [128, NSUP * 128], bf16, isOutput=False)
    i128_d = nc.declare_dram_parameter("i128", [128, 128], bf16, isOutput=False)
    l5_d = nc.declare_dram_parameter("l5", [RPC, 5, T], bf16, isOutput=False)
    w5_d = nc.declare_dram_parameter("w5", [5, E], bf16, isOutput=False)
    out = nc.declare_dram_parameter("out", [RPC, T, E], f32, isOutput=True)

    with tile.TileContext(nc) as tc, ExitStack() as ctx:
        const = ctx.enter_context(tc.tile_pool(name="const", bufs=1))
        pe_pool = ctx.enter_context(tc.tile_pool(name="pe", bufs=2))
        out_pool = ctx.enter_context(tc.tile_pool(name="outp", bufs=3))

        sj_sb = const.tile([128, NSUP * 128], bf16, tag="sj")
        nc.sync.dma_start(sj_sb[:], sj_d[:])
        i128_sb = const.tile([128, 128], bf16, tag="i128")
        nc.sync.dma_start(i128_sb[:], i128_d[:])
        w5_sb = const.tile([5, E], bf16, tag="w5")
        nc.sync.dma_start(w5_sb[:], w5_d[:])
        l5_sb, egh_keep, egl_keep = [], [], []
        for r in range(RPC):
            l5t = const.tile([5, T], bf16, tag=f"l5_{r}")
            nc.sync.dma_start(l5t[:], l5_d[r])
            l5_sb.append(l5t)
            egh_keep.append(
                const.tile([128, NSUP, E], bf16, tag=f"egh{r}", name=f"egh{r}")
            )
            egl_keep.append(
                const.tile([128, NSUP, E], bf16, tag=f"egl{r}", name=f"egl{r}")
            )

        # ---- phase A: encG = enc @ (I+w_pos) as 3-term bf16-split matmuls;
        # result kept in SBUF as bf16 hi/lo.  psum layout [128p, m, e] ==
        # keep layout: row m*128+p at (partition p, block m).
        gh0 = const.tile([128, E], bf16, tag="gh0", name="gh0")
        gh1 = const.tile([128, E], bf16, tag="gh1", name="gh1")
        gl0 = const.tile([128, E], bf16, tag="gl0", name="gl0")
        gl1 = const.tile([128, E], bf16, tag="gl1", name="gl1")
        nc.sync.dma_start(gh0[:], g_hi[0:128, :])
        nc.sync.dma_start(gh1[:], g_hi[128:256, :])
        nc.sync.dma_start(gl0[:], g_lo[0:128, :])
        nc.sync.dma_start(gl1[:], g_lo[128:256, :])
        with tc.tile_pool(name="psumA", bufs=2, space="PSUM") as psum_a, \
             tc.tile_pool(name="encT", bufs=2) as encT_pool:
            for r in range(RPC):
                eh0 = encT_pool.tile([128, P], bf16, tag="eh0")
                eh1 = encT_pool.tile([128, P], bf16, tag="eh1")
                el0 = encT_pool.tile([128, P], bf16, tag="el0")
                el1 = encT_pool.tile([128, P], bf16, tag="el1")
                nc.sync.dma_start(eh0[:], enc_hi[r, 0:128, :])
                nc.sync.dma_start(eh1[:], enc_hi[r, 128:256, :])
                nc.sync.dma_start(el0[:], enc_lo[r, 0:128, :])
                nc.sync.dma_start(el1[:], enc_lo[r, 128:256, :])
                ps = psum_a.tile([128, 8 * E], f32, tag="psA")
                for m in range(8):
                    sl = slice(m * 128, (m + 1) * 128)
                    terms = [
                        (eh0, gh0, True, False), (eh1, gh1, False, False),
                        (el0, gh0, False, False), (el1, gh1, False, False),
                        (eh0, gl0, False, False), (eh1, gl1, False, True),
                    ]
                    for lt, gt_, st, sp in terms:
                        nc.tensor.matmul(
                            ps[:, m * E:(m + 1) * E],
                            lhsT=lt[:, sl], rhs=gt_[:], start=st, stop=sp,
                        )
                hi = egh_keep[r][:].rearrange("p m e -> p (m e)")
                lo = egl_keep[r][:].rearrange("p m e -> p (m e)")
                nc.vector.tensor_copy(hi, ps[:])
                nc.vector.scalar_tensor_tensor(
                    out=lo, in0=hi, scalar=-1.0, in1=ps[:], op0=mult, op1=add
                )

        # ---- phase B: per group, 5 bf16 matmuls into one PSUM group:
        # expansion hi/lo + pe hi/lo + the K=5 rank-1 stream matmul
        # (pitch_hi/lo x w_pitch_hi/lo cross terms + beats x demb_hi/lo).
        with tc.tile_pool(name="psumB", bufs=8, space="PSUM") as psum_b:
            for s in range(T // (NSUP * 128)):
                peh = pe_pool.tile([128, NSUP, E], bf16, tag="peh")
                pel = pe_pool.tile([128, NSUP, E], bf16, tag="pel")
                nc.sync.dma_start(peh[:], pe_hi[:, s * NSUP:(s + 1) * NSUP, :])
                nc.sync.dma_start(pel[:], pe_lo[:, s * NSUP:(s + 1) * NSUP, :])
                for r in range(RPC):
                    ot = out_pool.tile([128, NSUP, E], f32, tag="ot")
                    for j in range(NSUP):
                        g = s * NSUP + j
                        ps = psum_b.tile([128, E], f32, tag="ps")
                        sj_ap = sj_sb[:, j * 128:(j + 1) * 128]
                        nc.tensor.matmul(
                            ps[:], lhsT=sj_ap, rhs=egh_keep[r][:, s, :],
                            start=True, stop=False,
                        )
                        nc.tensor.matmul(
                            ps[:], lhsT=sj_ap, rhs=egl_keep[r][:, s, :],
                            start=False, stop=False,
                        )
                        nc.tensor.matmul(
                            ps[:], lhsT=i128_sb[:], rhs=peh[:, j, :],
                            start=False, stop=False,
                        )
                        nc.tensor.matmul(
                            ps[:], lhsT=i128_sb[:], rhs=pel[:, j, :],
                            start=False, stop=False,
                        )
                        nc.tensor.matmul(
                            ps[:],
                            lhsT=l5_sb[r][:, g * 128:(g + 1) * 128],
                            rhs=w5_sb[:],
                            start=False, stop=True,
                        )
                        nc.vector.tensor_copy(ot[:, j, :], ps[:])
                    nc.sync.dma_start(
                        out[r, s * NSUP * 128:(s + 1) * NSUP * 128, :].rearrange(
                            "(n p) e -> p n e", p=128
                        ),
                        ot[:],
                    )
    nc.compile()
    return nc


def build_nc_general():
    """Arbitrary-idx path: per-128-token indirect row gathers."""
    import concourse.bass as bass
    from contextlib import ExitStack
    import concourse.tile as tile
    from concourse import bacc, mybir
    from concourse._compat import get_trn_type

    f32 = mybir.dt.float32
    i32 = mybir.dt.int32

    nc = bacc.Bacc(get_trn_type() or "TRN2", target_bir_lowering=False, debug=False)
    enc_t = nc.declare_dram_parameter("enc_t", [RPC, E, P], f32, isOutput=False)
    g_mat = nc.declare_dram_parameter("g_mat", [E, E], f32, isOutput=False)
    pe_w = nc.declare_dram_parameter("pe_w", [128, NGRP, E], f32, isOutput=False)
    p3 = nc.declare_dram_parameter("p3", [RPC, 3, T], f32, isOutput=False)
    w3 = nc.declare_dram_parameter("w3", [3, E], f32, isOutput=False)
    idxo = nc.declare_dram_parameter(
        "idxo", [RPC, 128, NGRP], i32, isOutput=False
    )
    out = nc.declare_dram_parameter("out", [RPC, T, E], f32, isOutput=True)
    encg = nc.dram_tensor("encg", [RPC, P, E], f32)

    with tile.TileContext(nc) as tc, ExitStack() as ctx:
        const = ctx.enter_context(tc.tile_pool(name="const", bufs=1))
        encT_pool = ctx.enter_context(tc.tile_pool(name="encT", bufs=2))
        psum_pool = ctx.enter_context(tc.tile_pool(name="psum", bufs=2, space="PSUM"))
        eg_pool = ctx.enter_context(tc.tile_pool(name="eg", bufs=2))
        pe_pool = ctx.enter_context(tc.tile_pool(name="pe", bufs=2))
        gath_pool = ctx.enter_context(tc.tile_pool(name="gath", bufs=3))

        g0 = const.tile([128, E], f32, tag="g0")
        g1 = const.tile([128, E], f32, tag="g1")
        nc.sync.dma_start(g0[:], g_mat[0:128, :])
        nc.sync.dma_start(g1[:], g_mat[128:256, :])
        w3_sb = const.tile([3, E], f32, tag="w3")
        nc.sync.dma_start(w3_sb[:], w3[:, :])
        p3_sb = []
        ixo_sb = []
        for r in range(RPC):
            p3t = const.tile([3, T], f32, tag=f"p3_{r}")
            nc.sync.dma_start(p3t[:], p3[r])
            p3_sb.append(p3t)
            ixt = const.tile([128, NGRP], i32, tag=f"ixo_{r}")
            nc.sync.dma_start(ixt[:], idxo[r])
            ixo_sb.append(ixt)

        for r in range(RPC):
            et0 = encT_pool.tile([128, P], f32, tag="et0")
            et1 = encT_pool.tile([128, P], f32, tag="et1")
            nc.sync.dma_start(et0[:], enc_t[r, 0:128, :])
            nc.sync.dma_start(et1[:], enc_t[r, 128:256, :])
            ps = psum_pool.tile([128, 8 * E], f32, tag="ps")
            for m in range(8):
                nc.tensor.matmul(
                    ps[:, m * E:(m + 1) * E],
                    lhsT=et0[:, m * 128:(m + 1) * 128],
                    rhs=g0[:], start=True, stop=False,
                )
                nc.tensor.matmul(
                    ps[:, m * E:(m + 1) * E],
                    lhsT=et1[:, m * 128:(m + 1) * 128],
                    rhs=g1[:], start=False, stop=True,
                )
            eg = eg_pool.tile([128, 8 * E], f32, tag="eg")
            nc.vector.tensor_copy(eg[:], ps[:])
            nc.sync.dma_start(
                encg[r].rearrange("(m p) e -> p m e", p=128),
                eg[:].rearrange("q (m e) -> q m e", e=E),
            )

        encg_flat = encg[:].rearrange("r p e -> (r p) e")
        for s in range(T // (NSUP * 128)):
            pe_t = pe_pool.tile([128, NSUP, E], f32, tag="pe")
            nc.sync.dma_start(pe_t[:], pe_w[:, s * NSUP:(s + 1) * NSUP, :])
            for r in range(RPC):
                gt = gath_pool.tile([128, NSUP, E], f32, tag="gt")
                for g in range(NSUP):
                    gi = s * NSUP + g
                    nc.gpsimd.indirect_dma_start(
                        out=gt[:, g, :],
                        out_offset=None,
                        in_=encg_flat,
                        in_offset=bass.IndirectOffsetOnAxis(
                            ap=ixo_sb[r][:, gi:gi + 1], axis=0
                        ),
                    )
                nc.vector.tensor_add(gt[:], gt[:], pe_t[:])
                ps = psum_pool.tile([128, 8 * E], f32, tag="ps")
                for g in range(NSUP):
                    gi = s * NSUP + g
                    nc.tensor.matmul(
                        ps[:, g * E:(g + 1) * E],
                        lhsT=p3_sb[r][:, gi * 128:(gi + 1) * 128],
                        rhs=w3_sb[:],
                        start=True, stop=True,
                    )
                nc.vector.tensor_add(
                    gt[:], gt[:], ps[:].rearrange("q (n e) -> q n e", e=E)
                )
                nc.sync.dma_start(
                    out[r, s * NSUP * 128:(s + 1) * NSUP * 128, :].rearrange(
                        "(n p) e -> p n e", p=128
                    ),
                    gt[:],
                )
    nc.compile()
    return nc


def get_nc(fast):
    key = "nc_fast" if fast else "nc_gen"
    if key not in _CACHE:
        _CACHE[key] = build_nc_fast() if fast else build_nc_general()
    return _CACHE[key]


# --------------------------------------------------------------------------
# Host wrapper
# --------------------------------------------------------------------------
def make_in_maps(encoder_out, align_phone, text_phone, pitch, beats,
                 w_pitch, b_pitch, emb_beats, w_pos, b_pos):
    import ml_dtypes

    encoder_out = np.asarray(encoder_out, np.float32)
    pitch = np.asarray(pitch, np.float32)
    beats = np.asarray(beats)
    w_pitch = np.asarray(w_pitch, np.float32)
    w_pos = np.asarray(w_pos, np.float32)

    idx = compute_idx(np.asarray(align_phone), np.asarray(text_phone))
    fast = bool(np.all(idx == (np.arange(T, dtype=np.int32) // DUR)[None, :]))
    if FORCE_GENERAL:
        fast = False

    g_mat = (np.eye(E, dtype=np.float64) + w_pos.astype(np.float64)).astype(np.float32)
    pe = _positional_encoding_f64(T, E)
    pe_proj = pe @ w_pos.astype(np.float64)                          # [T, E]
    bias = (np.asarray(emb_beats[0], np.float64)
            + np.asarray(b_pitch, np.float64)
            + np.asarray(b_pos, np.float64))
    demb = (np.asarray(emb_beats[1], np.float64)
            - np.asarray(emb_beats[0], np.float64)).astype(np.float32)

    if fast:
        pe_tot = (pe_proj + bias[None, :]).astype(np.float32)
        pe_wrap = np.ascontiguousarray(pe_tot.reshape(NGRP, 128, E).swapaxes(0, 1))
        pe_hi, pe_lo = _bf16_split(pe_wrap)
        # S_j[k, t'] = 1 iff k == j*16 + t'//8
        rows = np.arange(128)[:, None]
        sj = np.concatenate(
            [(rows == (j * NW + np.arange(128) // DUR)[None, :]) for j in range(NSUP)],
            axis=1,
        ).astype(ml_dtypes.bfloat16)
        i128 = np.eye(128, dtype=ml_dtypes.bfloat16)
        g_hi, g_lo = _bf16_split(g_mat)
        wp_hi, wp_lo = _bf16_split(w_pitch[0])
        db_hi, db_lo = _bf16_split(demb)
        w5 = np.stack([
            wp_hi, wp_lo, wp_hi,
            db_hi, db_lo,
        ]).astype(ml_dtypes.bfloat16)
        fast_common = {
            "pe_hi": pe_hi, "pe_lo": pe_lo, "sj": sj, "i128": i128,
            "g_hi": g_hi, "g_lo": g_lo, "w5": w5,
        }
    else:
        w3 = np.stack(
            [w_pitch[0].astype(np.float64), demb.astype(np.float64), bias]
        ).astype(np.float32)
        pe_wl = np.ascontiguousarray(
            pe_proj.astype(np.float32).reshape(NGRP, 128, E).swapaxes(0, 1)
        )

    in_maps = []
    for core in range(NCORES):
        rows_ = range(core * RPC, (core + 1) * RPC)
        enc_t = np.ascontiguousarray(
            encoder_out[core * RPC:(core + 1) * RPC].transpose(0, 2, 1)
        )
        if fast:
            import ml_dtypes as _md
            enc_hi, enc_lo = _bf16_split(enc_t)
            l5 = np.zeros((RPC, 5, T), _md.bfloat16)
            for j, b in enumerate(rows_):
                p_hi, p_lo = _bf16_split(pitch[b, :, 0])
                bt = beats[b, :, 0].astype(_md.bfloat16)
                l5[j, 0] = p_hi
                l5[j, 1] = p_hi
                l5[j, 2] = p_lo
                l5[j, 3] = bt
                l5[j, 4] = bt
            m = {"enc_hi": enc_hi, "enc_lo": enc_lo, "l5": l5, **fast_common}
        else:
            p3 = np.empty((RPC, 3, T), np.float32)
            idxo = np.empty((RPC, 128, NGRP), np.int32)
            for j, b in enumerate(rows_):
                p3[j, 0] = pitch[b, :, 0]
                p3[j, 1] = beats[b, :, 0].astype(np.float32)
                p3[j, 2] = 1.0
                idxo[j] = idx[b].reshape(NGRP, 128).T + j * P
            m = {"enc_t": enc_t, "g_mat": g_mat, "pe_w": pe_wl, "p3": p3,
                 "w3": w3, "idxo": idxo}
        in_maps.append(m)
    return fast, in_maps


def kernel(**inputs):
    from concourse.bass_utils import run_bass_kernel_spmd

    fast, in_maps = make_in_maps(**inputs)
    nc = get_nc(fast)
    res = run_bass_kernel_spmd(nc, in_maps, core_ids=list(range(NCORES)))
    out = np.concatenate([res.results[i]["out"] for i in range(NCORES)], axis=0)
    return np.ascontiguousarray(out.astype(np.float32))


# revision 29
# speedup vs baseline: 1.8870x; 1.0861x over previous
"""Trainium2 Bass kernel for nn_Encoder_Postnet (ragged_sequence).

Computation (reference):
    idx   = sequential aligner scan over (align_phone, text_phone)   [B,T]
    out   = enc[idx] + pitch @ w_pitch + b_pitch + emb_beats[beats]
            + (enc[idx] + pe) @ w_pos + b_pos

Key algebraic restructure: the duration-expansion gather commutes with the
E x E linear, so
    out[t] = encG[idx_t] + (pe@w_pos + bias)[t] + pitch[t]*w_pitch + beats[t]*demb
with encG = enc @ (I + w_pos) computed once per batch row ([P,E] not [T,E]),
collapsing the big [B*T,E]@[E,E] matmul 8x and making the kernel memory-bound.

Sharding: pure data parallel, 2 batch rows per core across 8 cores.

Fast path (the uniform duration-8 expansion this model's inputs produce,
idx == arange(T)//8 for every row):
  phase A: encG = enc_row @ (I+w_pos) on PE (fp32); the result stays resident
           in SBUF split into bf16 hi/lo pairs (hi+lo keeps ~17 mantissa
           bits, and bf16 matmuls run 4x faster than fp32 on the PE).
  phase B: per 128-token group, one PSUM accumulation group of 4 bf16
           matmuls: S_j-one-hot expansion of encG rows (hi+lo) + identity
           matmuls adding the (pe@w_pos + bias) tile (hi+lo).  Then the
           pitch rank-1 term rides the DVE op that drains PSUM->SBUF
           (scalar_tensor_tensor), and the beats term runs on the otherwise
           idle GPSIMD.  The only DMA traffic is inputs-once + outputs-once.

General path (arbitrary idx): per-128-token indirect-DMA row gathers
(production-shaped offset [128,1] DynamicAP descriptors) + K=3 stream matmul.

The aligner scan itself is index metadata ([B,T] int32); it is computed on
host with a run-compressed O(B*P) algorithm exactly equivalent to the
reference recurrence, then consumed either as a uniformity proof (fast path)
or as gather offsets (general path).
"""

import sys

for _p in ("/opt/trn_rl_repo",):
    if _p not in sys.path:
        sys.path.insert(0, _p)

import numpy as np

B, P, T, E = 16, 1024, 8192, 256
NCORES = 8
RPC = B // NCORES          # batch rows per core
NGRP = T // 128            # 64 groups of 128 tokens per row
NSUP = 8                   # groups per super-chunk
DUR = T // P               # uniform duration of the fast path (8)
NW = 128 // DUR            # encG rows per group (16)

FORCE_GENERAL = False      # test hook: force the arbitrary-idx path
_CACHE = {}


# --------------------------------------------------------------------------
# Host: aligner index computation (exact replica of the reference recurrence)
# --------------------------------------------------------------------------
def compute_idx(align, text):
    """idx[b,0]=0; idx[b,j] = idx[b,j-1] if align[b,j]==text[b,idx[b,j-1]]
    else min(idx[b,j-1]+1, P-1).   Vectorized over batch via segment starts:
    the pointer advances i->i+1 at s_{i+1} = first j >= s_i+1 with
    align[j] != text[i]; within a run of align values equal to text[i] the
    first mismatch is the run end."""
    align = np.asarray(align)
    text = np.asarray(text)
    Bn, Tn = align.shape
    Pn = text.shape[1]
    diff = align[:, 1:] != align[:, :-1]                       # [B, T-1]
    c = np.full((Bn, Tn), Tn, np.int64)
    c[:, :-1] = np.where(diff, np.arange(1, Tn)[None, :], Tn)
    re = np.flip(np.minimum.accumulate(np.flip(c, axis=1), axis=1), axis=1)

    s = np.full((Bn, Pn), Tn, np.int64)
    s[:, 0] = 0
    cur = np.zeros(Bn, np.int64)
    arB = np.arange(Bn)
    for i in range(Pn - 1):
        j0 = cur + 1
        active = j0 < Tn
        j0c = np.minimum(j0, Tn - 1)
        eq = (align[arB, j0c] == text[:, i]) & active
        nxt = np.where(active, np.where(eq, re[arB, j0c], j0), Tn)
        s[:, i + 1] = nxt
        cur = nxt
    idx = np.empty((Bn, Tn), np.int32)
    pos = np.arange(Tn)
    for b in range(Bn):
        idx[b] = (np.searchsorted(s[b], pos, side="right") - 1).astype(np.int32)
    return idx


def _positional_encoding_f64(t, e):
    pos = np.arange(t, dtype=np.float64)[:, None]
    div = np.exp(np.arange(0, e, 2, dtype=np.float64) * (-np.log(10000.0) / e))
    ang = pos * div[None, :]
    return np.stack([np.sin(ang), np.cos(ang)], axis=-1).reshape(t, e)


def _bf16_split(x):
    import ml_dtypes
    x = np.asarray(x, np.float32)
    hi = x.astype(ml_dtypes.bfloat16)
    lo = (x - hi.astype(np.float32)).astype(ml_dtypes.bfloat16)
    return hi, lo


# --------------------------------------------------------------------------
# Device programs
# --------------------------------------------------------------------------
def build_nc_fast():
    from contextlib import ExitStack
    import concourse.tile as tile
    from concourse import bacc, mybir
    from concourse._compat import get_trn_type

    f32 = mybir.dt.float32
    bf16 = mybir.dt.bfloat16
    mult = mybir.AluOpType.mult
    add = mybir.AluOpType.add

    nc = bacc.Bacc(get_trn_type() or "TRN2", target_bir_lowering=False, debug=False)
    enc_hi = nc.declare_dram_parameter("enc_hi", [RPC, E, P], bf16, isOutput=False)
    enc_lo = nc.declare_dram_parameter("enc_lo", [RPC, E, P], bf16, isOutput=False)
    g_hi = nc.declare_dram_parameter("g_hi", [E, E], bf16, isOutput=False)
    g_lo = nc.declare_dram_parameter("g_lo", [E, E], bf16, isOutput=False)
    pe_hi = nc.declare_dram_parameter("pe_hi", [128, NGRP, E], bf16, isOutput=False)
    pe_lo = nc.declare_dram_parameter("pe_lo", [128, NGRP, E], bf16, isOutput=False)
    sj_d = nc.declare_dram_parameter("sj", [128, NSUP * 128], bf16, isOutput=False)
    i128_d = nc.declare_dram_parameter("i128", [128, 128], bf16, isOutput=False)
    # stream-term lhsT rows banked at 32-partition strides (4 token-chunks)
    # to keep the per-partition footprint small for the DMA
    l5_d = nc.declare_dram_parameter("l5", [RPC, 128, T // 4], bf16, isOutput=False)
    w5_d = nc.declare_dram_parameter("w5", [128, E], bf16, isOutput=False)
    out = nc.declare_dram_parameter("out", [RPC, T, E], f32, isOutput=True)

    with tile.TileContext(nc) as tc, ExitStack() as ctx:
        const = ctx.enter_context(tc.tile_pool(name="const", bufs=1))
        pe_pool = ctx.enter_context(tc.tile_pool(name="pe", bufs=2))
        out_pool = ctx.enter_context(tc.tile_pool(name="outp", bufs=3))

        sj_sb = const.tile([128, NSUP * 128], bf16, tag="sj")
        nc.sync.dma_start(sj_sb[:], sj_d[:])
        i128_sb = const.tile([128, 128], bf16, tag="i128")
        nc.sync.dma_start(i128_sb[:], i128_d[:])
        w5_sb = const.tile([128, E], bf16, tag="w5")
        nc.sync.dma_start(w5_sb[:], w5_d[:])
        l5_sb, egh_keep, egl_keep = [], [], []
        for r in range(RPC):
            l5t = const.tile([128, T // 4], bf16, tag=f"l5_{r}")
            nc.sync.dma_start(l5t[:], l5_d[r])
            l5_sb.append(l5t)
            egh_keep.append(
                const.tile([128, NSUP, E], bf16, tag=f"egh{r}", name=f"egh{r}")
            )
            egl_keep.append(
                const.tile([128, NSUP, E], bf16, tag=f"egl{r}", name=f"egl{r}")
            )

        # ---- phase A: encG = enc @ (I+w_pos) as 3-term bf16-split matmuls;
        # result kept in SBUF as bf16 hi/lo.  psum layout [128p, m, e] ==
        # keep layout: row m*128+p at (partition p, block m).
        gh0 = const.tile([128, E], bf16, tag="gh0", name="gh0")
        gh1 = const.tile([128, E], bf16, tag="gh1", name="gh1")
        gl0 = const.tile([128, E], bf16, tag="gl0", name="gl0")
        gl1 = const.tile([128, E], bf16, tag="gl1", name="gl1")
        nc.sync.dma_start(gh0[:], g_hi[0:128, :])
        nc.sync.dma_start(gh1[:], g_hi[128:256, :])
        nc.sync.dma_start(gl0[:], g_lo[0:128, :])
        nc.sync.dma_start(gl1[:], g_lo[128:256, :])
        with tc.tile_pool(name="psumA", bufs=2, space="PSUM") as psum_a, \
             tc.tile_pool(name="encT", bufs=2) as encT_pool:
            for r in range(RPC):
                eh0 = encT_pool.tile([128, P], bf16, tag="eh0")
                eh1 = encT_pool.tile([128, P], bf16, tag="eh1")
                el0 = encT_pool.tile([128, P], bf16, tag="el0")
                el1 = encT_pool.tile([128, P], bf16, tag="el1")
                nc.sync.dma_start(eh0[:], enc_hi[r, 0:128, :])
                nc.sync.dma_start(eh1[:], enc_hi[r, 128:256, :])
                nc.sync.dma_start(el0[:], enc_lo[r, 0:128, :])
                nc.sync.dma_start(el1[:], enc_lo[r, 128:256, :])
                ps = psum_a.tile([128, 8 * E], f32, tag="psA")
                for m in range(8):
                    sl = slice(m * 128, (m + 1) * 128)
                    terms = [
                        (eh0, gh0, True, False), (eh1, gh1, False, False),
                        (el0, gh0, False, False), (el1, gh1, False, False),
                        (eh0, gl0, False, False), (eh1, gl1, False, True),
                    ]
                    for lt, gt_, st, sp in terms:
                        nc.tensor.matmul(
                            ps[:, m * E:(m + 1) * E],
                            lhsT=lt[:, sl], rhs=gt_[:], start=st, stop=sp,
                        )
                hi = egh_keep[r][:].rearrange("p m e -> p (m e)")
                lo = egl_keep[r][:].rearrange("p m e -> p (m e)")
                nc.vector.tensor_copy(hi, ps[:])
                nc.vector.scalar_tensor_tensor(
                    out=lo, in0=hi, scalar=-1.0, in1=ps[:], op0=mult, op1=add
                )

        # ---- phase B: per group, 5 bf16 matmuls into one PSUM group:
        # expansion hi/lo + pe hi/lo + the K=5 rank-1 stream matmul
        # (pitch_hi/lo x w_pitch_hi/lo cross terms + beats x demb_hi/lo).
        with tc.tile_pool(name="psumB", bufs=8, space="PSUM") as psum_b:
            for s in range(T // (NSUP * 128)):
                peh = pe_pool.tile([128, NSUP, E], bf16, tag="peh")
                pel = pe_pool.tile([128, NSUP, E], bf16, tag="pel")
                nc.sync.dma_start(peh[:], pe_hi[:, s * NSUP:(s + 1) * NSUP, :])
                nc.sync.dma_start(pel[:], pe_lo[:, s * NSUP:(s + 1) * NSUP, :])
                for r in range(RPC):
                    ot = out_pool.tile([128, NSUP, E], f32, tag="ot")
                    for j in range(NSUP):
                        g = s * NSUP + j
                        ps = psum_b.tile([128, E], f32, tag="ps")
                        sj_ap = sj_sb[:, j * 128:(j + 1) * 128]
                        nc.tensor.matmul(
                            ps[:], lhsT=sj_ap, rhs=egh_keep[r][:, s, :],
                            start=True, stop=False,
                        )
                        nc.tensor.matmul(
                            ps[:], lhsT=sj_ap, rhs=egl_keep[r][:, s, :],
                            start=False, stop=False,
                        )
                        nc.tensor.matmul(
                            ps[:], lhsT=i128_sb[:], rhs=peh[:, j, :],
                            start=False, stop=False,
                        )
                        nc.tensor.matmul(
                            ps[:], lhsT=i128_sb[:], rhs=pel[:, j, :],
                            start=False, stop=False,
                        )
                        cb = g // (NGRP // 4)          # token-chunk bank
                        u0 = (g % (NGRP // 4)) * 128
                        nc.tensor.matmul(
                            ps[:],
                            lhsT=l5_sb[r][32 * cb:32 * cb + 5, u0:u0 + 128],
                            rhs=w5_sb[32 * cb:32 * cb + 5, :],
                            start=False, stop=True,
                            tile_position=(32 * cb, 0),
                        )
                        nc.vector.tensor_copy(ot[:, j, :], ps[:])
                    nc.sync.dma_start(
                        out[r, s * NSUP * 128:(s + 1) * NSUP * 128, :].rearrange(
                            "(n p) e -> p n e", p=128
                        ),
                        ot[:],
                    )
    nc.compile()
    return nc


def build_nc_general():
    """Arbitrary-idx path: per-128-token indirect row gathers."""
    import concourse.bass as bass
    from contextlib import ExitStack
    import concourse.tile as tile
    from concourse import bacc, mybir
    from concourse._compat import get_trn_type

    f32 = mybir.dt.float32
    i32 = mybir.dt.int32

    nc = bacc.Bacc(get_trn_type() or "TRN2", target_bir_lowering=False, debug=False)
    enc_t = nc.declare_dram_parameter("enc_t", [RPC, E, P], f32, isOutput=False)
    g_mat = nc.declare_dram_parameter("g_mat", [E, E], f32, isOutput=False)
    pe_w = nc.declare_dram_parameter("pe_w", [128, NGRP, E], f32, isOutput=False)
    p3 = nc.declare_dram_parameter("p3", [RPC, 3, T], f32, isOutput=False)
    w3 = nc.declare_dram_parameter("w3", [3, E], f32, isOutput=False)
    idxo = nc.declare_dram_parameter(
        "idxo", [RPC, 128, NGRP], i32, isOutput=False
    )
    out = nc.declare_dram_parameter("out", [RPC, T, E], f32, isOutput=True)
    encg = nc.dram_tensor("encg", [RPC, P, E], f32)

    with tile.TileContext(nc) as tc, ExitStack() as ctx:
        const = ctx.enter_context(tc.tile_pool(name="const", bufs=1))
        encT_pool = ctx.enter_context(tc.tile_pool(name="encT", bufs=2))
        psum_pool = ctx.enter_context(tc.tile_pool(name="psum", bufs=2, space="PSUM"))
        eg_pool = ctx.enter_context(tc.tile_pool(name="eg", bufs=2))
        pe_pool = ctx.enter_context(tc.tile_pool(name="pe", bufs=2))
        gath_pool = ctx.enter_context(tc.tile_pool(name="gath", bufs=3))

        g0 = const.tile([128, E], f32, tag="g0")
        g1 = const.tile([128, E], f32, tag="g1")
        nc.sync.dma_start(g0[:], g_mat[0:128, :])
        nc.sync.dma_start(g1[:], g_mat[128:256, :])
        w3_sb = const.tile([3, E], f32, tag="w3")
        nc.sync.dma_start(w3_sb[:], w3[:, :])
        p3_sb = []
        ixo_sb = []
        for r in range(RPC):
            p3t = const.tile([3, T], f32, tag=f"p3_{r}")
            nc.sync.dma_start(p3t[:], p3[r])
            p3_sb.append(p3t)
            ixt = const.tile([128, NGRP], i32, tag=f"ixo_{r}")
            nc.sync.dma_start(ixt[:], idxo[r])
            ixo_sb.append(ixt)

        for r in range(RPC):
            et0 = encT_pool.tile([128, P], f32, tag="et0")
            et1 = encT_pool.tile([128, P], f32, tag="et1")
            nc.sync.dma_start(et0[:], enc_t[r, 0:128, :])
            nc.sync.dma_start(et1[:], enc_t[r, 128:256, :])
            ps = psum_pool.tile([128, 8 * E], f32, tag="ps")
            for m in range(8):
                nc.tensor.matmul(
                    ps[:, m * E:(m + 1) * E],
                    lhsT=et0[:, m * 128:(m + 1) * 128],
                    rhs=g0[:], start=True, stop=False,
                )
                nc.tensor.matmul(
                    ps[:, m * E:(m + 1) * E],
                    lhsT=et1[:, m * 128:(m + 1) * 128],
                    rhs=g1[:], start=False, stop=True,
                )
            eg = eg_pool.tile([128, 8 * E], f32, tag="eg")
            nc.vector.tensor_copy(eg[:], ps[:])
            nc.sync.dma_start(
                encg[r].rearrange("(m p) e -> p m e", p=128),
                eg[:].rearrange("q (m e) -> q m e", e=E),
            )

        encg_flat = encg[:].rearrange("r p e -> (r p) e")
        for s in range(T // (NSUP * 128)):
            pe_t = pe_pool.tile([128, NSUP, E], f32, tag="pe")
            nc.sync.dma_start(pe_t[:], pe_w[:, s * NSUP:(s + 1) * NSUP, :])
            for r in range(RPC):
                gt = gath_pool.tile([128, NSUP, E], f32, tag="gt")
                for g in range(NSUP):
                    gi = s * NSUP + g
                    nc.gpsimd.indirect_dma_start(
                        out=gt[:, g, :],
                        out_offset=None,
                        in_=encg_flat,
                        in_offset=bass.IndirectOffsetOnAxis(
                            ap=ixo_sb[r][:, gi:gi + 1], axis=0
                        ),
                    )
                nc.vector.tensor_add(gt[:], gt[:], pe_t[:])
                ps = psum_pool.tile([128, 8 * E], f32, tag="ps")
                for g in range(NSUP):
                    gi = s * NSUP + g
                    nc.tensor.matmul(
                        ps[:, g * E:(g + 1) * E],
                        lhsT=p3_sb[r][:, gi * 128:(gi + 1) * 128],
                        rhs=w3_sb[:],
                        start=True, stop=True,
                    )
                nc.vector.tensor_add(
                    gt[:], gt[:], ps[:].rearrange("q (n e) -> q n e", e=E)
                )
                nc.sync.dma_start(
                    out[r, s * NSUP * 128:(s + 1) * NSUP * 128, :].rearrange(
                        "(n p) e -> p n e", p=128
                    ),
                    gt[:],
                )
    nc.compile()
    return nc


def get_nc(fast):
    key = "nc_fast" if fast else "nc_gen"
    if key not in _CACHE:
        _CACHE[key] = build_nc_fast() if fast else build_nc_general()
    return _CACHE[key]


# --------------------------------------------------------------------------
# Host wrapper
# --------------------------------------------------------------------------
def make_in_maps(encoder_out, align_phone, text_phone, pitch, beats,
                 w_pitch, b_pitch, emb_beats, w_pos, b_pos):
    import ml_dtypes

    encoder_out = np.asarray(encoder_out, np.float32)
    pitch = np.asarray(pitch, np.float32)
    beats = np.asarray(beats)
    w_pitch = np.asarray(w_pitch, np.float32)
    w_pos = np.asarray(w_pos, np.float32)

    idx = compute_idx(np.asarray(align_phone), np.asarray(text_phone))
    fast = bool(np.all(idx == (np.arange(T, dtype=np.int32) // DUR)[None, :]))
    if FORCE_GENERAL:
        fast = False

    g_mat = (np.eye(E, dtype=np.float64) + w_pos.astype(np.float64)).astype(np.float32)
    pe = _positional_encoding_f64(T, E)
    pe_proj = pe @ w_pos.astype(np.float64)                          # [T, E]
    bias = (np.asarray(emb_beats[0], np.float64)
            + np.asarray(b_pitch, np.float64)
            + np.asarray(b_pos, np.float64))
    demb = (np.asarray(emb_beats[1], np.float64)
            - np.asarray(emb_beats[0], np.float64)).astype(np.float32)

    if fast:
        pe_tot = (pe_proj + bias[None, :]).astype(np.float32)
        pe_wrap = np.ascontiguousarray(pe_tot.reshape(NGRP, 128, E).swapaxes(0, 1))
        pe_hi, pe_lo = _bf16_split(pe_wrap)
        # S_j[k, t'] = 1 iff k == j*16 + t'//8
        rows = np.arange(128)[:, None]
        sj = np.concatenate(
            [(rows == (j * NW + np.arange(128) // DUR)[None, :]) for j in range(NSUP)],
            axis=1,
        ).astype(ml_dtypes.bfloat16)
        i128 = np.eye(128, dtype=ml_dtypes.bfloat16)
        g_hi, g_lo = _bf16_split(g_mat)
        wp_hi, wp_lo = _bf16_split(w_pitch[0])
        db_hi, db_lo = _bf16_split(demb)
        w5_rows = np.stack([
            wp_hi, wp_lo, wp_hi,
            db_hi, db_lo,
        ]).astype(ml_dtypes.bfloat16)
        w5 = np.zeros((128, E), ml_dtypes.bfloat16)
        for cb in range(4):
            w5[32 * cb:32 * cb + 5] = w5_rows
        fast_common = {
            "pe_hi": pe_hi, "pe_lo": pe_lo, "sj": sj, "i128": i128,
            "g_hi": g_hi, "g_lo": g_lo, "w5": w5,
        }
    else:
        w3 = np.stack(
            [w_pitch[0].astype(np.float64), demb.astype(np.float64), bias]
        ).astype(np.float32)
        pe_wl = np.ascontiguousarray(
            pe_proj.astype(np.float32).reshape(NGRP, 128, E).swapaxes(0, 1)
        )

    in_maps = []
    for core in range(NCORES):
        rows_ = range(core * RPC, (core + 1) * RPC)
        enc_t = np.ascontiguousarray(
            encoder_out[core * RPC:(core + 1) * RPC].transpose(0, 2, 1)
        )
        if fast:
            import ml_dtypes as _md
            enc_hi, enc_lo = _bf16_split(enc_t)
            l5 = np.zeros((RPC, 128, T // 4), _md.bfloat16)
            for j, b in enumerate(rows_):
                p_hi, p_lo = _bf16_split(pitch[b, :, 0])
                bt = beats[b, :, 0].astype(_md.bfloat16)
                rows5 = np.stack([p_hi, p_hi, p_lo, bt, bt])     # [5, T]
                for cb in range(4):
                    l5[j, 32 * cb:32 * cb + 5] = rows5[
                        :, cb * (T // 4):(cb + 1) * (T // 4)
                    ]
            m = {"enc_hi": enc_hi, "enc_lo": enc_lo, "l5": l5, **fast_common}
        else:
            p3 = np.empty((RPC, 3, T), np.float32)
            idxo = np.empty((RPC, 128, NGRP), np.int32)
            for j, b in enumerate(rows_):
                p3[j, 0] = pitch[b, :, 0]
                p3[j, 1] = beats[b, :, 0].astype(np.float32)
                p3[j, 2] = 1.0
                idxo[j] = idx[b].reshape(NGRP, 128).T + j * P
            m = {"enc_t": enc_t, "g_mat": g_mat, "pe_w": pe_wl, "p3": p3,
                 "w3": w3, "idxo": idxo}
        in_maps.append(m)
    return fast, in_maps


def kernel(**inputs):
    from concourse.bass_utils import run_bass_kernel_spmd

    fast, in_maps = make_in_maps(**inputs)
    nc = get_nc(fast)
    res = run_bass_kernel_spmd(nc, in_maps, core_ids=list(range(NCORES)))
    out = np.concatenate([res.results[i]["out"] for i in range(NCORES)], axis=0)
    return np.ascontiguousarray(out.astype(np.float32))


# revision 31
# speedup vs baseline: 1.8965x; 1.0050x over previous
"""Trainium2 Bass kernel for nn_Encoder_Postnet (ragged_sequence).

Computation (reference):
    idx   = sequential aligner scan over (align_phone, text_phone)   [B,T]
    out   = enc[idx] + pitch @ w_pitch + b_pitch + emb_beats[beats]
            + (enc[idx] + pe) @ w_pos + b_pos

Key algebraic restructure: the duration-expansion gather commutes with the
E x E linear, so
    out[t] = encG[idx_t] + (pe@w_pos + bias)[t] + pitch[t]*w_pitch + beats[t]*demb
with encG = enc @ (I + w_pos) computed once per batch row ([P,E] not [T,E]),
collapsing the big [B*T,E]@[E,E] matmul 8x and making the kernel memory-bound.

Sharding: pure data parallel, 2 batch rows per core across 8 cores.

Fast path (the uniform duration-8 expansion this model's inputs produce,
idx == arange(T)//8 for every row):
  phase A: encG = enc_row @ (I+w_pos) on PE (fp32); the result stays resident
           in SBUF split into bf16 hi/lo pairs (hi+lo keeps ~17 mantissa
           bits, and bf16 matmuls run 4x faster than fp32 on the PE).
  phase B: per 128-token group, one PSUM accumulation group of 4 bf16
           matmuls: S_j-one-hot expansion of encG rows (hi+lo) + identity
           matmuls adding the (pe@w_pos + bias) tile (hi+lo).  Then the
           pitch rank-1 term rides the DVE op that drains PSUM->SBUF
           (scalar_tensor_tensor), and the beats term runs on the otherwise
           idle GPSIMD.  The only DMA traffic is inputs-once + outputs-once.

General path (arbitrary idx): per-128-token indirect-DMA row gathers
(production-shaped offset [128,1] DynamicAP descriptors) + K=3 stream matmul.

The aligner scan itself is index metadata ([B,T] int32); it is computed on
host with a run-compressed O(B*P) algorithm exactly equivalent to the
reference recurrence, then consumed either as a uniformity proof (fast path)
or as gather offsets (general path).
"""

import sys

for _p in ("/opt/trn_rl_repo",):
    if _p not in sys.path:
        sys.path.insert(0, _p)

import numpy as np

B, P, T, E = 16, 1024, 8192, 256
NCORES = 8
RPC = B // NCORES          # batch rows per core
NGRP = T // 128            # 64 groups of 128 tokens per row
NSUP = 8                   # groups per super-chunk
DUR = T // P               # uniform duration of the fast path (8)
NW = 128 // DUR            # encG rows per group (16)

FORCE_GENERAL = False      # test hook: force the arbitrary-idx path
_CACHE = {}


# --------------------------------------------------------------------------
# Host: aligner index computation (exact replica of the reference recurrence)
# --------------------------------------------------------------------------
def compute_idx(align, text):
    """idx[b,0]=0; idx[b,j] = idx[b,j-1] if align[b,j]==text[b,idx[b,j-1]]
    else min(idx[b,j-1]+1, P-1).   Vectorized over batch via segment starts:
    the pointer advances i->i+1 at s_{i+1} = first j >= s_i+1 with
    align[j] != text[i]; within a run of align values equal to text[i] the
    first mismatch is the run end."""
    align = np.asarray(align)
    text = np.asarray(text)
    Bn, Tn = align.shape
    Pn = text.shape[1]
    diff = align[:, 1:] != align[:, :-1]                       # [B, T-1]
    c = np.full((Bn, Tn), Tn, np.int64)
    c[:, :-1] = np.where(diff, np.arange(1, Tn)[None, :], Tn)
    re = np.flip(np.minimum.accumulate(np.flip(c, axis=1), axis=1), axis=1)

    s = np.full((Bn, Pn), Tn, np.int64)
    s[:, 0] = 0
    cur = np.zeros(Bn, np.int64)
    arB = np.arange(Bn)
    for i in range(Pn - 1):
        j0 = cur + 1
        active = j0 < Tn
        j0c = np.minimum(j0, Tn - 1)
        eq = (align[arB, j0c] == text[:, i]) & active
        nxt = np.where(active, np.where(eq, re[arB, j0c], j0), Tn)
        s[:, i + 1] = nxt
        cur = nxt
    idx = np.empty((Bn, Tn), np.int32)
    pos = np.arange(Tn)
    for b in range(Bn):
        idx[b] = (np.searchsorted(s[b], pos, side="right") - 1).astype(np.int32)
    return idx


def _positional_encoding_f64(t, e):
    pos = np.arange(t, dtype=np.float64)[:, None]
    div = np.exp(np.arange(0, e, 2, dtype=np.float64) * (-np.log(10000.0) / e))
    ang = pos * div[None, :]
    return np.stack([np.sin(ang), np.cos(ang)], axis=-1).reshape(t, e)


def _bf16_split(x):
    import ml_dtypes
    x = np.asarray(x, np.float32)
    hi = x.astype(ml_dtypes.bfloat16)
    lo = (x - hi.astype(np.float32)).astype(ml_dtypes.bfloat16)
    return hi, lo


# --------------------------------------------------------------------------
# Device programs
# --------------------------------------------------------------------------
def build_nc_fast():
    from contextlib import ExitStack
    import concourse.tile as tile
    from concourse import bacc, mybir
    from concourse._compat import get_trn_type

    f32 = mybir.dt.float32
    bf16 = mybir.dt.bfloat16
    mult = mybir.AluOpType.mult
    add = mybir.AluOpType.add

    nc = bacc.Bacc(get_trn_type() or "TRN2", target_bir_lowering=False, debug=False)
    enc_hi = nc.declare_dram_parameter("enc_hi", [RPC, E, P], bf16, isOutput=False)
    enc_lo = nc.declare_dram_parameter("enc_lo", [RPC, E, P], bf16, isOutput=False)
    g_hi = nc.declare_dram_parameter("g_hi", [E, E], bf16, isOutput=False)
    g_lo = nc.declare_dram_parameter("g_lo", [E, E], bf16, isOutput=False)
    pe_hi = nc.declare_dram_parameter("pe_hi", [128, NGRP, E], bf16, isOutput=False)
    pe_lo = nc.declare_dram_parameter("pe_lo", [128, NGRP, E], bf16, isOutput=False)
    sj_d = nc.declare_dram_parameter("sj", [128, NSUP * 128], bf16, isOutput=False)
    i128_d = nc.declare_dram_parameter("i128", [128, 128], bf16, isOutput=False)
    # stream-term lhsT rows banked at 32-partition strides (4 token-chunks)
    # to keep the per-partition footprint small for the DMA
    l5_d = nc.declare_dram_parameter("l5", [RPC, 128, T // 4], bf16, isOutput=False)
    w5_d = nc.declare_dram_parameter("w5", [128, E], bf16, isOutput=False)
    out = nc.declare_dram_parameter("out", [RPC, T, E], f32, isOutput=True)

    with tile.TileContext(nc) as tc, ExitStack() as ctx:
        const = ctx.enter_context(tc.tile_pool(name="const", bufs=1))
        pe_pool = ctx.enter_context(tc.tile_pool(name="pe", bufs=2))
        out_pool = ctx.enter_context(tc.tile_pool(name="outp", bufs=3))

        sj_sb = const.tile([128, NSUP * 128], bf16, tag="sj")
        nc.sync.dma_start(sj_sb[:], sj_d[:])
        i128_sb = const.tile([128, 128], bf16, tag="i128")
        nc.sync.dma_start(i128_sb[:], i128_d[:])
        w5_sb = const.tile([128, E], bf16, tag="w5")
        nc.sync.dma_start(w5_sb[:], w5_d[:])
        l5_sb, egh_keep, egl_keep = [], [], []
        for r in range(RPC):
            l5t = const.tile([128, T // 4], bf16, tag=f"l5_{r}")
            nc.sync.dma_start(l5t[:], l5_d[r])
            l5_sb.append(l5t)
            egh_keep.append(
                const.tile([128, NSUP, E], bf16, tag=f"egh{r}", name=f"egh{r}")
            )
            egl_keep.append(
                const.tile([128, NSUP, E], bf16, tag=f"egl{r}", name=f"egl{r}")
            )

        # ---- phase A: encG = enc @ (I+w_pos) as 3-term bf16-split matmuls;
        # result kept in SBUF as bf16 hi/lo.  psum layout [128p, m, e] ==
        # keep layout: row m*128+p at (partition p, block m).
        gh0 = const.tile([128, E], bf16, tag="gh0", name="gh0")
        gh1 = const.tile([128, E], bf16, tag="gh1", name="gh1")
        gl0 = const.tile([128, E], bf16, tag="gl0", name="gl0")
        gl1 = const.tile([128, E], bf16, tag="gl1", name="gl1")
        nc.sync.dma_start(gh0[:], g_hi[0:128, :])
        nc.sync.dma_start(gh1[:], g_hi[128:256, :])
        nc.sync.dma_start(gl0[:], g_lo[0:128, :])
        nc.sync.dma_start(gl1[:], g_lo[128:256, :])
        psum_a = ctx.enter_context(tc.tile_pool(name="psumA", bufs=1, space="PSUM"))
        psum_b = ctx.enter_context(tc.tile_pool(name="psumB", bufs=4, space="PSUM"))
        with tc.tile_pool(name="encT", bufs=2) as encT_pool:
            for r in range(RPC):
                eh0 = encT_pool.tile([128, P], bf16, tag="eh0")
                eh1 = encT_pool.tile([128, P], bf16, tag="eh1")
                el0 = encT_pool.tile([128, P], bf16, tag="el0")
                el1 = encT_pool.tile([128, P], bf16, tag="el1")
                nc.sync.dma_start(eh0[:], enc_hi[r, 0:128, :])
                nc.sync.dma_start(eh1[:], enc_hi[r, 128:256, :])
                nc.sync.dma_start(el0[:], enc_lo[r, 0:128, :])
                nc.sync.dma_start(el1[:], enc_lo[r, 128:256, :])
                ps = psum_a.tile([128, 8 * E], f32, tag="psA")
                for m in range(8):
                    sl = slice(m * 128, (m + 1) * 128)
                    terms = [
                        (eh0, gh0, True, False), (eh1, gh1, False, False),
                        (el0, gh0, False, False), (el1, gh1, False, False),
                        (eh0, gl0, False, False), (eh1, gl1, False, True),
                    ]
                    for lt, gt_, st, sp in terms:
                        nc.tensor.matmul(
                            ps[:, m * E:(m + 1) * E],
                            lhsT=lt[:, sl], rhs=gt_[:], start=st, stop=sp,
                        )
                hi = egh_keep[r][:].rearrange("p m e -> p (m e)")
                lo = egl_keep[r][:].rearrange("p m e -> p (m e)")
                nc.vector.tensor_copy(hi, ps[:])
                nc.vector.scalar_tensor_tensor(
                    out=lo, in0=hi, scalar=-1.0, in1=ps[:], op0=mult, op1=add
                )

        # ---- phase B: per group, 5 bf16 matmuls into one PSUM group:
        # expansion hi/lo + pe hi/lo + the K=5 rank-1 stream matmul
        # (pitch_hi/lo x w_pitch_hi/lo cross terms + beats x demb_hi/lo).
        if True:
            for s in range(T // (NSUP * 128)):
                peh = pe_pool.tile([128, NSUP, E], bf16, tag="peh")
                pel = pe_pool.tile([128, NSUP, E], bf16, tag="pel")
                nc.sync.dma_start(peh[:], pe_hi[:, s * NSUP:(s + 1) * NSUP, :])
                nc.sync.dma_start(pel[:], pe_lo[:, s * NSUP:(s + 1) * NSUP, :])
                for r in range(RPC):
                    ot = out_pool.tile([128, NSUP, E], f32, tag="ot")
                    for j in range(NSUP):
                        g = s * NSUP + j
                        ps = psum_b.tile([128, E], f32, tag="ps")
                        sj_ap = sj_sb[:, j * 128:(j + 1) * 128]
                        nc.tensor.matmul(
                            ps[:], lhsT=sj_ap, rhs=egh_keep[r][:, s, :],
                            start=True, stop=False,
                        )
                        nc.tensor.matmul(
                            ps[:], lhsT=sj_ap, rhs=egl_keep[r][:, s, :],
                            start=False, stop=False,
                        )
                        nc.tensor.matmul(
                            ps[:], lhsT=i128_sb[:], rhs=peh[:, j, :],
                            start=False, stop=False,
                        )
                        nc.tensor.matmul(
                            ps[:], lhsT=i128_sb[:], rhs=pel[:, j, :],
                            start=False, stop=False,
                        )
                        cb = g // (NGRP // 4)          # token-chunk bank
                        u0 = (g % (NGRP // 4)) * 128
                        nc.tensor.matmul(
                            ps[:],
                            lhsT=l5_sb[r][32 * cb:32 * cb + 5, u0:u0 + 128],
                            rhs=w5_sb[32 * cb:32 * cb + 5, :],
                            start=False, stop=True,
                            tile_position=(32 * cb, 0),
                        )
                        nc.vector.tensor_copy(ot[:, j, :], ps[:])
                    nc.sync.dma_start(
                        out[r, s * NSUP * 128:(s + 1) * NSUP * 128, :].rearrange(
                            "(n p) e -> p n e", p=128
                        ),
                        ot[:],
                    )
    nc.compile()
    return nc


def build_nc_general():
    """Arbitrary-idx path: per-128-token indirect row gathers."""
    import concourse.bass as bass
    from contextlib import ExitStack
    import concourse.tile as tile
    from concourse import bacc, mybir
    from concourse._compat import get_trn_type

    f32 = mybir.dt.float32
    i32 = mybir.dt.int32

    nc = bacc.Bacc(get_trn_type() or "TRN2", target_bir_lowering=False, debug=False)
    enc_t = nc.declare_dram_parameter("enc_t", [RPC, E, P], f32, isOutput=False)
    g_mat = nc.declare_dram_parameter("g_mat", [E, E], f32, isOutput=False)
    pe_w = nc.declare_dram_parameter("pe_w", [128, NGRP, E], f32, isOutput=False)
    p3 = nc.declare_dram_parameter("p3", [RPC, 3, T], f32, isOutput=False)
    w3 = nc.declare_dram_parameter("w3", [3, E], f32, isOutput=False)
    idxo = nc.declare_dram_parameter(
        "idxo", [RPC, 128, NGRP], i32, isOutput=False
    )
    out = nc.declare_dram_parameter("out", [RPC, T, E], f32, isOutput=True)
    encg = nc.dram_tensor("encg", [RPC, P, E], f32)

    with tile.TileContext(nc) as tc, ExitStack() as ctx:
        const = ctx.enter_context(tc.tile_pool(name="const", bufs=1))
        encT_pool = ctx.enter_context(tc.tile_pool(name="encT", bufs=2))
        psum_pool = ctx.enter_context(tc.tile_pool(name="psum", bufs=2, space="PSUM"))
        eg_pool = ctx.enter_context(tc.tile_pool(name="eg", bufs=2))
        pe_pool = ctx.enter_context(tc.tile_pool(name="pe", bufs=2))
        gath_pool = ctx.enter_context(tc.tile_pool(name="gath", bufs=3))

        g0 = const.tile([128, E], f32, tag="g0")
        g1 = const.tile([128, E], f32, tag="g1")
        nc.sync.dma_start(g0[:], g_mat[0:128, :])
        nc.sync.dma_start(g1[:], g_mat[128:256, :])
        w3_sb = const.tile([3, E], f32, tag="w3")
        nc.sync.dma_start(w3_sb[:], w3[:, :])
        p3_sb = []
        ixo_sb = []
        for r in range(RPC):
            p3t = const.tile([3, T], f32, tag=f"p3_{r}")
            nc.sync.dma_start(p3t[:], p3[r])
            p3_sb.append(p3t)
            ixt = const.tile([128, NGRP], i32, tag=f"ixo_{r}")
            nc.sync.dma_start(ixt[:], idxo[r])
            ixo_sb.append(ixt)

        for r in range(RPC):
            et0 = encT_pool.tile([128, P], f32, tag="et0")
            et1 = encT_pool.tile([128, P], f32, tag="et1")
            nc.sync.dma_start(et0[:], enc_t[r, 0:128, :])
            nc.sync.dma_start(et1[:], enc_t[r, 128:256, :])
            ps = psum_pool.tile([128, 8 * E], f32, tag="ps")
            for m in range(8):
                nc.tensor.matmul(
                    ps[:, m * E:(m + 1) * E],
                    lhsT=et0[:, m * 128:(m + 1) * 128],
                    rhs=g0[:], start=True, stop=False,
                )
                nc.tensor.matmul(
                    ps[:, m * E:(m + 1) * E],
                    lhsT=et1[:, m * 128:(m + 1) * 128],
                    rhs=g1[:], start=False, stop=True,
                )
            eg = eg_pool.tile([128, 8 * E], f32, tag="eg")
            nc.vector.tensor_copy(eg[:], ps[:])
            nc.sync.dma_start(
                encg[r].rearrange("(m p) e -> p m e", p=128),
                eg[:].rearrange("q (m e) -> q m e", e=E),
            )

        encg_flat = encg[:].rearrange("r p e -> (r p) e")
        for s in range(T // (NSUP * 128)):
            pe_t = pe_pool.tile([128, NSUP, E], f32, tag="pe")
            nc.sync.dma_start(pe_t[:], pe_w[:, s * NSUP:(s + 1) * NSUP, :])
            for r in range(RPC):
                gt = gath_pool.tile([128, NSUP, E], f32, tag="gt")
                for g in range(NSUP):
                    gi = s * NSUP + g
                    nc.gpsimd.indirect_dma_start(
                        out=gt[:, g, :],
                        out_offset=None,
                        in_=encg_flat,
                        in_offset=bass.IndirectOffsetOnAxis(
                            ap=ixo_sb[r][:, gi:gi + 1], axis=0
                        ),
                    )
                nc.vector.tensor_add(gt[:], gt[:], pe_t[:])
                ps = psum_pool.tile([128, 8 * E], f32, tag="ps")
                for g in range(NSUP):
                    gi = s * NSUP + g
                    nc.tensor.matmul(
                        ps[:, g * E:(g + 1) * E],
                        lhsT=p3_sb[r][:, gi * 128:(gi + 1) * 128],
                        rhs=w3_sb[:],
                        start=True, stop=True,
                    )
                nc.vector.tensor_add(
                    gt[:], gt[:], ps[:].rearrange("q (n e) -> q n e", e=E)
                )
                nc.sync.dma_start(
                    out[r, s * NSUP * 128:(s + 1) * NSUP * 128, :].rearrange(
                        "(n p) e -> p n e", p=128
                    ),
                    gt[:],
                )
    nc.compile()
    return nc


def get_nc(fast):
    key = "nc_fast" if fast else "nc_gen"
    if key not in _CACHE:
        _CACHE[key] = build_nc_fast() if fast else build_nc_general()
    return _CACHE[key]


# --------------------------------------------------------------------------
# Host wrapper
# --------------------------------------------------------------------------
def make_in_maps(encoder_out, align_phone, text_phone, pitch, beats,
                 w_pitch, b_pitch, emb_beats, w_pos, b_pos):
    import ml_dtypes

    encoder_out = np.asarray(encoder_out, np.float32)
    pitch = np.asarray(pitch, np.float32)
    beats = np.asarray(beats)
    w_pitch = np.asarray(w_pitch, np.float32)
    w_pos = np.asarray(w_pos, np.float32)

    idx = compute_idx(np.asarray(align_phone), np.asarray(text_phone))
    fast = bool(np.all(idx == (np.arange(T, dtype=np.int32) // DUR)[None, :]))
    if FORCE_GENERAL:
        fast = False

    g_mat = (np.eye(E, dtype=np.float64) + w_pos.astype(np.float64)).astype(np.float32)
    pe = _positional_encoding_f64(T, E)
    pe_proj = pe @ w_pos.astype(np.float64)                          # [T, E]
    bias = (np.asarray(emb_beats[0], np.float64)
            + np.asarray(b_pitch, np.float64)
            + np.asarray(b_pos, np.float64))
    demb = (np.asarray(emb_beats[1], np.float64)
            - np.asarray(emb_beats[0], np.float64)).astype(np.float32)

    if fast:
        pe_tot = (pe_proj + bias[None, :]).astype(np.float32)
        pe_wrap = np.ascontiguousarray(pe_tot.reshape(NGRP, 128, E).swapaxes(0, 1))
        pe_hi, pe_lo = _bf16_split(pe_wrap)
        # S_j[k, t'] = 1 iff k == j*16 + t'//8
        rows = np.arange(128)[:, None]
        sj = np.concatenate(
            [(rows == (j * NW + np.arange(128) // DUR)[None, :]) for j in range(NSUP)],
            axis=1,
        ).astype(ml_dtypes.bfloat16)
        i128 = np.eye(128, dtype=ml_dtypes.bfloat16)
        g_hi, g_lo = _bf16_split(g_mat)
        wp_hi, wp_lo = _bf16_split(w_pitch[0])
        db_hi, db_lo = _bf16_split(demb)
        w5_rows = np.stack([
            wp_hi, wp_lo, wp_hi,
            db_hi, db_lo,
        ]).astype(ml_dtypes.bfloat16)
        w5 = np.zeros((128, E), ml_dtypes.bfloat16)
        for cb in range(4):
            w5[32 * cb:32 * cb + 5] = w5_rows
        fast_common = {
            "pe_hi": pe_hi, "pe_lo": pe_lo, "sj": sj, "i128": i128,
            "g_hi": g_hi, "g_lo": g_lo, "w5": w5,
        }
    else:
        w3 = np.stack(
            [w_pitch[0].astype(np.float64), demb.astype(np.float64), bias]
        ).astype(np.float32)
        pe_wl = np.ascontiguousarray(
            pe_proj.astype(np.float32).reshape(NGRP, 128, E).swapaxes(0, 1)
        )

    in_maps = []
    for core in range(NCORES):
        rows_ = range(core * RPC, (core + 1) * RPC)
        enc_t = np.ascontiguousarray(
            encoder_out[core * RPC:(core + 1) * RPC].transpose(0, 2, 1)
        )
        if fast:
            import ml_dtypes as _md
            enc_hi, enc_lo = _bf16_split(enc_t)
            l5 = np.zeros((RPC, 128, T // 4), _md.bfloat16)
            for j, b in enumerate(rows_):
                p_hi, p_lo = _bf16_split(pitch[b, :, 0])
                bt = beats[b, :, 0].astype(_md.bfloat16)
                rows5 = np.stack([p_hi, p_hi, p_lo, bt, bt])     # [5, T]
                for cb in range(4):
                    l5[j, 32 * cb:32 * cb + 5] = rows5[
                        :, cb * (T // 4):(cb + 1) * (T // 4)
                    ]
            m = {"enc_hi": enc_hi, "enc_lo": enc_lo, "l5": l5, **fast_common}
        else:
            p3 = np.empty((RPC, 3, T), np.float32)
            idxo = np.empty((RPC, 128, NGRP), np.int32)
            for j, b in enumerate(rows_):
                p3[j, 0] = pitch[b, :, 0]
                p3[j, 1] = beats[b, :, 0].astype(np.float32)
                p3[j, 2] = 1.0
                idxo[j] = idx[b].reshape(NGRP, 128).T + j * P
            m = {"enc_t": enc_t, "g_mat": g_mat, "pe_w": pe_wl, "p3": p3,
                 "w3": w3, "idxo": idxo}
        in_maps.append(m)
    return fast, in_maps


def kernel(**inputs):
    from concourse.bass_utils import run_bass_kernel_spmd

    fast, in_maps = make_in_maps(**inputs)
    nc = get_nc(fast)
    res = run_bass_kernel_spmd(nc, in_maps, core_ids=list(range(NCORES)))
    out = np.concatenate([res.results[i]["out"] for i in range(NCORES)], axis=0)
    return np.ascontiguousarray(out.astype(np.float32))


# revision 32
# speedup vs baseline: 2.0026x; 1.0560x over previous
"""Trainium2 Bass kernel for nn_Encoder_Postnet (ragged_sequence).

Computation (reference):
    idx   = sequential aligner scan over (align_phone, text_phone)   [B,T]
    out   = enc[idx] + pitch @ w_pitch + b_pitch + emb_beats[beats]
            + (enc[idx] + pe) @ w_pos + b_pos

Key algebraic restructure: the duration-expansion gather commutes with the
E x E linear, so
    out[t] = encG[idx_t] + (pe@w_pos + bias)[t] + pitch[t]*w_pitch + beats[t]*demb
with encG = enc @ (I + w_pos) computed once per batch row ([P,E] not [T,E]),
collapsing the big [B*T,E]@[E,E] matmul 8x and making the kernel memory-bound.

Sharding: pure data parallel, 2 batch rows per core across 8 cores.

Fast path (the uniform duration-8 expansion this model's inputs produce,
idx == arange(T)//8 for every row):
  phase A: encG = enc_row @ (I+w_pos) on PE (fp32); the result stays resident
           in SBUF split into bf16 hi/lo pairs (hi+lo keeps ~17 mantissa
           bits, and bf16 matmuls run 4x faster than fp32 on the PE).
  phase B: per 128-token group, one PSUM accumulation group of 4 bf16
           matmuls: S_j-one-hot expansion of encG rows (hi+lo) + identity
           matmuls adding the (pe@w_pos + bias) tile (hi+lo).  Then the
           pitch rank-1 term rides the DVE op that drains PSUM->SBUF
           (scalar_tensor_tensor), and the beats term runs on the otherwise
           idle GPSIMD.  The only DMA traffic is inputs-once + outputs-once.

General path (arbitrary idx): per-128-token indirect-DMA row gathers
(production-shaped offset [128,1] DynamicAP descriptors) + K=3 stream matmul.

The aligner scan itself is index metadata ([B,T] int32); it is computed on
host with a run-compressed O(B*P) algorithm exactly equivalent to the
reference recurrence, then consumed either as a uniformity proof (fast path)
or as gather offsets (general path).
"""

import sys

for _p in ("/opt/trn_rl_repo",):
    if _p not in sys.path:
        sys.path.insert(0, _p)

import numpy as np

B, P, T, E = 16, 1024, 8192, 256
NCORES = 8
RPC = B // NCORES          # batch rows per core
NGRP = T // 128            # 64 groups of 128 tokens per row
NSUP = 8                   # groups per super-chunk
DUR = T // P               # uniform duration of the fast path (8)
NW = 128 // DUR            # encG rows per group (16)

FORCE_GENERAL = False      # test hook: force the arbitrary-idx path
_CACHE = {}


# --------------------------------------------------------------------------
# Host: aligner index computation (exact replica of the reference recurrence)
# --------------------------------------------------------------------------
def compute_idx(align, text):
    """idx[b,0]=0; idx[b,j] = idx[b,j-1] if align[b,j]==text[b,idx[b,j-1]]
    else min(idx[b,j-1]+1, P-1).   Vectorized over batch via segment starts:
    the pointer advances i->i+1 at s_{i+1} = first j >= s_i+1 with
    align[j] != text[i]; within a run of align values equal to text[i] the
    first mismatch is the run end."""
    align = np.asarray(align)
    text = np.asarray(text)
    Bn, Tn = align.shape
    Pn = text.shape[1]
    diff = align[:, 1:] != align[:, :-1]                       # [B, T-1]
    c = np.full((Bn, Tn), Tn, np.int64)
    c[:, :-1] = np.where(diff, np.arange(1, Tn)[None, :], Tn)
    re = np.flip(np.minimum.accumulate(np.flip(c, axis=1), axis=1), axis=1)

    s = np.full((Bn, Pn), Tn, np.int64)
    s[:, 0] = 0
    cur = np.zeros(Bn, np.int64)
    arB = np.arange(Bn)
    for i in range(Pn - 1):
        j0 = cur + 1
        active = j0 < Tn
        j0c = np.minimum(j0, Tn - 1)
        eq = (align[arB, j0c] == text[:, i]) & active
        nxt = np.where(active, np.where(eq, re[arB, j0c], j0), Tn)
        s[:, i + 1] = nxt
        cur = nxt
    idx = np.empty((Bn, Tn), np.int32)
    pos = np.arange(Tn)
    for b in range(Bn):
        idx[b] = (np.searchsorted(s[b], pos, side="right") - 1).astype(np.int32)
    return idx


def _positional_encoding_f64(t, e):
    pos = np.arange(t, dtype=np.float64)[:, None]
    div = np.exp(np.arange(0, e, 2, dtype=np.float64) * (-np.log(10000.0) / e))
    ang = pos * div[None, :]
    return np.stack([np.sin(ang), np.cos(ang)], axis=-1).reshape(t, e)


def _bf16_split(x):
    import ml_dtypes
    x = np.asarray(x, np.float32)
    hi = x.astype(ml_dtypes.bfloat16)
    lo = (x - hi.astype(np.float32)).astype(ml_dtypes.bfloat16)
    return hi, lo


# --------------------------------------------------------------------------
# Device programs
# --------------------------------------------------------------------------
def build_nc_fast():
    from contextlib import ExitStack
    import concourse.tile as tile
    from concourse import bacc, mybir
    from concourse._compat import get_trn_type

    f32 = mybir.dt.float32
    bf16 = mybir.dt.bfloat16
    mult = mybir.AluOpType.mult
    add = mybir.AluOpType.add

    nc = bacc.Bacc(get_trn_type() or "TRN2", target_bir_lowering=False, debug=False)
    enc_hi = nc.declare_dram_parameter("enc_hi", [RPC, E, P], bf16, isOutput=False)
    enc_lo = nc.declare_dram_parameter("enc_lo", [RPC, E, P], bf16, isOutput=False)
    g_hi = nc.declare_dram_parameter("g_hi", [E, E], bf16, isOutput=False)
    g_lo = nc.declare_dram_parameter("g_lo", [E, E], bf16, isOutput=False)
    pe_hi = nc.declare_dram_parameter("pe_hi", [128, NGRP, E], bf16, isOutput=False)
    pe_lo = nc.declare_dram_parameter("pe_lo", [128, NGRP, E], bf16, isOutput=False)
    sj_d = nc.declare_dram_parameter("sj", [128, NSUP * 128], bf16, isOutput=False)
    i128_d = nc.declare_dram_parameter("i128", [128, 128], bf16, isOutput=False)
    # stream-term lhsT rows banked at 32-partition strides (4 token-chunks)
    # to keep the per-partition footprint small for the DMA
    l5_d = nc.declare_dram_parameter("l5", [RPC, 128, T // 4], bf16, isOutput=False)
    w5_d = nc.declare_dram_parameter("w5", [128, E], bf16, isOutput=False)
    out = nc.declare_dram_parameter("out", [RPC, T, E], f32, isOutput=True)

    with tile.TileContext(nc) as tc, ExitStack() as ctx:
        const = ctx.enter_context(tc.tile_pool(name="const", bufs=1))
        pe_pool = ctx.enter_context(tc.tile_pool(name="pe", bufs=8))
        out_pool = ctx.enter_context(tc.tile_pool(name="outp", bufs=4))

        sj_sb = const.tile([128, NSUP * 128], bf16, tag="sj")
        nc.sync.dma_start(sj_sb[:], sj_d[:])
        i128_sb = const.tile([128, 128], bf16, tag="i128")
        nc.sync.dma_start(i128_sb[:], i128_d[:])
        w5_sb = const.tile([128, E], bf16, tag="w5")
        nc.sync.dma_start(w5_sb[:], w5_d[:])
        l5_sb, egh_keep, egl_keep = [], [], []
        for r in range(RPC):
            l5t = const.tile([128, T // 4], bf16, tag=f"l5_{r}")
            nc.sync.dma_start(l5t[:], l5_d[r])
            l5_sb.append(l5t)
            egh_keep.append(
                const.tile([128, NSUP, E], bf16, tag=f"egh{r}", name=f"egh{r}")
            )
            egl_keep.append(
                const.tile([128, NSUP, E], bf16, tag=f"egl{r}", name=f"egl{r}")
            )

        # ---- phase A: encG = enc @ (I+w_pos) as 3-term bf16-split matmuls;
        # result kept in SBUF as bf16 hi/lo.  psum layout [128p, m, e] ==
        # keep layout: row m*128+p at (partition p, block m).
        gh0 = const.tile([128, E], bf16, tag="gh0", name="gh0")
        gh1 = const.tile([128, E], bf16, tag="gh1", name="gh1")
        gl0 = const.tile([128, E], bf16, tag="gl0", name="gl0")
        gl1 = const.tile([128, E], bf16, tag="gl1", name="gl1")
        nc.sync.dma_start(gh0[:], g_hi[0:128, :])
        nc.sync.dma_start(gh1[:], g_hi[128:256, :])
        nc.sync.dma_start(gl0[:], g_lo[0:128, :])
        nc.sync.dma_start(gl1[:], g_lo[128:256, :])
        psum_a = ctx.enter_context(tc.tile_pool(name="psumA", bufs=1, space="PSUM"))
        psum_b = ctx.enter_context(tc.tile_pool(name="psumB", bufs=4, space="PSUM"))
        with tc.tile_pool(name="encT", bufs=2) as encT_pool:
            for r in range(RPC):
                eh0 = encT_pool.tile([128, P], bf16, tag="eh0")
                eh1 = encT_pool.tile([128, P], bf16, tag="eh1")
                el0 = encT_pool.tile([128, P], bf16, tag="el0")
                el1 = encT_pool.tile([128, P], bf16, tag="el1")
                nc.sync.dma_start(eh0[:], enc_hi[r, 0:128, :])
                nc.sync.dma_start(eh1[:], enc_hi[r, 128:256, :])
                nc.sync.dma_start(el0[:], enc_lo[r, 0:128, :])
                nc.sync.dma_start(el1[:], enc_lo[r, 128:256, :])
                ps = psum_a.tile([128, 8 * E], f32, tag="psA")
                for m in range(8):
                    sl = slice(m * 128, (m + 1) * 128)
                    terms = [
                        (eh0, gh0, True, False), (eh1, gh1, False, False),
                        (el0, gh0, False, False), (el1, gh1, False, False),
                        (eh0, gl0, False, False), (eh1, gl1, False, True),
                    ]
                    for lt, gt_, st, sp in terms:
                        nc.tensor.matmul(
                            ps[:, m * E:(m + 1) * E],
                            lhsT=lt[:, sl], rhs=gt_[:], start=st, stop=sp,
                        )
                hi = egh_keep[r][:].rearrange("p m e -> p (m e)")
                lo = egl_keep[r][:].rearrange("p m e -> p (m e)")
                nc.vector.tensor_copy(hi, ps[:])
                nc.vector.scalar_tensor_tensor(
                    out=lo, in0=hi, scalar=-1.0, in1=ps[:], op0=mult, op1=add
                )

        # ---- phase B: per group, 5 bf16 matmuls into one PSUM group:
        # expansion hi/lo + pe hi/lo + the K=5 rank-1 stream matmul
        # (pitch_hi/lo x w_pitch_hi/lo cross terms + beats x demb_hi/lo).
        if True:
            for s in range(T // (NSUP * 128)):
                peh = pe_pool.tile([128, NSUP, E], bf16, tag="peh")
                pel = pe_pool.tile([128, NSUP, E], bf16, tag="pel")
                nc.sync.dma_start(peh[:], pe_hi[:, s * NSUP:(s + 1) * NSUP, :])
                nc.sync.dma_start(pel[:], pe_lo[:, s * NSUP:(s + 1) * NSUP, :])
                for r in range(RPC):
                    ot = out_pool.tile([128, NSUP, E], f32, tag="ot")
                    for j in range(NSUP):
                        g = s * NSUP + j
                        ps = psum_b.tile([128, E], f32, tag="ps")
                        sj_ap = sj_sb[:, j * 128:(j + 1) * 128]
                        nc.tensor.matmul(
                            ps[:], lhsT=sj_ap, rhs=egh_keep[r][:, s, :],
                            start=True, stop=False,
                        )
                        nc.tensor.matmul(
                            ps[:], lhsT=sj_ap, rhs=egl_keep[r][:, s, :],
                            start=False, stop=False,
                        )
                        nc.tensor.matmul(
                            ps[:], lhsT=i128_sb[:], rhs=peh[:, j, :],
                            start=False, stop=False,
                        )
                        nc.tensor.matmul(
                            ps[:], lhsT=i128_sb[:], rhs=pel[:, j, :],
                            start=False, stop=False,
                        )
                        cb = g // (NGRP // 4)          # token-chunk bank
                        u0 = (g % (NGRP // 4)) * 128
                        nc.tensor.matmul(
                            ps[:],
                            lhsT=l5_sb[r][32 * cb:32 * cb + 5, u0:u0 + 128],
                            rhs=w5_sb[32 * cb:32 * cb + 5, :],
                            start=False, stop=True,
                            tile_position=(32 * cb, 0),
                        )
                        nc.vector.tensor_copy(ot[:, j, :], ps[:])
                    nc.sync.dma_start(
                        out[r, s * NSUP * 128:(s + 1) * NSUP * 128, :].rearrange(
                            "(n p) e -> p n e", p=128
                        ),
                        ot[:],
                    )
    nc.compile()
    return nc


def build_nc_general():
    """Arbitrary-idx path: per-128-token indirect row gathers."""
    import concourse.bass as bass
    from contextlib import ExitStack
    import concourse.tile as tile
    from concourse import bacc, mybir
    from concourse._compat import get_trn_type

    f32 = mybir.dt.float32
    i32 = mybir.dt.int32

    nc = bacc.Bacc(get_trn_type() or "TRN2", target_bir_lowering=False, debug=False)
    enc_t = nc.declare_dram_parameter("enc_t", [RPC, E, P], f32, isOutput=False)
    g_mat = nc.declare_dram_parameter("g_mat", [E, E], f32, isOutput=False)
    pe_w = nc.declare_dram_parameter("pe_w", [128, NGRP, E], f32, isOutput=False)
    p3 = nc.declare_dram_parameter("p3", [RPC, 3, T], f32, isOutput=False)
    w3 = nc.declare_dram_parameter("w3", [3, E], f32, isOutput=False)
    idxo = nc.declare_dram_parameter(
        "idxo", [RPC, 128, NGRP], i32, isOutput=False
    )
    out = nc.declare_dram_parameter("out", [RPC, T, E], f32, isOutput=True)
    encg = nc.dram_tensor("encg", [RPC, P, E], f32)

    with tile.TileContext(nc) as tc, ExitStack() as ctx:
        const = ctx.enter_context(tc.tile_pool(name="const", bufs=1))
        encT_pool = ctx.enter_context(tc.tile_pool(name="encT", bufs=2))
        psum_pool = ctx.enter_context(tc.tile_pool(name="psum", bufs=2, space="PSUM"))
        eg_pool = ctx.enter_context(tc.tile_pool(name="eg", bufs=2))
        pe_pool = ctx.enter_context(tc.tile_pool(name="pe", bufs=2))
        gath_pool = ctx.enter_context(tc.tile_pool(name="gath", bufs=3))

        g0 = const.tile([128, E], f32, tag="g0")
        g1 = const.tile([128, E], f32, tag="g1")
        nc.sync.dma_start(g0[:], g_mat[0:128, :])
        nc.sync.dma_start(g1[:], g_mat[128:256, :])
        w3_sb = const.tile([3, E], f32, tag="w3")
        nc.sync.dma_start(w3_sb[:], w3[:, :])
        p3_sb = []
        ixo_sb = []
        for r in range(RPC):
            p3t = const.tile([3, T], f32, tag=f"p3_{r}")
            nc.sync.dma_start(p3t[:], p3[r])
            p3_sb.append(p3t)
            ixt = const.tile([128, NGRP], i32, tag=f"ixo_{r}")
            nc.sync.dma_start(ixt[:], idxo[r])
            ixo_sb.append(ixt)

        for r in range(RPC):
            et0 = encT_pool.tile([128, P], f32, tag="et0")
            et1 = encT_pool.tile([128, P], f32, tag="et1")
            nc.sync.dma_start(et0[:], enc_t[r, 0:128, :])
            nc.sync.dma_start(et1[:], enc_t[r, 128:256, :])
            ps = psum_pool.tile([128, 8 * E], f32, tag="ps")
            for m in range(8):
                nc.tensor.matmul(
                    ps[:, m * E:(m + 1) * E],
                    lhsT=et0[:, m * 128:(m + 1) * 128],
                    rhs=g0[:], start=True, stop=False,
                )
                nc.tensor.matmul(
                    ps[:, m * E:(m + 1) * E],
                    lhsT=et1[:, m * 128:(m + 1) * 128],
                    rhs=g1[:], start=False, stop=True,
                )
            eg = eg_pool.tile([128, 8 * E], f32, tag="eg")
            nc.vector.tensor_copy(eg[:], ps[:])
            nc.sync.dma_start(
                encg[r].rearrange("(m p) e -> p m e", p=128),
                eg[:].rearrange("q (m e) -> q m e", e=E),
            )

        encg_flat = encg[:].rearrange("r p e -> (r p) e")
        for s in range(T // (NSUP * 128)):
            pe_t = pe_pool.tile([128, NSUP, E], f32, tag="pe")
            nc.sync.dma_start(pe_t[:], pe_w[:, s * NSUP:(s + 1) * NSUP, :])
            for r in range(RPC):
                gt = gath_pool.tile([128, NSUP, E], f32, tag="gt")
                for g in range(NSUP):
                    gi = s * NSUP + g
                    nc.gpsimd.indirect_dma_start(
                        out=gt[:, g, :],
                        out_offset=None,
                        in_=encg_flat,
                        in_offset=bass.IndirectOffsetOnAxis(
                            ap=ixo_sb[r][:, gi:gi + 1], axis=0
                        ),
                    )
                nc.vector.tensor_add(gt[:], gt[:], pe_t[:])
                ps = psum_pool.tile([128, 8 * E], f32, tag="ps")
                for g in range(NSUP):
                    gi = s * NSUP + g
                    nc.tensor.matmul(
                        ps[:, g * E:(g + 1) * E],
                        lhsT=p3_sb[r][:, gi * 128:(gi + 1) * 128],
                        rhs=w3_sb[:],
                        start=True, stop=True,
                    )
                nc.vector.tensor_add(
                    gt[:], gt[:], ps[:].rearrange("q (n e) -> q n e", e=E)
                )
                nc.sync.dma_start(
                    out[r, s * NSUP * 128:(s + 1) * NSUP * 128, :].rearrange(
                        "(n p) e -> p n e", p=128
                    ),
                    gt[:],
                )
    nc.compile()
    return nc


def get_nc(fast):
    key = "nc_fast" if fast else "nc_gen"
    if key not in _CACHE:
        _CACHE[key] = build_nc_fast() if fast else build_nc_general()
    return _CACHE[key]


# --------------------------------------------------------------------------
# Host wrapper
# --------------------------------------------------------------------------
def make_in_maps(encoder_out, align_phone, text_phone, pitch, beats,
                 w_pitch, b_pitch, emb_beats, w_pos, b_pos):
    import ml_dtypes

    encoder_out = np.asarray(encoder_out, np.float32)
    pitch = np.asarray(pitch, np.float32)
    beats = np.asarray(beats)
    w_pitch = np.asarray(w_pitch, np.float32)
    w_pos = np.asarray(w_pos, np.float32)

    idx = compute_idx(np.asarray(align_phone), np.asarray(text_phone))
    fast = bool(np.all(idx == (np.arange(T, dtype=np.int32) // DUR)[None, :]))
    if FORCE_GENERAL:
        fast = False

    g_mat = (np.eye(E, dtype=np.float64) + w_pos.astype(np.float64)).astype(np.float32)
    pe = _positional_encoding_f64(T, E)
    pe_proj = pe @ w_pos.astype(np.float64)                          # [T, E]
    bias = (np.asarray(emb_beats[0], np.float64)
            + np.asarray(b_pitch, np.float64)
            + np.asarray(b_pos, np.float64))
    demb = (np.asarray(emb_beats[1], np.float64)
            - np.asarray(emb_beats[0], np.float64)).astype(np.float32)

    if fast:
        pe_tot = (pe_proj + bias[None, :]).astype(np.float32)
        pe_wrap = np.ascontiguousarray(pe_tot.reshape(NGRP, 128, E).swapaxes(0, 1))
        pe_hi, pe_lo = _bf16_split(pe_wrap)
        # S_j[k, t'] = 1 iff k == j*16 + t'//8
        rows = np.arange(128)[:, None]
        sj = np.concatenate(
            [(rows == (j * NW + np.arange(128) // DUR)[None, :]) for j in range(NSUP)],
            axis=1,
        ).astype(ml_dtypes.bfloat16)
        i128 = np.eye(128, dtype=ml_dtypes.bfloat16)
        g_hi, g_lo = _bf16_split(g_mat)
        wp_hi, wp_lo = _bf16_split(w_pitch[0])
        db_hi, db_lo = _bf16_split(demb)
        w5_rows = np.stack([
            wp_hi, wp_lo, wp_hi,
            db_hi, db_lo,
        ]).astype(ml_dtypes.bfloat16)
        w5 = np.zeros((128, E), ml_dtypes.bfloat16)
        for cb in range(4):
            w5[32 * cb:32 * cb + 5] = w5_rows
        fast_common = {
            "pe_hi": pe_hi, "pe_lo": pe_lo, "sj": sj, "i128": i128,
            "g_hi": g_hi, "g_lo": g_lo, "w5": w5,
        }
    else:
        w3 = np.stack(
            [w_pitch[0].astype(np.float64), demb.astype(np.float64), bias]
        ).astype(np.float32)
        pe_wl = np.ascontiguousarray(
            pe_proj.astype(np.float32).reshape(NGRP, 128, E).swapaxes(0, 1)
        )

    in_maps = []
    for core in range(NCORES):
        rows_ = range(core * RPC, (core + 1) * RPC)
        enc_t = np.ascontiguousarray(
            encoder_out[core * RPC:(core + 1) * RPC].transpose(0, 2, 1)
        )
        if fast:
            import ml_dtypes as _md
            enc_hi, enc_lo = _bf16_split(enc_t)
            l5 = np.zeros((RPC, 128, T // 4), _md.bfloat16)
            for j, b in enumerate(rows_):
                p_hi, p_lo = _bf16_split(pitch[b, :, 0])
                bt = beats[b, :, 0].astype(_md.bfloat16)
                rows5 = np.stack([p_hi, p_hi, p_lo, bt, bt])     # [5, T]
                for cb in range(4):
                    l5[j, 32 * cb:32 * cb + 5] = rows5[
                        :, cb * (T // 4):(cb + 1) * (T // 4)
                    ]
            m = {"enc_hi": enc_hi, "enc_lo": enc_lo, "l5": l5, **fast_common}
        else:
            p3 = np.empty((RPC, 3, T), np.float32)
            idxo = np.empty((RPC, 128, NGRP), np.int32)
            for j, b in enumerate(rows_):
                p3[j, 0] = pitch[b, :, 0]
                p3[j, 1] = beats[b, :, 0].astype(np.float32)
                p3[j, 2] = 1.0
                idxo[j] = idx[b].reshape(NGRP, 128).T + j * P
            m = {"enc_t": enc_t, "g_mat": g_mat, "pe_w": pe_wl, "p3": p3,
                 "w3": w3, "idxo": idxo}
        in_maps.append(m)
    return fast, in_maps


def kernel(**inputs):
    from concourse.bass_utils import run_bass_kernel_spmd

    fast, in_maps = make_in_maps(**inputs)
    nc = get_nc(fast)
    res = run_bass_kernel_spmd(nc, in_maps, core_ids=list(range(NCORES)))
    out = np.concatenate([res.results[i]["out"] for i in range(NCORES)], axis=0)
    return np.ascontiguousarray(out.astype(np.float32))


# revision 33
# speedup vs baseline: 2.0281x; 1.0127x over previous
"""Trainium2 Bass kernel for nn_Encoder_Postnet (ragged_sequence).

Computation (reference):
    idx   = sequential aligner scan over (align_phone, text_phone)   [B,T]
    out   = enc[idx] + pitch @ w_pitch + b_pitch + emb_beats[beats]
            + (enc[idx] + pe) @ w_pos + b_pos

Key algebraic restructure: the duration-expansion gather commutes with the
E x E linear, so
    out[t] = encG[idx_t] + (pe@w_pos + bias)[t] + pitch[t]*w_pitch + beats[t]*demb
with encG = enc @ (I + w_pos) computed once per batch row ([P,E] not [T,E]),
collapsing the big [B*T,E]@[E,E] matmul 8x and making the kernel memory-bound.

Sharding: pure data parallel, 2 batch rows per core across 8 cores.

Fast path (the uniform duration-8 expansion this model's inputs produce,
idx == arange(T)//8 for every row):
  phase A: encG = enc_row @ (I+w_pos) on PE (fp32); the result stays resident
           in SBUF split into bf16 hi/lo pairs (hi+lo keeps ~17 mantissa
           bits, and bf16 matmuls run 4x faster than fp32 on the PE).
  phase B: per 128-token group, one PSUM accumulation group of 4 bf16
           matmuls: S_j-one-hot expansion of encG rows (hi+lo) + identity
           matmuls adding the (pe@w_pos + bias) tile (hi+lo).  Then the
           pitch rank-1 term rides the DVE op that drains PSUM->SBUF
           (scalar_tensor_tensor), and the beats term runs on the otherwise
           idle GPSIMD.  The only DMA traffic is inputs-once + outputs-once.

General path (arbitrary idx): per-128-token indirect-DMA row gathers
(production-shaped offset [128,1] DynamicAP descriptors) + K=3 stream matmul.

The aligner scan itself is index metadata ([B,T] int32); it is computed on
host with a run-compressed O(B*P) algorithm exactly equivalent to the
reference recurrence, then consumed either as a uniformity proof (fast path)
or as gather offsets (general path).
"""

import sys

for _p in ("/opt/trn_rl_repo",):
    if _p not in sys.path:
        sys.path.insert(0, _p)

import numpy as np

B, P, T, E = 16, 1024, 8192, 256
NCORES = 8
RPC = B // NCORES          # batch rows per core
NGRP = T // 128            # 64 groups of 128 tokens per row
NSUP = 8                   # groups per super-chunk
DUR = T // P               # uniform duration of the fast path (8)
NW = 128 // DUR            # encG rows per group (16)

FORCE_GENERAL = False      # test hook: force the arbitrary-idx path
_CACHE = {}


# --------------------------------------------------------------------------
# Host: aligner index computation (exact replica of the reference recurrence)
# --------------------------------------------------------------------------
def compute_idx(align, text):
    """idx[b,0]=0; idx[b,j] = idx[b,j-1] if align[b,j]==text[b,idx[b,j-1]]
    else min(idx[b,j-1]+1, P-1).   Vectorized over batch via segment starts:
    the pointer advances i->i+1 at s_{i+1} = first j >= s_i+1 with
    align[j] != text[i]; within a run of align values equal to text[i] the
    first mismatch is the run end."""
    align = np.asarray(align)
    text = np.asarray(text)
    Bn, Tn = align.shape
    Pn = text.shape[1]
    diff = align[:, 1:] != align[:, :-1]                       # [B, T-1]
    c = np.full((Bn, Tn), Tn, np.int64)
    c[:, :-1] = np.where(diff, np.arange(1, Tn)[None, :], Tn)
    re = np.flip(np.minimum.accumulate(np.flip(c, axis=1), axis=1), axis=1)

    s = np.full((Bn, Pn), Tn, np.int64)
    s[:, 0] = 0
    cur = np.zeros(Bn, np.int64)
    arB = np.arange(Bn)
    for i in range(Pn - 1):
        j0 = cur + 1
        active = j0 < Tn
        j0c = np.minimum(j0, Tn - 1)
        eq = (align[arB, j0c] == text[:, i]) & active
        nxt = np.where(active, np.where(eq, re[arB, j0c], j0), Tn)
        s[:, i + 1] = nxt
        cur = nxt
    idx = np.empty((Bn, Tn), np.int32)
    pos = np.arange(Tn)
    for b in range(Bn):
        idx[b] = (np.searchsorted(s[b], pos, side="right") - 1).astype(np.int32)
    return idx


def _positional_encoding_f64(t, e):
    pos = np.arange(t, dtype=np.float64)[:, None]
    div = np.exp(np.arange(0, e, 2, dtype=np.float64) * (-np.log(10000.0) / e))
    ang = pos * div[None, :]
    return np.stack([np.sin(ang), np.cos(ang)], axis=-1).reshape(t, e)


def _bf16_split(x):
    import ml_dtypes
    x = np.asarray(x, np.float32)
    hi = x.astype(ml_dtypes.bfloat16)
    lo = (x - hi.astype(np.float32)).astype(ml_dtypes.bfloat16)
    return hi, lo


# --------------------------------------------------------------------------
# Device programs
# --------------------------------------------------------------------------
def build_nc_fast():
    from contextlib import ExitStack
    import concourse.tile as tile
    from concourse import bacc, mybir
    from concourse._compat import get_trn_type

    f32 = mybir.dt.float32
    bf16 = mybir.dt.bfloat16
    mult = mybir.AluOpType.mult
    add = mybir.AluOpType.add

    nc = bacc.Bacc(get_trn_type() or "TRN2", target_bir_lowering=False, debug=False)
    enc_hi = nc.declare_dram_parameter("enc_hi", [RPC, E, P], bf16, isOutput=False)
    enc_lo = nc.declare_dram_parameter("enc_lo", [RPC, E, P], bf16, isOutput=False)
    g_hi = nc.declare_dram_parameter("g_hi", [E, E], bf16, isOutput=False)
    g_lo = nc.declare_dram_parameter("g_lo", [E, E], bf16, isOutput=False)
    pe_hi = nc.declare_dram_parameter("pe_hi", [128, NGRP, E], bf16, isOutput=False)
    pe_lo = nc.declare_dram_parameter("pe_lo", [128, NGRP, E], bf16, isOutput=False)
    sj_d = nc.declare_dram_parameter("sj", [128, NSUP * 128], bf16, isOutput=False)
    i128_d = nc.declare_dram_parameter("i128", [128, 128], bf16, isOutput=False)
    # stream-term lhsT rows banked at 32-partition strides (4 token-chunks)
    # to keep the per-partition footprint small for the DMA
    l5_d = nc.declare_dram_parameter("l5", [RPC, 128, T // 4], bf16, isOutput=False)
    w5_d = nc.declare_dram_parameter("w5", [128, E], bf16, isOutput=False)
    out = nc.declare_dram_parameter("out", [RPC, T, E], f32, isOutput=True)

    with tile.TileContext(nc) as tc, ExitStack() as ctx:
        const = ctx.enter_context(tc.tile_pool(name="const", bufs=1))
        pe_pool = ctx.enter_context(tc.tile_pool(name="pe", bufs=8))
        out_pool = ctx.enter_context(tc.tile_pool(name="outp", bufs=4))

        sj_sb = const.tile([128, NSUP * 128], bf16, tag="sj")
        nc.sync.dma_start(sj_sb[:], sj_d[:])
        i128_sb = const.tile([128, 128], bf16, tag="i128")
        nc.sync.dma_start(i128_sb[:], i128_d[:])
        w5_sb = const.tile([128, E], bf16, tag="w5")
        nc.sync.dma_start(w5_sb[:], w5_d[:])
        l5_sb, egh_keep, egl_keep = [], [], []
        for r in range(RPC):
            l5t = const.tile([128, T // 4], bf16, tag=f"l5_{r}")
            nc.sync.dma_start(l5t[:], l5_d[r])
            l5_sb.append(l5t)
            egh_keep.append(
                const.tile([128, NSUP, E], bf16, tag=f"egh{r}", name=f"egh{r}")
            )
            egl_keep.append(
                const.tile([128, NSUP, E], bf16, tag=f"egl{r}", name=f"egl{r}")
            )

        # ---- phase A: encG = enc @ (I+w_pos) as 3-term bf16-split matmuls;
        # result kept in SBUF as bf16 hi/lo.  psum layout [128p, m, e] ==
        # keep layout: row m*128+p at (partition p, block m).
        gh0 = const.tile([128, E], bf16, tag="gh0", name="gh0")
        gh1 = const.tile([128, E], bf16, tag="gh1", name="gh1")
        gl0 = const.tile([128, E], bf16, tag="gl0", name="gl0")
        gl1 = const.tile([128, E], bf16, tag="gl1", name="gl1")
        nc.sync.dma_start(gh0[:], g_hi[0:128, :])
        nc.sync.dma_start(gh1[:], g_hi[128:256, :])
        nc.sync.dma_start(gl0[:], g_lo[0:128, :])
        nc.sync.dma_start(gl1[:], g_lo[128:256, :])
        psum_a = ctx.enter_context(tc.tile_pool(name="psumA", bufs=1, space="PSUM"))
        psum_b = ctx.enter_context(tc.tile_pool(name="psumB", bufs=4, space="PSUM"))
        with tc.tile_pool(name="encT", bufs=2) as encT_pool:
            for r in range(RPC):
                eh0 = encT_pool.tile([128, P], bf16, tag="eh0")
                eh1 = encT_pool.tile([128, P], bf16, tag="eh1")
                el0 = encT_pool.tile([128, P], bf16, tag="el0")
                el1 = encT_pool.tile([128, P], bf16, tag="el1")
                nc.sync.dma_start(eh0[:], enc_hi[r, 0:128, :])
                nc.sync.dma_start(eh1[:], enc_hi[r, 128:256, :])
                nc.sync.dma_start(el0[:], enc_lo[r, 0:128, :])
                nc.sync.dma_start(el1[:], enc_lo[r, 128:256, :])
                ps = psum_a.tile([128, 8 * E], f32, tag="psA")
                for m in range(8):
                    sl = slice(m * 128, (m + 1) * 128)
                    terms = [
                        (eh0, gh0, True, False), (eh1, gh1, False, False),
                        (el0, gh0, False, False), (el1, gh1, False, False),
                        (eh0, gl0, False, False), (eh1, gl1, False, True),
                    ]
                    for lt, gt_, st, sp in terms:
                        nc.tensor.matmul(
                            ps[:, m * E:(m + 1) * E],
                            lhsT=lt[:, sl], rhs=gt_[:], start=st, stop=sp,
                        )
                    # drain per m-chunk so phase B super-chunk m can start
                    # before the rest of phase A finishes
                    hi = egh_keep[r][:, m, :]
                    nc.vector.tensor_copy(hi, ps[:, m * E:(m + 1) * E])
                    nc.vector.scalar_tensor_tensor(
                        out=egl_keep[r][:, m, :], in0=hi, scalar=-1.0,
                        in1=ps[:, m * E:(m + 1) * E], op0=mult, op1=add,
                    )

        # ---- phase B: per group, 5 bf16 matmuls into one PSUM group:
        # expansion hi/lo + pe hi/lo + the K=5 rank-1 stream matmul
        # (pitch_hi/lo x w_pitch_hi/lo cross terms + beats x demb_hi/lo).
        if True:
            for s in range(T // (NSUP * 128)):
                peh = pe_pool.tile([128, NSUP, E], bf16, tag="peh")
                pel = pe_pool.tile([128, NSUP, E], bf16, tag="pel")
                nc.sync.dma_start(peh[:], pe_hi[:, s * NSUP:(s + 1) * NSUP, :])
                nc.sync.dma_start(pel[:], pe_lo[:, s * NSUP:(s + 1) * NSUP, :])
                for r in range(RPC):
                    ot = out_pool.tile([128, NSUP, E], f32, tag="ot")
                    for j in range(NSUP):
                        g = s * NSUP + j
                        ps = psum_b.tile([128, E], f32, tag="ps")
                        sj_ap = sj_sb[:, j * 128:(j + 1) * 128]
                        nc.tensor.matmul(
                            ps[:], lhsT=sj_ap, rhs=egh_keep[r][:, s, :],
                            start=True, stop=False,
                        )
                        nc.tensor.matmul(
                            ps[:], lhsT=sj_ap, rhs=egl_keep[r][:, s, :],
                            start=False, stop=False,
                        )
                        nc.tensor.matmul(
                            ps[:], lhsT=i128_sb[:], rhs=peh[:, j, :],
                            start=False, stop=False,
                        )
                        nc.tensor.matmul(
                            ps[:], lhsT=i128_sb[:], rhs=pel[:, j, :],
                            start=False, stop=False,
                        )
                        cb = g // (NGRP // 4)          # token-chunk bank
                        u0 = (g % (NGRP // 4)) * 128
                        nc.tensor.matmul(
                            ps[:],
                            lhsT=l5_sb[r][32 * cb:32 * cb + 5, u0:u0 + 128],
                            rhs=w5_sb[32 * cb:32 * cb + 5, :],
                            start=False, stop=True,
                            tile_position=(32 * cb, 0),
                        )
                        nc.vector.tensor_copy(ot[:, j, :], ps[:])
                    nc.sync.dma_start(
                        out[r, s * NSUP * 128:(s + 1) * NSUP * 128, :].rearrange(
                            "(n p) e -> p n e", p=128
                        ),
                        ot[:],
                    )
    nc.compile()
    return nc


def build_nc_general():
    """Arbitrary-idx path: per-128-token indirect row gathers."""
    import concourse.bass as bass
    from contextlib import ExitStack
    import concourse.tile as tile
    from concourse import bacc, mybir
    from concourse._compat import get_trn_type

    f32 = mybir.dt.float32
    i32 = mybir.dt.int32

    nc = bacc.Bacc(get_trn_type() or "TRN2", target_bir_lowering=False, debug=False)
    enc_t = nc.declare_dram_parameter("enc_t", [RPC, E, P], f32, isOutput=False)
    g_mat = nc.declare_dram_parameter("g_mat", [E, E], f32, isOutput=False)
    pe_w = nc.declare_dram_parameter("pe_w", [128, NGRP, E], f32, isOutput=False)
    p3 = nc.declare_dram_parameter("p3", [RPC, 3, T], f32, isOutput=False)
    w3 = nc.declare_dram_parameter("w3", [3, E], f32, isOutput=False)
    idxo = nc.declare_dram_parameter(
        "idxo", [RPC, 128, NGRP], i32, isOutput=False
    )
    out = nc.declare_dram_parameter("out", [RPC, T, E], f32, isOutput=True)
    encg = nc.dram_tensor("encg", [RPC, P, E], f32)

    with tile.TileContext(nc) as tc, ExitStack() as ctx:
        const = ctx.enter_context(tc.tile_pool(name="const", bufs=1))
        encT_pool = ctx.enter_context(tc.tile_pool(name="encT", bufs=2))
        psum_pool = ctx.enter_context(tc.tile_pool(name="psum", bufs=2, space="PSUM"))
        eg_pool = ctx.enter_context(tc.tile_pool(name="eg", bufs=2))
        pe_pool = ctx.enter_context(tc.tile_pool(name="pe", bufs=2))
        gath_pool = ctx.enter_context(tc.tile_pool(name="gath", bufs=3))

        g0 = const.tile([128, E], f32, tag="g0")
        g1 = const.tile([128, E], f32, tag="g1")
        nc.sync.dma_start(g0[:], g_mat[0:128, :])
        nc.sync.dma_start(g1[:], g_mat[128:256, :])
        w3_sb = const.tile([3, E], f32, tag="w3")
        nc.sync.dma_start(w3_sb[:], w3[:, :])
        p3_sb = []
        ixo_sb = []
        for r in range(RPC):
            p3t = const.tile([3, T], f32, tag=f"p3_{r}")
            nc.sync.dma_start(p3t[:], p3[r])
            p3_sb.append(p3t)
            ixt = const.tile([128, NGRP], i32, tag=f"ixo_{r}")
            nc.sync.dma_start(ixt[:], idxo[r])
            ixo_sb.append(ixt)

        for r in range(RPC):
            et0 = encT_pool.tile([128, P], f32, tag="et0")
            et1 = encT_pool.tile([128, P], f32, tag="et1")
            nc.sync.dma_start(et0[:], enc_t[r, 0:128, :])
            nc.sync.dma_start(et1[:], enc_t[r, 128:256, :])
            ps = psum_pool.tile([128, 8 * E], f32, tag="ps")
            for m in range(8):
                nc.tensor.matmul(
                    ps[:, m * E:(m + 1) * E],
                    lhsT=et0[:, m * 128:(m + 1) * 128],
                    rhs=g0[:], start=True, stop=False,
                )
                nc.tensor.matmul(
                    ps[:, m * E:(m + 1) * E],
                    lhsT=et1[:, m * 128:(m + 1) * 128],
                    rhs=g1[:], start=False, stop=True,
                )
            eg = eg_pool.tile([128, 8 * E], f32, tag="eg")
            nc.vector.tensor_copy(eg[:], ps[:])
            nc.sync.dma_start(
                encg[r].rearrange("(m p) e -> p m e", p=128),
                eg[:].rearrange("q (m e) -> q m e", e=E),
            )

        encg_flat = encg[:].rearrange("r p e -> (r p) e")
        for s in range(T // (NSUP * 128)):
            pe_t = pe_pool.tile([128, NSUP, E], f32, tag="pe")
            nc.sync.dma_start(pe_t[:], pe_w[:, s * NSUP:(s + 1) * NSUP, :])
            for r in range(RPC):
                gt = gath_pool.tile([128, NSUP, E], f32, tag="gt")
                for g in range(NSUP):
                    gi = s * NSUP + g
                    nc.gpsimd.indirect_dma_start(
                        out=gt[:, g, :],
                        out_offset=None,
                        in_=encg_flat,
                        in_offset=bass.IndirectOffsetOnAxis(
                            ap=ixo_sb[r][:, gi:gi + 1], axis=0
                        ),
                    )
                nc.vector.tensor_add(gt[:], gt[:], pe_t[:])
                ps = psum_pool.tile([128, 8 * E], f32, tag="ps")
                for g in range(NSUP):
                    gi = s * NSUP + g
                    nc.tensor.matmul(
                        ps[:, g * E:(g + 1) * E],
                        lhsT=p3_sb[r][:, gi * 128:(gi + 1) * 128],
                        rhs=w3_sb[:],
                        start=True, stop=True,
                    )
                nc.vector.tensor_add(
                    gt[:], gt[:], ps[:].rearrange("q (n e) -> q n e", e=E)
                )
                nc.sync.dma_start(
                    out[r, s * NSUP * 128:(s + 1) * NSUP * 128, :].rearrange(
                        "(n p) e -> p n e", p=128
                    ),
                    gt[:],
                )
    nc.compile()
    return nc


def get_nc(fast):
    key = "nc_fast" if fast else "nc_gen"
    if key not in _CACHE:
        _CACHE[key] = build_nc_fast() if fast else build_nc_general()
    return _CACHE[key]


# --------------------------------------------------------------------------
# Host wrapper
# --------------------------------------------------------------------------
def make_in_maps(encoder_out, align_phone, text_phone, pitch, beats,
                 w_pitch, b_pitch, emb_beats, w_pos, b_pos):
    import ml_dtypes

    encoder_out = np.asarray(encoder_out, np.float32)
    pitch = np.asarray(pitch, np.float32)
    beats = np.asarray(beats)
    w_pitch = np.asarray(w_pitch, np.float32)
    w_pos = np.asarray(w_pos, np.float32)

    idx = compute_idx(np.asarray(align_phone), np.asarray(text_phone))
    fast = bool(np.all(idx == (np.arange(T, dtype=np.int32) // DUR)[None, :]))
    if FORCE_GENERAL:
        fast = False

    g_mat = (np.eye(E, dtype=np.float64) + w_pos.astype(np.float64)).astype(np.float32)
    pe = _positional_encoding_f64(T, E)
    pe_proj = pe @ w_pos.astype(np.float64)                          # [T, E]
    bias = (np.asarray(emb_beats[0], np.float64)
            + np.asarray(b_pitch, np.float64)
            + np.asarray(b_pos, np.float64))
    demb = (np.asarray(emb_beats[1], np.float64)
            - np.asarray(emb_beats[0], np.float64)).astype(np.float32)

    if fast:
        pe_tot = (pe_proj + bias[None, :]).astype(np.float32)
        pe_wrap = np.ascontiguousarray(pe_tot.reshape(NGRP, 128, E).swapaxes(0, 1))
        pe_hi, pe_lo = _bf16_split(pe_wrap)
        # S_j[k, t'] = 1 iff k == j*16 + t'//8
        rows = np.arange(128)[:, None]
        sj = np.concatenate(
            [(rows == (j * NW + np.arange(128) // DUR)[None, :]) for j in range(NSUP)],
            axis=1,
        ).astype(ml_dtypes.bfloat16)
        i128 = np.eye(128, dtype=ml_dtypes.bfloat16)
        g_hi, g_lo = _bf16_split(g_mat)
        wp_hi, wp_lo = _bf16_split(w_pitch[0])
        db_hi, db_lo = _bf16_split(demb)
        w5_rows = np.stack([
            wp_hi, wp_lo, wp_hi,
            db_hi, db_lo,
        ]).astype(ml_dtypes.bfloat16)
        w5 = np.zeros((128, E), ml_dtypes.bfloat16)
        for cb in range(4):
            w5[32 * cb:32 * cb + 5] = w5_rows
        fast_common = {
            "pe_hi": pe_hi, "pe_lo": pe_lo, "sj": sj, "i128": i128,
            "g_hi": g_hi, "g_lo": g_lo, "w5": w5,
        }
    else:
        w3 = np.stack(
            [w_pitch[0].astype(np.float64), demb.astype(np.float64), bias]
        ).astype(np.float32)
        pe_wl = np.ascontiguousarray(
            pe_proj.astype(np.float32).reshape(NGRP, 128, E).swapaxes(0, 1)
        )

    in_maps = []
    for core in range(NCORES):
        rows_ = range(core * RPC, (core + 1) * RPC)
        enc_t = np.ascontiguousarray(
            encoder_out[core * RPC:(core + 1) * RPC].transpose(0, 2, 1)
        )
        if fast:
            import ml_dtypes as _md
            enc_hi, enc_lo = _bf16_split(enc_t)
            l5 = np.zeros((RPC, 128, T // 4), _md.bfloat16)
            for j, b in enumerate(rows_):
                p_hi, p_lo = _bf16_split(pitch[b, :, 0])
                bt = beats[b, :, 0].astype(_md.bfloat16)
                rows5 = np.stack([p_hi, p_hi, p_lo, bt, bt])     # [5, T]
                for cb in range(4):
                    l5[j, 32 * cb:32 * cb + 5] = rows5[
                        :, cb * (T // 4):(cb + 1) * (T // 4)
                    ]
            m = {"enc_hi": enc_hi, "enc_lo": enc_lo, "l5": l5, **fast_common}
        else:
            p3 = np.empty((RPC, 3, T), np.float32)
            idxo = np.empty((RPC, 128, NGRP), np.int32)
            for j, b in enumerate(rows_):
                p3[j, 0] = pitch[b, :, 0]
                p3[j, 1] = beats[b, :, 0].astype(np.float32)
                p3[j, 2] = 1.0
                idxo[j] = idx[b].reshape(NGRP, 128).T + j * P
            m = {"enc_t": enc_t, "g_mat": g_mat, "pe_w": pe_wl, "p3": p3,
                 "w3": w3, "idxo": idxo}
        in_maps.append(m)
    return fast, in_maps


def kernel(**inputs):
    from concourse.bass_utils import run_bass_kernel_spmd

    fast, in_maps = make_in_maps(**inputs)
    nc = get_nc(fast)
    res = run_bass_kernel_spmd(nc, in_maps, core_ids=list(range(NCORES)))
    out = np.concatenate([res.results[i]["out"] for i in range(NCORES)], axis=0)
    return np.ascontiguousarray(out.astype(np.float32))


# revision 38
# speedup vs baseline: 2.1464x; 1.0583x over previous
"""Trainium2 Bass kernel for nn_Encoder_Postnet (ragged_sequence).

Computation (reference):
    idx   = sequential aligner scan over (align_phone, text_phone)   [B,T]
    out   = enc[idx] + pitch @ w_pitch + b_pitch + emb_beats[beats]
            + (enc[idx] + pe) @ w_pos + b_pos

Key algebraic restructure: the duration-expansion gather commutes with the
E x E linear, so
    out[t] = encG[idx_t] + (pe@w_pos + bias)[t] + pitch[t]*w_pitch + beats[t]*demb
with encG = enc @ (I + w_pos) computed once per batch row ([P,E] not [T,E]),
collapsing the big [B*T,E]@[E,E] matmul 8x and making the kernel memory-bound.

Sharding: pure data parallel, 2 batch rows per core across 8 cores.

Fast path (the uniform duration-8 expansion this model's inputs produce,
idx == arange(T)//8 for every row):
  phase A: encG = enc_row @ (I+w_pos) on PE (fp32); the result stays resident
           in SBUF split into bf16 hi/lo pairs (hi+lo keeps ~17 mantissa
           bits, and bf16 matmuls run 4x faster than fp32 on the PE).
  phase B: per 128-token group, one PSUM accumulation group of 4 bf16
           matmuls: S_j-one-hot expansion of encG rows (hi+lo) + identity
           matmuls adding the (pe@w_pos + bias) tile (hi+lo).  Then the
           pitch rank-1 term rides the DVE op that drains PSUM->SBUF
           (scalar_tensor_tensor), and the beats term runs on the otherwise
           idle GPSIMD.  The only DMA traffic is inputs-once + outputs-once.

General path (arbitrary idx): per-128-token indirect-DMA row gathers
(production-shaped offset [128,1] DynamicAP descriptors) + K=3 stream matmul.

The aligner scan itself is index metadata ([B,T] int32); it is computed on
host with a run-compressed O(B*P) algorithm exactly equivalent to the
reference recurrence, then consumed either as a uniformity proof (fast path)
or as gather offsets (general path).
"""

import sys

for _p in ("/opt/trn_rl_repo",):
    if _p not in sys.path:
        sys.path.insert(0, _p)

import numpy as np

B, P, T, E = 16, 1024, 8192, 256
NCORES = 8
RPC = B // NCORES          # batch rows per core
NGRP = T // 128            # 64 groups of 128 tokens per row
NSUP = 8                   # groups per super-chunk
DUR = T // P               # uniform duration of the fast path (8)
NW = 128 // DUR            # encG rows per group (16)

FORCE_GENERAL = False      # test hook: force the arbitrary-idx path
_CACHE = {}


# --------------------------------------------------------------------------
# Host: aligner index computation (exact replica of the reference recurrence)
# --------------------------------------------------------------------------
def compute_idx(align, text):
    """idx[b,0]=0; idx[b,j] = idx[b,j-1] if align[b,j]==text[b,idx[b,j-1]]
    else min(idx[b,j-1]+1, P-1).   Vectorized over batch via segment starts:
    the pointer advances i->i+1 at s_{i+1} = first j >= s_i+1 with
    align[j] != text[i]; within a run of align values equal to text[i] the
    first mismatch is the run end."""
    align = np.asarray(align)
    text = np.asarray(text)
    Bn, Tn = align.shape
    Pn = text.shape[1]
    diff = align[:, 1:] != align[:, :-1]                       # [B, T-1]
    c = np.full((Bn, Tn), Tn, np.int64)
    c[:, :-1] = np.where(diff, np.arange(1, Tn)[None, :], Tn)
    re = np.flip(np.minimum.accumulate(np.flip(c, axis=1), axis=1), axis=1)

    s = np.full((Bn, Pn), Tn, np.int64)
    s[:, 0] = 0
    cur = np.zeros(Bn, np.int64)
    arB = np.arange(Bn)
    for i in range(Pn - 1):
        j0 = cur + 1
        active = j0 < Tn
        j0c = np.minimum(j0, Tn - 1)
        eq = (align[arB, j0c] == text[:, i]) & active
        nxt = np.where(active, np.where(eq, re[arB, j0c], j0), Tn)
        s[:, i + 1] = nxt
        cur = nxt
    idx = np.empty((Bn, Tn), np.int32)
    pos = np.arange(Tn)
    for b in range(Bn):
        idx[b] = (np.searchsorted(s[b], pos, side="right") - 1).astype(np.int32)
    return idx


def _positional_encoding_f64(t, e):
    pos = np.arange(t, dtype=np.float64)[:, None]
    div = np.exp(np.arange(0, e, 2, dtype=np.float64) * (-np.log(10000.0) / e))
    ang = pos * div[None, :]
    return np.stack([np.sin(ang), np.cos(ang)], axis=-1).reshape(t, e)


def _bf16_split(x):
    import ml_dtypes
    x = np.asarray(x, np.float32)
    hi = x.astype(ml_dtypes.bfloat16)
    lo = (x - hi.astype(np.float32)).astype(ml_dtypes.bfloat16)
    return hi, lo


# --------------------------------------------------------------------------
# Device programs
# --------------------------------------------------------------------------
def build_nc_fast():
    from contextlib import ExitStack
    import concourse.tile as tile
    from concourse import bacc, mybir
    from concourse._compat import get_trn_type

    f32 = mybir.dt.float32
    bf16 = mybir.dt.bfloat16
    mult = mybir.AluOpType.mult
    add = mybir.AluOpType.add

    nc = bacc.Bacc(get_trn_type() or "TRN2", target_bir_lowering=False, debug=False)
    enc_hi = nc.declare_dram_parameter("enc_hi", [RPC, E, P], bf16, isOutput=False)
    enc_lo = nc.declare_dram_parameter("enc_lo", [RPC, E, P], bf16, isOutput=False)
    g_hi = nc.declare_dram_parameter("g_hi", [E, E], bf16, isOutput=False)
    g_lo = nc.declare_dram_parameter("g_lo", [E, E], bf16, isOutput=False)
    pe_w = nc.declare_dram_parameter("pe_w", [128, NGRP, E], f32, isOutput=False)
    sj_d = nc.declare_dram_parameter("sj", [128, NSUP * 128], bf16, isOutput=False)
    # stream-term lhsT rows banked at 32-partition strides (4 token-chunks)
    # to keep the per-partition footprint small for the DMA
    l5_d = nc.declare_dram_parameter("l5", [RPC, 128, T // 4], bf16, isOutput=False)
    w5_d = nc.declare_dram_parameter("w5", [128, E], bf16, isOutput=False)
    out = nc.declare_dram_parameter("out", [RPC, T, E], f32, isOutput=True)

    with tile.TileContext(nc) as tc, ExitStack() as ctx:
        const = ctx.enter_context(tc.tile_pool(name="const", bufs=1))
        pe_pool = ctx.enter_context(tc.tile_pool(name="pe", bufs=8))
        out_pool = ctx.enter_context(tc.tile_pool(name="outp", bufs=4))

        sj_sb = const.tile([128, NSUP * 128], bf16, tag="sj")
        nc.sync.dma_start(sj_sb[:], sj_d[:])
        w5_sb = const.tile([128, E], bf16, tag="w5")
        nc.sync.dma_start(w5_sb[:], w5_d[:])
        l5_sb, egh_keep, egl_keep = [], [], []
        for r in range(RPC):
            l5t = const.tile([128, T // 4], bf16, tag=f"l5_{r}")
            nc.sync.dma_start(l5t[:], l5_d[r])
            l5_sb.append(l5t)
            egh_keep.append(
                const.tile([128, NSUP, E], bf16, tag=f"egh{r}", name=f"egh{r}")
            )
            egl_keep.append(
                const.tile([128, NSUP, E], bf16, tag=f"egl{r}", name=f"egl{r}")
            )

        # ---- phase A: encG = enc @ (I+w_pos) as 3-term bf16-split matmuls;
        # result kept in SBUF as bf16 hi/lo.  psum layout [128p, m, e] ==
        # keep layout: row m*128+p at (partition p, block m).
        gh0 = const.tile([128, E], bf16, tag="gh0", name="gh0")
        gh1 = const.tile([128, E], bf16, tag="gh1", name="gh1")
        gl0 = const.tile([128, E], bf16, tag="gl0", name="gl0")
        gl1 = const.tile([128, E], bf16, tag="gl1", name="gl1")
        nc.sync.dma_start(gh0[:], g_hi[0:128, :])
        nc.sync.dma_start(gh1[:], g_hi[128:256, :])
        nc.sync.dma_start(gl0[:], g_lo[0:128, :])
        nc.sync.dma_start(gl1[:], g_lo[128:256, :])
        psum_a = ctx.enter_context(tc.tile_pool(name="psumA", bufs=1, space="PSUM"))
        psum_b = ctx.enter_context(tc.tile_pool(name="psumB", bufs=4, space="PSUM"))
        with tc.tile_pool(name="encT", bufs=2) as encT_pool:
            for r in range(RPC):
                eh0 = encT_pool.tile([128, P], bf16, tag="eh0")
                eh1 = encT_pool.tile([128, P], bf16, tag="eh1")
                el0 = encT_pool.tile([128, P], bf16, tag="el0")
                el1 = encT_pool.tile([128, P], bf16, tag="el1")
                nc.sync.dma_start(eh0[:], enc_hi[r, 0:128, :])
                nc.sync.dma_start(eh1[:], enc_hi[r, 128:256, :])
                nc.sync.dma_start(el0[:], enc_lo[r, 0:128, :])
                nc.sync.dma_start(el1[:], enc_lo[r, 128:256, :])
                ps = psum_a.tile([128, 8 * E], f32, tag="psA")
                for m in range(8):
                    sl = slice(m * 128, (m + 1) * 128)
                    terms = [
                        (eh0, gh0, True, False), (eh1, gh1, False, False),
                        (el0, gh0, False, False), (el1, gh1, False, False),
                        (eh0, gl0, False, False), (eh1, gl1, False, True),
                    ]
                    for lt, gt_, st, sp in terms:
                        nc.tensor.matmul(
                            ps[:, m * E:(m + 1) * E],
                            lhsT=lt[:, sl], rhs=gt_[:], start=st, stop=sp,
                        )
                    # drain per m-chunk so phase B super-chunk m can start
                    # before the rest of phase A finishes
                    hi = egh_keep[r][:, m, :]
                    nc.vector.tensor_copy(hi, ps[:, m * E:(m + 1) * E])
                    nc.vector.scalar_tensor_tensor(
                        out=egl_keep[r][:, m, :], in0=hi, scalar=-1.0,
                        in1=ps[:, m * E:(m + 1) * E], op0=mult, op1=add,
                    )

        # ---- phase B: per group, 5 bf16 matmuls into one PSUM group:
        # expansion hi/lo + pe hi/lo + the K=5 rank-1 stream matmul
        # (pitch_hi/lo x w_pitch_hi/lo cross terms + beats x demb_hi/lo).
        if True:
            for s in range(T // (NSUP * 128)):
                pe_t = pe_pool.tile([128, NSUP, E], f32, tag="pe")
                nc.sync.dma_start(pe_t[:], pe_w[:, s * NSUP:(s + 1) * NSUP, :])
                for r in range(RPC):
                    ot = out_pool.tile([128, NSUP, E], f32, tag="ot")
                    for j in range(NSUP):
                        g = s * NSUP + j
                        ps = psum_b.tile([128, E], f32, tag="ps")
                        sj_ap = sj_sb[:, j * 128:(j + 1) * 128]
                        nc.tensor.matmul(
                            ps[:], lhsT=sj_ap, rhs=egh_keep[r][:, s, :],
                            start=True, stop=False,
                        )
                        nc.tensor.matmul(
                            ps[:], lhsT=sj_ap, rhs=egl_keep[r][:, s, :],
                            start=False, stop=False,
                        )
                        cb = g // (NGRP // 4)          # token-chunk bank
                        u0 = (g % (NGRP // 4)) * 128
                        nc.tensor.matmul(
                            ps[:],
                            lhsT=l5_sb[r][32 * cb:32 * cb + 5, u0:u0 + 128],
                            rhs=w5_sb[32 * cb:32 * cb + 5, :],
                            start=False, stop=True,
                            tile_position=(32 * cb, 0),
                        )
                        # drain PSUM -> SBUF fused with the (pe@w_pos+bias) add
                        nc.vector.tensor_add(ot[:, j, :], ps[:], pe_t[:, j, :])
                    nc.sync.dma_start(
                        out[r, s * NSUP * 128:(s + 1) * NSUP * 128, :].rearrange(
                            "(n p) e -> p n e", p=128
                        ),
                        ot[:],
                    )
    nc.compile()
    return nc


def build_nc_general():
    """Arbitrary-idx path: per-128-token indirect row gathers."""
    import concourse.bass as bass
    from contextlib import ExitStack
    import concourse.tile as tile
    from concourse import bacc, mybir
    from concourse._compat import get_trn_type

    f32 = mybir.dt.float32
    i32 = mybir.dt.int32

    nc = bacc.Bacc(get_trn_type() or "TRN2", target_bir_lowering=False, debug=False)
    enc_t = nc.declare_dram_parameter("enc_t", [RPC, E, P], f32, isOutput=False)
    g_mat = nc.declare_dram_parameter("g_mat", [E, E], f32, isOutput=False)
    pe_w = nc.declare_dram_parameter("pe_w", [128, NGRP, E], f32, isOutput=False)
    p3 = nc.declare_dram_parameter("p3", [RPC, 3, T], f32, isOutput=False)
    w3 = nc.declare_dram_parameter("w3", [3, E], f32, isOutput=False)
    idxo = nc.declare_dram_parameter(
        "idxo", [RPC, 128, NGRP], i32, isOutput=False
    )
    out = nc.declare_dram_parameter("out", [RPC, T, E], f32, isOutput=True)
    encg = nc.dram_tensor("encg", [RPC, P, E], f32)

    with tile.TileContext(nc) as tc, ExitStack() as ctx:
        const = ctx.enter_context(tc.tile_pool(name="const", bufs=1))
        encT_pool = ctx.enter_context(tc.tile_pool(name="encT", bufs=2))
        psum_pool = ctx.enter_context(tc.tile_pool(name="psum", bufs=2, space="PSUM"))
        eg_pool = ctx.enter_context(tc.tile_pool(name="eg", bufs=2))
        pe_pool = ctx.enter_context(tc.tile_pool(name="pe", bufs=2))
        gath_pool = ctx.enter_context(tc.tile_pool(name="gath", bufs=3))

        g0 = const.tile([128, E], f32, tag="g0")
        g1 = const.tile([128, E], f32, tag="g1")
        nc.sync.dma_start(g0[:], g_mat[0:128, :])
        nc.sync.dma_start(g1[:], g_mat[128:256, :])
        w3_sb = const.tile([3, E], f32, tag="w3")
        nc.sync.dma_start(w3_sb[:], w3[:, :])
        p3_sb = []
        ixo_sb = []
        for r in range(RPC):
            p3t = const.tile([3, T], f32, tag=f"p3_{r}")
            nc.sync.dma_start(p3t[:], p3[r])
            p3_sb.append(p3t)
            ixt = const.tile([128, NGRP], i32, tag=f"ixo_{r}")
            nc.sync.dma_start(ixt[:], idxo[r])
            ixo_sb.append(ixt)

        for r in range(RPC):
            et0 = encT_pool.tile([128, P], f32, tag="et0")
            et1 = encT_pool.tile([128, P], f32, tag="et1")
            nc.sync.dma_start(et0[:], enc_t[r, 0:128, :])
            nc.sync.dma_start(et1[:], enc_t[r, 128:256, :])
            ps = psum_pool.tile([128, 8 * E], f32, tag="ps")
            for m in range(8):
                nc.tensor.matmul(
                    ps[:, m * E:(m + 1) * E],
                    lhsT=et0[:, m * 128:(m + 1) * 128],
                    rhs=g0[:], start=True, stop=False,
                )
                nc.tensor.matmul(
                    ps[:, m * E:(m + 1) * E],
                    lhsT=et1[:, m * 128:(m + 1) * 128],
                    rhs=g1[:], start=False, stop=True,
                )
            eg = eg_pool.tile([128, 8 * E], f32, tag="eg")
            nc.vector.tensor_copy(eg[:], ps[:])
            nc.sync.dma_start(
                encg[r].rearrange("(m p) e -> p m e", p=128),
                eg[:].rearrange("q (m e) -> q m e", e=E),
            )

        encg_flat = encg[:].rearrange("r p e -> (r p) e")
        for s in range(T // (NSUP * 128)):
            pe_t = pe_pool.tile([128, NSUP, E], f32, tag="pe")
            nc.sync.dma_start(pe_t[:], pe_w[:, s * NSUP:(s + 1) * NSUP, :])
            for r in range(RPC):
                gt = gath_pool.tile([128, NSUP, E], f32, tag="gt")
                for g in range(NSUP):
                    gi = s * NSUP + g
                    nc.gpsimd.indirect_dma_start(
                        out=gt[:, g, :],
                        out_offset=None,
                        in_=encg_flat,
                        in_offset=bass.IndirectOffsetOnAxis(
                            ap=ixo_sb[r][:, gi:gi + 1], axis=0
                        ),
                    )
                nc.vector.tensor_add(gt[:], gt[:], pe_t[:])
                ps = psum_pool.tile([128, 8 * E], f32, tag="ps")
                for g in range(NSUP):
                    gi = s * NSUP + g
                    nc.tensor.matmul(
                        ps[:, g * E:(g + 1) * E],
                        lhsT=p3_sb[r][:, gi * 128:(gi + 1) * 128],
                        rhs=w3_sb[:],
                        start=True, stop=True,
                    )
                nc.vector.tensor_add(
                    gt[:], gt[:], ps[:].rearrange("q (n e) -> q n e", e=E)
                )
                nc.sync.dma_start(
                    out[r, s * NSUP * 128:(s + 1) * NSUP * 128, :].rearrange(
                        "(n p) e -> p n e", p=128
                    ),
                    gt[:],
                )
    nc.compile()
    return nc


def get_nc(fast):
    key = "nc_fast" if fast else "nc_gen"
    if key not in _CACHE:
        _CACHE[key] = build_nc_fast() if fast else build_nc_general()
    return _CACHE[key]


# --------------------------------------------------------------------------
# Host wrapper
# --------------------------------------------------------------------------
def make_in_maps(encoder_out, align_phone, text_phone, pitch, beats,
                 w_pitch, b_pitch, emb_beats, w_pos, b_pos):
    import ml_dtypes

    encoder_out = np.asarray(encoder_out, np.float32)
    pitch = np.asarray(pitch, np.float32)
    beats = np.asarray(beats)
    w_pitch = np.asarray(w_pitch, np.float32)
    w_pos = np.asarray(w_pos, np.float32)

    idx = compute_idx(np.asarray(align_phone), np.asarray(text_phone))
    fast = bool(np.all(idx == (np.arange(T, dtype=np.int32) // DUR)[None, :]))
    if FORCE_GENERAL:
        fast = False

    g_mat = (np.eye(E, dtype=np.float64) + w_pos.astype(np.float64)).astype(np.float32)
    pe = _positional_encoding_f64(T, E)
    pe_proj = pe @ w_pos.astype(np.float64)                          # [T, E]
    bias = (np.asarray(emb_beats[0], np.float64)
            + np.asarray(b_pitch, np.float64)
            + np.asarray(b_pos, np.float64))
    demb = (np.asarray(emb_beats[1], np.float64)
            - np.asarray(emb_beats[0], np.float64)).astype(np.float32)

    if fast:
        pe_tot = (pe_proj + bias[None, :]).astype(np.float32)
        pe_wrap = np.ascontiguousarray(pe_tot.reshape(NGRP, 128, E).swapaxes(0, 1))
        # S_j[k, t'] = 1 iff k == j*16 + t'//8
        rows = np.arange(128)[:, None]
        sj = np.concatenate(
            [(rows == (j * NW + np.arange(128) // DUR)[None, :]) for j in range(NSUP)],
            axis=1,
        ).astype(ml_dtypes.bfloat16)
        g_hi, g_lo = _bf16_split(g_mat)
        wp_hi, wp_lo = _bf16_split(w_pitch[0])
        db_hi, db_lo = _bf16_split(demb)
        w5_rows = np.stack([
            wp_hi, wp_lo, wp_hi,
            db_hi, db_lo,
        ]).astype(ml_dtypes.bfloat16)
        w5 = np.zeros((128, E), ml_dtypes.bfloat16)
        for cb in range(4):
            w5[32 * cb:32 * cb + 5] = w5_rows
        fast_common = {
            "pe_w": pe_wrap, "sj": sj,
            "g_hi": g_hi, "g_lo": g_lo, "w5": w5,
        }
    else:
        w3 = np.stack(
            [w_pitch[0].astype(np.float64), demb.astype(np.float64), bias]
        ).astype(np.float32)
        pe_wl = np.ascontiguousarray(
            pe_proj.astype(np.float32).reshape(NGRP, 128, E).swapaxes(0, 1)
        )

    in_maps = []
    for core in range(NCORES):
        rows_ = range(core * RPC, (core + 1) * RPC)
        enc_t = np.ascontiguousarray(
            encoder_out[core * RPC:(core + 1) * RPC].transpose(0, 2, 1)
        )
        if fast:
            import ml_dtypes as _md
            enc_hi, enc_lo = _bf16_split(enc_t)
            l5 = np.zeros((RPC, 128, T // 4), _md.bfloat16)
            for j, b in enumerate(rows_):
                p_hi, p_lo = _bf16_split(pitch[b, :, 0])
                bt = beats[b, :, 0].astype(_md.bfloat16)
                rows5 = np.stack([p_hi, p_hi, p_lo, bt, bt])     # [5, T]
                for cb in range(4):
                    l5[j, 32 * cb:32 * cb + 5] = rows5[
                        :, cb * (T // 4):(cb + 1) * (T // 4)
                    ]
            m = {"enc_hi": enc_hi, "enc_lo": enc_lo, "l5": l5, **fast_common}
        else:
            p3 = np.empty((RPC, 3, T), np.float32)
            idxo = np.empty((RPC, 128, NGRP), np.int32)
            for j, b in enumerate(rows_):
                p3[j, 0] = pitch[b, :, 0]
                p3[j, 1] = beats[b, :, 0].astype(np.float32)
                p3[j, 2] = 1.0
                idxo[j] = idx[b].reshape(NGRP, 128).T + j * P
            m = {"enc_t": enc_t, "g_mat": g_mat, "pe_w": pe_wl, "p3": p3,
                 "w3": w3, "idxo": idxo}
        in_maps.append(m)
    return fast, in_maps


def kernel(**inputs):
    from concourse.bass_utils import run_bass_kernel_spmd

    fast, in_maps = make_in_maps(**inputs)
    nc = get_nc(fast)
    res = run_bass_kernel_spmd(nc, in_maps, core_ids=list(range(NCORES)))
    out = np.concatenate([res.results[i]["out"] for i in range(NCORES)], axis=0)
    return np.ascontiguousarray(out.astype(np.float32))


# revision 41
# speedup vs baseline: 2.1622x; 1.0073x over previous
"""Trainium2 Bass kernel for nn_Encoder_Postnet (ragged_sequence).

Computation (reference):
    idx   = sequential aligner scan over (align_phone, text_phone)   [B,T]
    out   = enc[idx] + pitch @ w_pitch + b_pitch + emb_beats[beats]
            + (enc[idx] + pe) @ w_pos + b_pos

Key algebraic restructure: the duration-expansion gather commutes with the
E x E linear, so
    out[t] = encG[idx_t] + (pe@w_pos + bias)[t] + pitch[t]*w_pitch + beats[t]*demb
with encG = enc @ (I + w_pos) computed once per batch row ([P,E] not [T,E]),
collapsing the big [B*T,E]@[E,E] matmul 8x and making the kernel memory-bound.

Sharding: pure data parallel, 2 batch rows per core across 8 cores.

Fast path (the uniform duration-8 expansion this model's inputs produce,
idx == arange(T)//8 for every row):
  phase A: encG = enc_row @ (I+w_pos) on PE (fp32); the result stays resident
           in SBUF split into bf16 hi/lo pairs (hi+lo keeps ~17 mantissa
           bits, and bf16 matmuls run 4x faster than fp32 on the PE).
  phase B: per 128-token group, one PSUM accumulation group of 4 bf16
           matmuls: S_j-one-hot expansion of encG rows (hi+lo) + identity
           matmuls adding the (pe@w_pos + bias) tile (hi+lo).  Then the
           pitch rank-1 term rides the DVE op that drains PSUM->SBUF
           (scalar_tensor_tensor), and the beats term runs on the otherwise
           idle GPSIMD.  The only DMA traffic is inputs-once + outputs-once.

General path (arbitrary idx): per-128-token indirect-DMA row gathers
(production-shaped offset [128,1] DynamicAP descriptors) + K=3 stream matmul.

The aligner scan itself is index metadata ([B,T] int32); it is computed on
host with a run-compressed O(B*P) algorithm exactly equivalent to the
reference recurrence, then consumed either as a uniformity proof (fast path)
or as gather offsets (general path).
"""

import sys

for _p in ("/opt/trn_rl_repo",):
    if _p not in sys.path:
        sys.path.insert(0, _p)

import numpy as np

B, P, T, E = 16, 1024, 8192, 256
NCORES = 8
RPC = B // NCORES          # batch rows per core
NGRP = T // 128            # 64 groups of 128 tokens per row
NSUP = 8                   # groups per super-chunk
DUR = T // P               # uniform duration of the fast path (8)
NW = 128 // DUR            # encG rows per group (16)

FORCE_GENERAL = False      # test hook: force the arbitrary-idx path
_CACHE = {}


# --------------------------------------------------------------------------
# Host: aligner index computation (exact replica of the reference recurrence)
# --------------------------------------------------------------------------
def compute_idx(align, text):
    """idx[b,0]=0; idx[b,j] = idx[b,j-1] if align[b,j]==text[b,idx[b,j-1]]
    else min(idx[b,j-1]+1, P-1).   Vectorized over batch via segment starts:
    the pointer advances i->i+1 at s_{i+1} = first j >= s_i+1 with
    align[j] != text[i]; within a run of align values equal to text[i] the
    first mismatch is the run end."""
    align = np.asarray(align)
    text = np.asarray(text)
    Bn, Tn = align.shape
    Pn = text.shape[1]
    diff = align[:, 1:] != align[:, :-1]                       # [B, T-1]
    c = np.full((Bn, Tn), Tn, np.int64)
    c[:, :-1] = np.where(diff, np.arange(1, Tn)[None, :], Tn)
    re = np.flip(np.minimum.accumulate(np.flip(c, axis=1), axis=1), axis=1)

    s = np.full((Bn, Pn), Tn, np.int64)
    s[:, 0] = 0
    cur = np.zeros(Bn, np.int64)
    arB = np.arange(Bn)
    for i in range(Pn - 1):
        j0 = cur + 1
        active = j0 < Tn
        j0c = np.minimum(j0, Tn - 1)
        eq = (align[arB, j0c] == text[:, i]) & active
        nxt = np.where(active, np.where(eq, re[arB, j0c], j0), Tn)
        s[:, i + 1] = nxt
        cur = nxt
    idx = np.empty((Bn, Tn), np.int32)
    pos = np.arange(Tn)
    for b in range(Bn):
        idx[b] = (np.searchsorted(s[b], pos, side="right") - 1).astype(np.int32)
    return idx


def _positional_encoding_f64(t, e):
    pos = np.arange(t, dtype=np.float64)[:, None]
    div = np.exp(np.arange(0, e, 2, dtype=np.float64) * (-np.log(10000.0) / e))
    ang = pos * div[None, :]
    return np.stack([np.sin(ang), np.cos(ang)], axis=-1).reshape(t, e)


def _bf16_split(x):
    import ml_dtypes
    x = np.asarray(x, np.float32)
    hi = x.astype(ml_dtypes.bfloat16)
    lo = (x - hi.astype(np.float32)).astype(ml_dtypes.bfloat16)
    return hi, lo


# --------------------------------------------------------------------------
# Device programs
# --------------------------------------------------------------------------
def build_nc_fast():
    from contextlib import ExitStack
    import concourse.tile as tile
    from concourse import bacc, mybir
    from concourse._compat import get_trn_type

    f32 = mybir.dt.float32
    bf16 = mybir.dt.bfloat16
    mult = mybir.AluOpType.mult
    add = mybir.AluOpType.add

    nc = bacc.Bacc(get_trn_type() or "TRN2", target_bir_lowering=False, debug=False)
    enc_hi = nc.declare_dram_parameter("enc_hi", [RPC, E, P], bf16, isOutput=False)
    enc_lo = nc.declare_dram_parameter("enc_lo", [RPC, E, P], bf16, isOutput=False)
    g_hi = nc.declare_dram_parameter("g_hi", [E, E], bf16, isOutput=False)
    g_lo = nc.declare_dram_parameter("g_lo", [E, E], bf16, isOutput=False)
    pe_w = nc.declare_dram_parameter("pe_w", [128, NGRP, E], f32, isOutput=False)
    sj_d = nc.declare_dram_parameter("sj", [128, NSUP * 128], bf16, isOutput=False)
    # stream-term lhsT rows banked at 32-partition strides (4 token-chunks)
    # to keep the per-partition footprint small for the DMA
    l5_d = nc.declare_dram_parameter("l5", [RPC, 128, T // 4], bf16, isOutput=False)
    w5_d = nc.declare_dram_parameter("w5", [128, E], bf16, isOutput=False)
    out = nc.declare_dram_parameter("out", [RPC, T, E], f32, isOutput=True)

    with tile.TileContext(nc) as tc, ExitStack() as ctx:
        const = ctx.enter_context(tc.tile_pool(name="const", bufs=1))
        pe_pool = ctx.enter_context(tc.tile_pool(name="pe", bufs=8))
        out_pool = ctx.enter_context(tc.tile_pool(name="outp", bufs=4))

        sj_sb = const.tile([128, NSUP * 128], bf16, tag="sj")
        nc.sync.dma_start(sj_sb[:], sj_d[:])
        w5_sb = const.tile([128, E], bf16, tag="w5")
        nc.sync.dma_start(w5_sb[:], w5_d[:])
        l5_sb, egh_keep, egl_keep = [], [], []
        for r in range(RPC):
            l5t = const.tile([128, T // 4], bf16, tag=f"l5_{r}")
            nc.sync.dma_start(l5t[:], l5_d[r])
            l5_sb.append(l5t)
            egh_keep.append(
                const.tile([128, NSUP, E], bf16, tag=f"egh{r}", name=f"egh{r}")
            )
            egl_keep.append(
                const.tile([128, NSUP, E], bf16, tag=f"egl{r}", name=f"egl{r}")
            )

        # ---- phase A: encG = enc @ (I+w_pos) as 3-term bf16-split matmuls;
        # result kept in SBUF as bf16 hi/lo.  psum layout [128p, m, e] ==
        # keep layout: row m*128+p at (partition p, block m).
        gh0 = const.tile([128, E], bf16, tag="gh0", name="gh0")
        gh1 = const.tile([128, E], bf16, tag="gh1", name="gh1")
        gl0 = const.tile([128, E], bf16, tag="gl0", name="gl0")
        gl1 = const.tile([128, E], bf16, tag="gl1", name="gl1")
        nc.sync.dma_start(gh0[:], g_hi[0:128, :])
        nc.sync.dma_start(gh1[:], g_hi[128:256, :])
        nc.sync.dma_start(gl0[:], g_lo[0:128, :])
        nc.sync.dma_start(gl1[:], g_lo[128:256, :])
        psum_a = ctx.enter_context(tc.tile_pool(name="psumA", bufs=1, space="PSUM"))
        psum_b = ctx.enter_context(tc.tile_pool(name="psumB", bufs=6, space="PSUM"))
        with tc.tile_pool(name="encT", bufs=2) as encT_pool:
            for r in range(RPC):
                eh0 = encT_pool.tile([128, P], bf16, tag="eh0")
                eh1 = encT_pool.tile([128, P], bf16, tag="eh1")
                el0 = encT_pool.tile([128, P], bf16, tag="el0")
                el1 = encT_pool.tile([128, P], bf16, tag="el1")
                nc.sync.dma_start(eh0[:], enc_hi[r, 0:128, :])
                nc.sync.dma_start(eh1[:], enc_hi[r, 128:256, :])
                nc.sync.dma_start(el0[:], enc_lo[r, 0:128, :])
                nc.sync.dma_start(el1[:], enc_lo[r, 128:256, :])
                for mh in range(2):
                    ps = psum_a.tile([128, 4 * E], f32, tag="psA", name="psA")
                    for mi in range(4):
                        m = mh * 4 + mi
                        sl = slice(m * 128, (m + 1) * 128)
                        terms = [
                            (eh0, gh0, True, False), (eh1, gh1, False, False),
                            (el0, gh0, False, False), (el1, gh1, False, False),
                            (eh0, gl0, False, False), (eh1, gl1, False, True),
                        ]
                        for lt, gt_, st, sp in terms:
                            nc.tensor.matmul(
                                ps[:, mi * E:(mi + 1) * E],
                                lhsT=lt[:, sl], rhs=gt_[:], start=st, stop=sp,
                            )
                        # drain per m-chunk so phase B super-chunk m can start
                        # before the rest of phase A finishes
                        hi = egh_keep[r][:, m, :]
                        nc.vector.tensor_copy(hi, ps[:, mi * E:(mi + 1) * E])
                        nc.vector.scalar_tensor_tensor(
                            out=egl_keep[r][:, m, :], in0=hi, scalar=-1.0,
                            in1=ps[:, mi * E:(mi + 1) * E], op0=mult, op1=add,
                        )

        # ---- phase B: per group, 5 bf16 matmuls into one PSUM group:
        # expansion hi/lo + pe hi/lo + the K=5 rank-1 stream matmul
        # (pitch_hi/lo x w_pitch_hi/lo cross terms + beats x demb_hi/lo).
        if True:
            for s in range(T // (NSUP * 128)):
                pe_t = pe_pool.tile([128, NSUP, E], f32, tag="pe")
                nc.sync.dma_start(pe_t[:], pe_w[:, s * NSUP:(s + 1) * NSUP, :])
                for r in range(RPC):
                    ot = out_pool.tile([128, NSUP, E], f32, tag="ot")
                    for j in range(NSUP):
                        g = s * NSUP + j
                        ps = psum_b.tile([128, E], f32, tag="ps")
                        sj_ap = sj_sb[:, j * 128:(j + 1) * 128]
                        nc.tensor.matmul(
                            ps[:], lhsT=sj_ap, rhs=egh_keep[r][:, s, :],
                            start=True, stop=False,
                        )
                        nc.tensor.matmul(
                            ps[:], lhsT=sj_ap, rhs=egl_keep[r][:, s, :],
                            start=False, stop=False,
                        )
                        cb = g // (NGRP // 4)          # token-chunk bank
                        u0 = (g % (NGRP // 4)) * 128
                        nc.tensor.matmul(
                            ps[:],
                            lhsT=l5_sb[r][32 * cb:32 * cb + 5, u0:u0 + 128],
                            rhs=w5_sb[32 * cb:32 * cb + 5, :],
                            start=False, stop=True,
                            tile_position=(32 * cb, 0),
                        )
                        # drain PSUM -> SBUF fused with the (pe@w_pos+bias) add
                        nc.vector.tensor_add(ot[:, j, :], ps[:], pe_t[:, j, :])
                    nc.sync.dma_start(
                        out[r, s * NSUP * 128:(s + 1) * NSUP * 128, :].rearrange(
                            "(n p) e -> p n e", p=128
                        ),
                        ot[:],
                    )
    nc.compile()
    return nc


def build_nc_general():
    """Arbitrary-idx path: per-128-token indirect row gathers."""
    import concourse.bass as bass
    from contextlib import ExitStack
    import concourse.tile as tile
    from concourse import bacc, mybir
    from concourse._compat import get_trn_type

    f32 = mybir.dt.float32
    i32 = mybir.dt.int32

    nc = bacc.Bacc(get_trn_type() or "TRN2", target_bir_lowering=False, debug=False)
    enc_t = nc.declare_dram_parameter("enc_t", [RPC, E, P], f32, isOutput=False)
    g_mat = nc.declare_dram_parameter("g_mat", [E, E], f32, isOutput=False)
    pe_w = nc.declare_dram_parameter("pe_w", [128, NGRP, E], f32, isOutput=False)
    p3 = nc.declare_dram_parameter("p3", [RPC, 3, T], f32, isOutput=False)
    w3 = nc.declare_dram_parameter("w3", [3, E], f32, isOutput=False)
    idxo = nc.declare_dram_parameter(
        "idxo", [RPC, 128, NGRP], i32, isOutput=False
    )
    out = nc.declare_dram_parameter("out", [RPC, T, E], f32, isOutput=True)
    encg = nc.dram_tensor("encg", [RPC, P, E], f32)

    with tile.TileContext(nc) as tc, ExitStack() as ctx:
        const = ctx.enter_context(tc.tile_pool(name="const", bufs=1))
        encT_pool = ctx.enter_context(tc.tile_pool(name="encT", bufs=2))
        psum_pool = ctx.enter_context(tc.tile_pool(name="psum", bufs=2, space="PSUM"))
        eg_pool = ctx.enter_context(tc.tile_pool(name="eg", bufs=2))
        pe_pool = ctx.enter_context(tc.tile_pool(name="pe", bufs=2))
        gath_pool = ctx.enter_context(tc.tile_pool(name="gath", bufs=3))

        g0 = const.tile([128, E], f32, tag="g0")
        g1 = const.tile([128, E], f32, tag="g1")
        nc.sync.dma_start(g0[:], g_mat[0:128, :])
        nc.sync.dma_start(g1[:], g_mat[128:256, :])
        w3_sb = const.tile([3, E], f32, tag="w3")
        nc.sync.dma_start(w3_sb[:], w3[:, :])
        p3_sb = []
        ixo_sb = []
        for r in range(RPC):
            p3t = const.tile([3, T], f32, tag=f"p3_{r}")
            nc.sync.dma_start(p3t[:], p3[r])
            p3_sb.append(p3t)
            ixt = const.tile([128, NGRP], i32, tag=f"ixo_{r}")
            nc.sync.dma_start(ixt[:], idxo[r])
            ixo_sb.append(ixt)

        for r in range(RPC):
            et0 = encT_pool.tile([128, P], f32, tag="et0")
            et1 = encT_pool.tile([128, P], f32, tag="et1")
            nc.sync.dma_start(et0[:], enc_t[r, 0:128, :])
            nc.sync.dma_start(et1[:], enc_t[r, 128:256, :])
            ps = psum_pool.tile([128, 8 * E], f32, tag="ps")
            for m in range(8):
                nc.tensor.matmul(
                    ps[:, m * E:(m + 1) * E],
                    lhsT=et0[:, m * 128:(m + 1) * 128],
                    rhs=g0[:], start=True, stop=False,
                )
                nc.tensor.matmul(
                    ps[:, m * E:(m + 1) * E],
                    lhsT=et1[:, m * 128:(m + 1) * 128],
                    rhs=g1[:], start=False, stop=True,
                )
            eg = eg_pool.tile([128, 8 * E], f32, tag="eg")
            nc.vector.tensor_copy(eg[:], ps[:])
            nc.sync.dma_start(
                encg[r].rearrange("(m p) e -> p m e", p=128),
                eg[:].rearrange("q (m e) -> q m e", e=E),
            )

        encg_flat = encg[:].rearrange("r p e -> (r p) e")
        for s in range(T // (NSUP * 128)):
            pe_t = pe_pool.tile([128, NSUP, E], f32, tag="pe")
            nc.sync.dma_start(pe_t[:], pe_w[:, s * NSUP:(s + 1) * NSUP, :])
            for r in range(RPC):
                gt = gath_pool.tile([128, NSUP, E], f32, tag="gt")
                for g in range(NSUP):
                    gi = s * NSUP + g
                    nc.gpsimd.indirect_dma_start(
                        out=gt[:, g, :],
                        out_offset=None,
                        in_=encg_flat,
                        in_offset=bass.IndirectOffsetOnAxis(
                            ap=ixo_sb[r][:, gi:gi + 1], axis=0
                        ),
                    )
                nc.vector.tensor_add(gt[:], gt[:], pe_t[:])
                ps = psum_pool.tile([128, 8 * E], f32, tag="ps")
                for g in range(NSUP):
                    gi = s * NSUP + g
                    nc.tensor.matmul(
                        ps[:, g * E:(g + 1) * E],
                        lhsT=p3_sb[r][:, gi * 128:(gi + 1) * 128],
                        rhs=w3_sb[:],
                        start=True, stop=True,
                    )
                nc.vector.tensor_add(
                    gt[:], gt[:], ps[:].rearrange("q (n e) -> q n e", e=E)
                )
                nc.sync.dma_start(
                    out[r, s * NSUP * 128:(s + 1) * NSUP * 128, :].rearrange(
                        "(n p) e -> p n e", p=128
                    ),
                    gt[:],
                )
    nc.compile()
    return nc


def get_nc(fast):
    key = "nc_fast" if fast else "nc_gen"
    if key not in _CACHE:
        _CACHE[key] = build_nc_fast() if fast else build_nc_general()
    return _CACHE[key]


# --------------------------------------------------------------------------
# Host wrapper
# --------------------------------------------------------------------------
def make_in_maps(encoder_out, align_phone, text_phone, pitch, beats,
                 w_pitch, b_pitch, emb_beats, w_pos, b_pos):
    import ml_dtypes

    encoder_out = np.asarray(encoder_out, np.float32)
    pitch = np.asarray(pitch, np.float32)
    beats = np.asarray(beats)
    w_pitch = np.asarray(w_pitch, np.float32)
    w_pos = np.asarray(w_pos, np.float32)

    idx = compute_idx(np.asarray(align_phone), np.asarray(text_phone))
    fast = bool(np.all(idx == (np.arange(T, dtype=np.int32) // DUR)[None, :]))
    if FORCE_GENERAL:
        fast = False

    g_mat = (np.eye(E, dtype=np.float64) + w_pos.astype(np.float64)).astype(np.float32)
    pe = _positional_encoding_f64(T, E)
    pe_proj = pe @ w_pos.astype(np.float64)                          # [T, E]
    bias = (np.asarray(emb_beats[0], np.float64)
            + np.asarray(b_pitch, np.float64)
            + np.asarray(b_pos, np.float64))
    demb = (np.asarray(emb_beats[1], np.float64)
            - np.asarray(emb_beats[0], np.float64)).astype(np.float32)

    if fast:
        pe_tot = (pe_proj + bias[None, :]).astype(np.float32)
        pe_wrap = np.ascontiguousarray(pe_tot.reshape(NGRP, 128, E).swapaxes(0, 1))
        # S_j[k, t'] = 1 iff k == j*16 + t'//8
        rows = np.arange(128)[:, None]
        sj = np.concatenate(
            [(rows == (j * NW + np.arange(128) // DUR)[None, :]) for j in range(NSUP)],
            axis=1,
        ).astype(ml_dtypes.bfloat16)
        g_hi, g_lo = _bf16_split(g_mat)
        wp_hi, wp_lo = _bf16_split(w_pitch[0])
        db_hi, db_lo = _bf16_split(demb)
        w5_rows = np.stack([
            wp_hi, wp_lo, wp_hi,
            db_hi, db_lo,
        ]).astype(ml_dtypes.bfloat16)
        w5 = np.zeros((128, E), ml_dtypes.bfloat16)
        for cb in range(4):
            w5[32 * cb:32 * cb + 5] = w5_rows
        fast_common = {
            "pe_w": pe_wrap, "sj": sj,
            "g_hi": g_hi, "g_lo": g_lo, "w5": w5,
        }
    else:
        w3 = np.stack(
            [w_pitch[0].astype(np.float64), demb.astype(np.float64), bias]
        ).astype(np.float32)
        pe_wl = np.ascontiguousarray(
            pe_proj.astype(np.float32).reshape(NGRP, 128, E).swapaxes(0, 1)
        )

    in_maps = []
    for core in range(NCORES):
        rows_ = range(core * RPC, (core + 1) * RPC)
        enc_t = np.ascontiguousarray(
            encoder_out[core * RPC:(core + 1) * RPC].transpose(0, 2, 1)
        )
        if fast:
            import ml_dtypes as _md
            enc_hi, enc_lo = _bf16_split(enc_t)
            l5 = np.zeros((RPC, 128, T // 4), _md.bfloat16)
            for j, b in enumerate(rows_):
                p_hi, p_lo = _bf16_split(pitch[b, :, 0])
                bt = beats[b, :, 0].astype(_md.bfloat16)
                rows5 = np.stack([p_hi, p_hi, p_lo, bt, bt])     # [5, T]
                for cb in range(4):
                    l5[j, 32 * cb:32 * cb + 5] = rows5[
                        :, cb * (T // 4):(cb + 1) * (T // 4)
                    ]
            m = {"enc_hi": enc_hi, "enc_lo": enc_lo, "l5": l5, **fast_common}
        else:
            p3 = np.empty((RPC, 3, T), np.float32)
            idxo = np.empty((RPC, 128, NGRP), np.int32)
            for j, b in enumerate(rows_):
                p3[j, 0] = pitch[b, :, 0]
                p3[j, 1] = beats[b, :, 0].astype(np.float32)
                p3[j, 2] = 1.0
                idxo[j] = idx[b].reshape(NGRP, 128).T + j * P
            m = {"enc_t": enc_t, "g_mat": g_mat, "pe_w": pe_wl, "p3": p3,
                 "w3": w3, "idxo": idxo}
        in_maps.append(m)
    return fast, in_maps


def kernel(**inputs):
    from concourse.bass_utils import run_bass_kernel_spmd

    fast, in_maps = make_in_maps(**inputs)
    nc = get_nc(fast)
    res = run_bass_kernel_spmd(nc, in_maps, core_ids=list(range(NCORES)))
    out = np.concatenate([res.results[i]["out"] for i in range(NCORES)], axis=0)
    return np.ascontiguousarray(out.astype(np.float32))


# revision 42
# speedup vs baseline: 2.4072x; 1.1133x over previous
"""Trainium2 Bass kernel for nn_Encoder_Postnet (ragged_sequence).

Computation (reference):
    idx   = sequential aligner scan over (align_phone, text_phone)   [B,T]
    out   = enc[idx] + pitch @ w_pitch + b_pitch + emb_beats[beats]
            + (enc[idx] + pe) @ w_pos + b_pos

Key algebraic restructure: the duration-expansion gather commutes with the
E x E linear, so
    out[t] = encG[idx_t] + (pe@w_pos + bias)[t] + pitch[t]*w_pitch + beats[t]*demb
with encG = enc @ (I + w_pos) computed once per batch row ([P,E] not [T,E]),
collapsing the big [B*T,E]@[E,E] matmul 8x and making the kernel memory-bound.

Sharding: pure data parallel, 2 batch rows per core across 8 cores.

Fast path (the uniform duration-8 expansion this model's inputs produce,
idx == arange(T)//8 for every row):
  phase A: encG = enc_row @ (I+w_pos) on PE (fp32); the result stays resident
           in SBUF split into bf16 hi/lo pairs (hi+lo keeps ~17 mantissa
           bits, and bf16 matmuls run 4x faster than fp32 on the PE).
  phase B: per 128-token group, one PSUM accumulation group of 4 bf16
           matmuls: S_j-one-hot expansion of encG rows (hi+lo) + identity
           matmuls adding the (pe@w_pos + bias) tile (hi+lo).  Then the
           pitch rank-1 term rides the DVE op that drains PSUM->SBUF
           (scalar_tensor_tensor), and the beats term runs on the otherwise
           idle GPSIMD.  The only DMA traffic is inputs-once + outputs-once.

General path (arbitrary idx): per-128-token indirect-DMA row gathers
(production-shaped offset [128,1] DynamicAP descriptors) + K=3 stream matmul.

The aligner scan itself is index metadata ([B,T] int32); it is computed on
host with a run-compressed O(B*P) algorithm exactly equivalent to the
reference recurrence, then consumed either as a uniformity proof (fast path)
or as gather offsets (general path).
"""

import sys

for _p in ("/opt/trn_rl_repo",):
    if _p not in sys.path:
        sys.path.insert(0, _p)

import numpy as np

B, P, T, E = 16, 1024, 8192, 256
NCORES = 8
RPC = B // NCORES          # batch rows per core
NGRP = T // 128            # 64 groups of 128 tokens per row
NSUP = 8                   # groups per super-chunk
DUR = T // P               # uniform duration of the fast path (8)
NW = 128 // DUR            # encG rows per group (16)

FORCE_GENERAL = False      # test hook: force the arbitrary-idx path
_CACHE = {}


# --------------------------------------------------------------------------
# Host: aligner index computation (exact replica of the reference recurrence)
# --------------------------------------------------------------------------
def compute_idx(align, text):
    """idx[b,0]=0; idx[b,j] = idx[b,j-1] if align[b,j]==text[b,idx[b,j-1]]
    else min(idx[b,j-1]+1, P-1).   Vectorized over batch via segment starts:
    the pointer advances i->i+1 at s_{i+1} = first j >= s_i+1 with
    align[j] != text[i]; within a run of align values equal to text[i] the
    first mismatch is the run end."""
    align = np.asarray(align)
    text = np.asarray(text)
    Bn, Tn = align.shape
    Pn = text.shape[1]
    diff = align[:, 1:] != align[:, :-1]                       # [B, T-1]
    c = np.full((Bn, Tn), Tn, np.int64)
    c[:, :-1] = np.where(diff, np.arange(1, Tn)[None, :], Tn)
    re = np.flip(np.minimum.accumulate(np.flip(c, axis=1), axis=1), axis=1)

    s = np.full((Bn, Pn), Tn, np.int64)
    s[:, 0] = 0
    cur = np.zeros(Bn, np.int64)
    arB = np.arange(Bn)
    for i in range(Pn - 1):
        j0 = cur + 1
        active = j0 < Tn
        j0c = np.minimum(j0, Tn - 1)
        eq = (align[arB, j0c] == text[:, i]) & active
        nxt = np.where(active, np.where(eq, re[arB, j0c], j0), Tn)
        s[:, i + 1] = nxt
        cur = nxt
    idx = np.empty((Bn, Tn), np.int32)
    pos = np.arange(Tn)
    for b in range(Bn):
        idx[b] = (np.searchsorted(s[b], pos, side="right") - 1).astype(np.int32)
    return idx


def _positional_encoding_f64(t, e):
    pos = np.arange(t, dtype=np.float64)[:, None]
    div = np.exp(np.arange(0, e, 2, dtype=np.float64) * (-np.log(10000.0) / e))
    ang = pos * div[None, :]
    return np.stack([np.sin(ang), np.cos(ang)], axis=-1).reshape(t, e)


def _bf16_split(x):
    import ml_dtypes
    x = np.asarray(x, np.float32)
    hi = x.astype(ml_dtypes.bfloat16)
    lo = (x - hi.astype(np.float32)).astype(ml_dtypes.bfloat16)
    return hi, lo


# --------------------------------------------------------------------------
# Device programs
# --------------------------------------------------------------------------
def build_nc_fast():
    from contextlib import ExitStack
    import concourse.tile as tile
    from concourse import bacc, mybir
    from concourse._compat import get_trn_type

    f32 = mybir.dt.float32
    bf16 = mybir.dt.bfloat16
    mult = mybir.AluOpType.mult
    add = mybir.AluOpType.add

    nc = bacc.Bacc(get_trn_type() or "TRN2", target_bir_lowering=False, debug=False)
    enc_hi = nc.declare_dram_parameter("enc_hi", [RPC, E, P], bf16, isOutput=False)
    enc_lo = nc.declare_dram_parameter("enc_lo", [RPC, E, P], bf16, isOutput=False)
    g_hi = nc.declare_dram_parameter("g_hi", [E, E], bf16, isOutput=False)
    g_lo = nc.declare_dram_parameter("g_lo", [E, E], bf16, isOutput=False)
    pe_w = nc.declare_dram_parameter("pe_w", [128, NGRP, E], f32, isOutput=False)
    sj_d = nc.declare_dram_parameter("sj", [128, NSUP * 128], bf16, isOutput=False)
    # stream-term lhsT rows banked at 32-partition strides (4 token-chunks)
    # to keep the per-partition footprint small for the DMA
    l5_d = nc.declare_dram_parameter("l5", [RPC, 128, T // 4], bf16, isOutput=False)
    w5_d = nc.declare_dram_parameter("w5", [128, E], bf16, isOutput=False)
    out = nc.declare_dram_parameter("out", [RPC, T, E], f32, isOutput=True)

    with tile.TileContext(nc) as tc, ExitStack() as ctx:
        const = ctx.enter_context(tc.tile_pool(name="const", bufs=1))
        pe_pool = ctx.enter_context(tc.tile_pool(name="pe", bufs=8))
        out_pool = ctx.enter_context(tc.tile_pool(name="outp", bufs=4))

        sj_sb = const.tile([128, NSUP * 128], bf16, tag="sj")
        nc.sync.dma_start(sj_sb[:], sj_d[:])
        w5_sb = const.tile([128, E], bf16, tag="w5")
        nc.sync.dma_start(w5_sb[:], w5_d[:])
        l5_sb, egh_keep, egl_keep = [], [], []
        for r in range(RPC):
            l5t = const.tile([128, T // 4], bf16, tag=f"l5_{r}")
            nc.sync.dma_start(l5t[:], l5_d[r])
            l5_sb.append(l5t)
            egh_keep.append(
                const.tile([128, NSUP, E], bf16, tag=f"egh{r}", name=f"egh{r}")
            )
            egl_keep.append(
                const.tile([128, NSUP, E], bf16, tag=f"egl{r}", name=f"egl{r}")
            )

        # ---- phase A: encG = enc @ (I+w_pos) as 3-term bf16-split matmuls;
        # result kept in SBUF as bf16 hi/lo.  psum layout [128p, m, e] ==
        # keep layout: row m*128+p at (partition p, block m).
        gh0 = const.tile([128, E], bf16, tag="gh0", name="gh0")
        gh1 = const.tile([128, E], bf16, tag="gh1", name="gh1")
        gl0 = const.tile([128, E], bf16, tag="gl0", name="gl0")
        gl1 = const.tile([128, E], bf16, tag="gl1", name="gl1")
        nc.sync.dma_start(gh0[:], g_hi[0:128, :])
        nc.sync.dma_start(gh1[:], g_hi[128:256, :])
        nc.sync.dma_start(gl0[:], g_lo[0:128, :])
        nc.sync.dma_start(gl1[:], g_lo[128:256, :])
        psum_a = ctx.enter_context(tc.tile_pool(name="psumA", bufs=1, space="PSUM"))
        psum_b = ctx.enter_context(tc.tile_pool(name="psumB", bufs=6, space="PSUM"))
        with tc.tile_pool(name="encT", bufs=2) as encT_pool:
            for r in range(RPC):
                eh0 = encT_pool.tile([128, P], bf16, tag="eh0")
                eh1 = encT_pool.tile([128, P], bf16, tag="eh1")
                el0 = encT_pool.tile([128, P], bf16, tag="el0")
                el1 = encT_pool.tile([128, P], bf16, tag="el1")
                nc.sync.dma_start(eh0[:], enc_hi[r, 0:128, :])
                nc.sync.dma_start(eh1[:], enc_hi[r, 128:256, :])
                nc.sync.dma_start(el0[:], enc_lo[r, 0:128, :])
                nc.sync.dma_start(el1[:], enc_lo[r, 128:256, :])
                for mh in range(2):
                    ps = psum_a.tile([128, 4 * E], f32, tag="psA", name="psA")
                    for mi in range(4):
                        m = mh * 4 + mi
                        sl = slice(m * 128, (m + 1) * 128)
                        terms = [
                            (eh0, gh0, True, False), (eh1, gh1, False, False),
                            (el0, gh0, False, False), (el1, gh1, False, False),
                            (eh0, gl0, False, False), (eh1, gl1, False, True),
                        ]
                        for lt, gt_, st, sp in terms:
                            nc.tensor.matmul(
                                ps[:, mi * E:(mi + 1) * E],
                                lhsT=lt[:, sl], rhs=gt_[:], start=st, stop=sp,
                            )
                        # drain per m-chunk so phase B super-chunk m can start
                        # before the rest of phase A finishes
                        hi = egh_keep[r][:, m, :]
                        nc.vector.tensor_copy(hi, ps[:, mi * E:(mi + 1) * E])
                        nc.vector.scalar_tensor_tensor(
                            out=egl_keep[r][:, m, :], in0=hi, scalar=-1.0,
                            in1=ps[:, mi * E:(mi + 1) * E], op0=mult, op1=add,
                        )

        # ---- phase B: per group, 5 bf16 matmuls into one PSUM group:
        # expansion hi/lo + pe hi/lo + the K=5 rank-1 stream matmul
        # (pitch_hi/lo x w_pitch_hi/lo cross terms + beats x demb_hi/lo).
        if True:
            for s in range(T // (NSUP * 128)):
                pe_t = pe_pool.tile([128, NSUP, E], f32, tag="pe")
                nc.sync.dma_start(pe_t[:], pe_w[:, s * NSUP:(s + 1) * NSUP, :])
                for r in range(RPC):
                    ot = out_pool.tile([128, NSUP, E], f32, tag="ot")
                    for j in range(NSUP):
                        g = s * NSUP + j
                        ps = psum_b.tile([128, E], f32, tag="ps")
                        sj_ap = sj_sb[:, j * 128:(j + 1) * 128]
                        nc.tensor.matmul(
                            ps[:], lhsT=sj_ap, rhs=egh_keep[r][:, s, :],
                            start=True, stop=False,
                        )
                        nc.tensor.matmul(
                            ps[:], lhsT=sj_ap, rhs=egl_keep[r][:, s, :],
                            start=False, stop=False,
                        )
                        cb = g // (NGRP // 4)          # token-chunk bank
                        u0 = (g % (NGRP // 4)) * 128
                        nc.tensor.matmul(
                            ps[:],
                            lhsT=l5_sb[r][32 * cb:32 * cb + 5, u0:u0 + 128],
                            rhs=w5_sb[32 * cb:32 * cb + 5, :],
                            start=False, stop=True,
                            tile_position=(32 * cb, 0),
                        )
                        # drain PSUM -> SBUF fused with the (pe@w_pos+bias) add
                        nc.vector.tensor_add(ot[:, j, :], ps[:], pe_t[:, j, :])
                    # out-stores go via the ACT HWDGE queue so they don't
                    # head-of-line-block SP's input loads
                    nc.scalar.dma_start(
                        out[r, s * NSUP * 128:(s + 1) * NSUP * 128, :].rearrange(
                            "(n p) e -> p n e", p=128
                        ),
                        ot[:],
                    )
    nc.compile()
    return nc


def build_nc_general():
    """Arbitrary-idx path: per-128-token indirect row gathers."""
    import concourse.bass as bass
    from contextlib import ExitStack
    import concourse.tile as tile
    from concourse import bacc, mybir
    from concourse._compat import get_trn_type

    f32 = mybir.dt.float32
    i32 = mybir.dt.int32

    nc = bacc.Bacc(get_trn_type() or "TRN2", target_bir_lowering=False, debug=False)
    enc_t = nc.declare_dram_parameter("enc_t", [RPC, E, P], f32, isOutput=False)
    g_mat = nc.declare_dram_parameter("g_mat", [E, E], f32, isOutput=False)
    pe_w = nc.declare_dram_parameter("pe_w", [128, NGRP, E], f32, isOutput=False)
    p3 = nc.declare_dram_parameter("p3", [RPC, 3, T], f32, isOutput=False)
    w3 = nc.declare_dram_parameter("w3", [3, E], f32, isOutput=False)
    idxo = nc.declare_dram_parameter(
        "idxo", [RPC, 128, NGRP], i32, isOutput=False
    )
    out = nc.declare_dram_parameter("out", [RPC, T, E], f32, isOutput=True)
    encg = nc.dram_tensor("encg", [RPC, P, E], f32)

    with tile.TileContext(nc) as tc, ExitStack() as ctx:
        const = ctx.enter_context(tc.tile_pool(name="const", bufs=1))
        encT_pool = ctx.enter_context(tc.tile_pool(name="encT", bufs=2))
        psum_pool = ctx.enter_context(tc.tile_pool(name="psum", bufs=2, space="PSUM"))
        eg_pool = ctx.enter_context(tc.tile_pool(name="eg", bufs=2))
        pe_pool = ctx.enter_context(tc.tile_pool(name="pe", bufs=2))
        gath_pool = ctx.enter_context(tc.tile_pool(name="gath", bufs=3))

        g0 = const.tile([128, E], f32, tag="g0")
        g1 = const.tile([128, E], f32, tag="g1")
        nc.sync.dma_start(g0[:], g_mat[0:128, :])
        nc.sync.dma_start(g1[:], g_mat[128:256, :])
        w3_sb = const.tile([3, E], f32, tag="w3")
        nc.sync.dma_start(w3_sb[:], w3[:, :])
        p3_sb = []
        ixo_sb = []
        for r in range(RPC):
            p3t = const.tile([3, T], f32, tag=f"p3_{r}")
            nc.sync.dma_start(p3t[:], p3[r])
            p3_sb.append(p3t)
            ixt = const.tile([128, NGRP], i32, tag=f"ixo_{r}")
            nc.sync.dma_start(ixt[:], idxo[r])
            ixo_sb.append(ixt)

        for r in range(RPC):
            et0 = encT_pool.tile([128, P], f32, tag="et0")
            et1 = encT_pool.tile([128, P], f32, tag="et1")
            nc.sync.dma_start(et0[:], enc_t[r, 0:128, :])
            nc.sync.dma_start(et1[:], enc_t[r, 128:256, :])
            ps = psum_pool.tile([128, 8 * E], f32, tag="ps")
            for m in range(8):
                nc.tensor.matmul(
                    ps[:, m * E:(m + 1) * E],
                    lhsT=et0[:, m * 128:(m + 1) * 128],
                    rhs=g0[:], start=True, stop=False,
                )
                nc.tensor.matmul(
                    ps[:, m * E:(m + 1) * E],
                    lhsT=et1[:, m * 128:(m + 1) * 128],
                    rhs=g1[:], start=False, stop=True,
                )
            eg = eg_pool.tile([128, 8 * E], f32, tag="eg")
            nc.vector.tensor_copy(eg[:], ps[:])
            nc.sync.dma_start(
                encg[r].rearrange("(m p) e -> p m e", p=128),
                eg[:].rearrange("q (m e) -> q m e", e=E),
            )

        encg_flat = encg[:].rearrange("r p e -> (r p) e")
        for s in range(T // (NSUP * 128)):
            pe_t = pe_pool.tile([128, NSUP, E], f32, tag="pe")
            nc.sync.dma_start(pe_t[:], pe_w[:, s * NSUP:(s + 1) * NSUP, :])
            for r in range(RPC):
                gt = gath_pool.tile([128, NSUP, E], f32, tag="gt")
                for g in range(NSUP):
                    gi = s * NSUP + g
                    nc.gpsimd.indirect_dma_start(
                        out=gt[:, g, :],
                        out_offset=None,
                        in_=encg_flat,
                        in_offset=bass.IndirectOffsetOnAxis(
                            ap=ixo_sb[r][:, gi:gi + 1], axis=0
                        ),
                    )
                nc.vector.tensor_add(gt[:], gt[:], pe_t[:])
                ps = psum_pool.tile([128, 8 * E], f32, tag="ps")
                for g in range(NSUP):
                    gi = s * NSUP + g
                    nc.tensor.matmul(
                        ps[:, g * E:(g + 1) * E],
                        lhsT=p3_sb[r][:, gi * 128:(gi + 1) * 128],
                        rhs=w3_sb[:],
                        start=True, stop=True,
                    )
                nc.vector.tensor_add(
                    gt[:], gt[:], ps[:].rearrange("q (n e) -> q n e", e=E)
                )
                nc.sync.dma_start(
                    out[r, s * NSUP * 128:(s + 1) * NSUP * 128, :].rearrange(
                        "(n p) e -> p n e", p=128
                    ),
                    gt[:],
                )
    nc.compile()
    return nc


def get_nc(fast):
    key = "nc_fast" if fast else "nc_gen"
    if key not in _CACHE:
        _CACHE[key] = build_nc_fast() if fast else build_nc_general()
    return _CACHE[key]


# --------------------------------------------------------------------------
# Host wrapper
# --------------------------------------------------------------------------
def make_in_maps(encoder_out, align_phone, text_phone, pitch, beats,
                 w_pitch, b_pitch, emb_beats, w_pos, b_pos):
    import ml_dtypes

    encoder_out = np.asarray(encoder_out, np.float32)
    pitch = np.asarray(pitch, np.float32)
    beats = np.asarray(beats)
    w_pitch = np.asarray(w_pitch, np.float32)
    w_pos = np.asarray(w_pos, np.float32)

    idx = compute_idx(np.asarray(align_phone), np.asarray(text_phone))
    fast = bool(np.all(idx == (np.arange(T, dtype=np.int32) // DUR)[None, :]))
    if FORCE_GENERAL:
        fast = False

    g_mat = (np.eye(E, dtype=np.float64) + w_pos.astype(np.float64)).astype(np.float32)
    pe = _positional_encoding_f64(T, E)
    pe_proj = pe @ w_pos.astype(np.float64)                          # [T, E]
    bias = (np.asarray(emb_beats[0], np.float64)
            + np.asarray(b_pitch, np.float64)
            + np.asarray(b_pos, np.float64))
    demb = (np.asarray(emb_beats[1], np.float64)
            - np.asarray(emb_beats[0], np.float64)).astype(np.float32)

    if fast:
        pe_tot = (pe_proj + bias[None, :]).astype(np.float32)
        pe_wrap = np.ascontiguousarray(pe_tot.reshape(NGRP, 128, E).swapaxes(0, 1))
        # S_j[k, t'] = 1 iff k == j*16 + t'//8
        rows = np.arange(128)[:, None]
        sj = np.concatenate(
            [(rows == (j * NW + np.arange(128) // DUR)[None, :]) for j in range(NSUP)],
            axis=1,
        ).astype(ml_dtypes.bfloat16)
        g_hi, g_lo = _bf16_split(g_mat)
        wp_hi, wp_lo = _bf16_split(w_pitch[0])
        db_hi, db_lo = _bf16_split(demb)
        w5_rows = np.stack([
            wp_hi, wp_lo, wp_hi,
            db_hi, db_lo,
        ]).astype(ml_dtypes.bfloat16)
        w5 = np.zeros((128, E), ml_dtypes.bfloat16)
        for cb in range(4):
            w5[32 * cb:32 * cb + 5] = w5_rows
        fast_common = {
            "pe_w": pe_wrap, "sj": sj,
            "g_hi": g_hi, "g_lo": g_lo, "w5": w5,
        }
    else:
        w3 = np.stack(
            [w_pitch[0].astype(np.float64), demb.astype(np.float64), bias]
        ).astype(np.float32)
        pe_wl = np.ascontiguousarray(
            pe_proj.astype(np.float32).reshape(NGRP, 128, E).swapaxes(0, 1)
        )

    in_maps = []
    for core in range(NCORES):
        rows_ = range(core * RPC, (core + 1) * RPC)
        enc_t = np.ascontiguousarray(
            encoder_out[core * RPC:(core + 1) * RPC].transpose(0, 2, 1)
        )
        if fast:
            import ml_dtypes as _md
            enc_hi, enc_lo = _bf16_split(enc_t)
            l5 = np.zeros((RPC, 128, T // 4), _md.bfloat16)
            for j, b in enumerate(rows_):
                p_hi, p_lo = _bf16_split(pitch[b, :, 0])
                bt = beats[b, :, 0].astype(_md.bfloat16)
                rows5 = np.stack([p_hi, p_hi, p_lo, bt, bt])     # [5, T]
                for cb in range(4):
                    l5[j, 32 * cb:32 * cb + 5] = rows5[
                        :, cb * (T // 4):(cb + 1) * (T // 4)
                    ]
            m = {"enc_hi": enc_hi, "enc_lo": enc_lo, "l5": l5, **fast_common}
        else:
            p3 = np.empty((RPC, 3, T), np.float32)
            idxo = np.empty((RPC, 128, NGRP), np.int32)
            for j, b in enumerate(rows_):
                p3[j, 0] = pitch[b, :, 0]
                p3[j, 1] = beats[b, :, 0].astype(np.float32)
                p3[j, 2] = 1.0
                idxo[j] = idx[b].reshape(NGRP, 128).T + j * P
            m = {"enc_t": enc_t, "g_mat": g_mat, "pe_w": pe_wl, "p3": p3,
                 "w3": w3, "idxo": idxo}
        in_maps.append(m)
    return fast, in_maps


def kernel(**inputs):
    from concourse.bass_utils import run_bass_kernel_spmd

    fast, in_maps = make_in_maps(**inputs)
    nc = get_nc(fast)
    res = run_bass_kernel_spmd(nc, in_maps, core_ids=list(range(NCORES)))
    out = np.concatenate([res.results[i]["out"] for i in range(NCORES)], axis=0)
    return np.ascontiguousarray(out.astype(np.float32))


# revision 43
# speedup vs baseline: 2.4987x; 1.0380x over previous
"""Trainium2 Bass kernel for nn_Encoder_Postnet (ragged_sequence).

Computation (reference):
    idx   = sequential aligner scan over (align_phone, text_phone)   [B,T]
    out   = enc[idx] + pitch @ w_pitch + b_pitch + emb_beats[beats]
            + (enc[idx] + pe) @ w_pos + b_pos

Key algebraic restructure: the duration-expansion gather commutes with the
E x E linear, so
    out[t] = encG[idx_t] + (pe@w_pos + bias)[t] + pitch[t]*w_pitch + beats[t]*demb
with encG = enc @ (I + w_pos) computed once per batch row ([P,E] not [T,E]),
collapsing the big [B*T,E]@[E,E] matmul 8x and making the kernel memory-bound.

Sharding: pure data parallel, 2 batch rows per core across 8 cores.

Fast path (the uniform duration-8 expansion this model's inputs produce,
idx == arange(T)//8 for every row):
  phase A: encG = enc_row @ (I+w_pos) on PE (fp32); the result stays resident
           in SBUF split into bf16 hi/lo pairs (hi+lo keeps ~17 mantissa
           bits, and bf16 matmuls run 4x faster than fp32 on the PE).
  phase B: per 128-token group, one PSUM accumulation group of 4 bf16
           matmuls: S_j-one-hot expansion of encG rows (hi+lo) + identity
           matmuls adding the (pe@w_pos + bias) tile (hi+lo).  Then the
           pitch rank-1 term rides the DVE op that drains PSUM->SBUF
           (scalar_tensor_tensor), and the beats term runs on the otherwise
           idle GPSIMD.  The only DMA traffic is inputs-once + outputs-once.

General path (arbitrary idx): per-128-token indirect-DMA row gathers
(production-shaped offset [128,1] DynamicAP descriptors) + K=3 stream matmul.

The aligner scan itself is index metadata ([B,T] int32); it is computed on
host with a run-compressed O(B*P) algorithm exactly equivalent to the
reference recurrence, then consumed either as a uniformity proof (fast path)
or as gather offsets (general path).
"""

import sys

for _p in ("/opt/trn_rl_repo",):
    if _p not in sys.path:
        sys.path.insert(0, _p)

import numpy as np

B, P, T, E = 16, 1024, 8192, 256
NCORES = 8
RPC = B // NCORES          # batch rows per core
NGRP = T // 128            # 64 groups of 128 tokens per row
NSUP = 8                   # groups per super-chunk
DUR = T // P               # uniform duration of the fast path (8)
NW = 128 // DUR            # encG rows per group (16)

FORCE_GENERAL = False      # test hook: force the arbitrary-idx path
_CACHE = {}


# --------------------------------------------------------------------------
# Host: aligner index computation (exact replica of the reference recurrence)
# --------------------------------------------------------------------------
def compute_idx(align, text):
    """idx[b,0]=0; idx[b,j] = idx[b,j-1] if align[b,j]==text[b,idx[b,j-1]]
    else min(idx[b,j-1]+1, P-1).   Vectorized over batch via segment starts:
    the pointer advances i->i+1 at s_{i+1} = first j >= s_i+1 with
    align[j] != text[i]; within a run of align values equal to text[i] the
    first mismatch is the run end."""
    align = np.asarray(align)
    text = np.asarray(text)
    Bn, Tn = align.shape
    Pn = text.shape[1]
    diff = align[:, 1:] != align[:, :-1]                       # [B, T-1]
    c = np.full((Bn, Tn), Tn, np.int64)
    c[:, :-1] = np.where(diff, np.arange(1, Tn)[None, :], Tn)
    re = np.flip(np.minimum.accumulate(np.flip(c, axis=1), axis=1), axis=1)

    s = np.full((Bn, Pn), Tn, np.int64)
    s[:, 0] = 0
    cur = np.zeros(Bn, np.int64)
    arB = np.arange(Bn)
    for i in range(Pn - 1):
        j0 = cur + 1
        active = j0 < Tn
        j0c = np.minimum(j0, Tn - 1)
        eq = (align[arB, j0c] == text[:, i]) & active
        nxt = np.where(active, np.where(eq, re[arB, j0c], j0), Tn)
        s[:, i + 1] = nxt
        cur = nxt
    idx = np.empty((Bn, Tn), np.int32)
    pos = np.arange(Tn)
    for b in range(Bn):
        idx[b] = (np.searchsorted(s[b], pos, side="right") - 1).astype(np.int32)
    return idx


def _positional_encoding_f64(t, e):
    pos = np.arange(t, dtype=np.float64)[:, None]
    div = np.exp(np.arange(0, e, 2, dtype=np.float64) * (-np.log(10000.0) / e))
    ang = pos * div[None, :]
    return np.stack([np.sin(ang), np.cos(ang)], axis=-1).reshape(t, e)


def _bf16_split(x):
    import ml_dtypes
    x = np.asarray(x, np.float32)
    hi = x.astype(ml_dtypes.bfloat16)
    lo = (x - hi.astype(np.float32)).astype(ml_dtypes.bfloat16)
    return hi, lo


# --------------------------------------------------------------------------
# Device programs
# --------------------------------------------------------------------------
def build_nc_fast():
    from contextlib import ExitStack
    import concourse.tile as tile
    from concourse import bacc, mybir
    from concourse._compat import get_trn_type

    f32 = mybir.dt.float32
    bf16 = mybir.dt.bfloat16
    mult = mybir.AluOpType.mult
    add = mybir.AluOpType.add

    nc = bacc.Bacc(get_trn_type() or "TRN2", target_bir_lowering=False, debug=False)
    enc_hi = nc.declare_dram_parameter("enc_hi", [RPC, E, P], bf16, isOutput=False)
    enc_lo = nc.declare_dram_parameter("enc_lo", [RPC, E, P], bf16, isOutput=False)
    g_hi = nc.declare_dram_parameter("g_hi", [E, E], bf16, isOutput=False)
    g_lo = nc.declare_dram_parameter("g_lo", [E, E], bf16, isOutput=False)
    pe_w = nc.declare_dram_parameter("pe_w", [128, NGRP, E], f32, isOutput=False)
    sj_d = nc.declare_dram_parameter("sj", [128, NSUP * 128], bf16, isOutput=False)
    # stream-term lhsT rows banked at 32-partition strides (4 token-chunks)
    # to keep the per-partition footprint small for the DMA
    l5_d = nc.declare_dram_parameter("l5", [RPC, 128, T // 4], bf16, isOutput=False)
    w5_d = nc.declare_dram_parameter("w5", [128, E], bf16, isOutput=False)
    out = nc.declare_dram_parameter("out", [RPC, T, E], f32, isOutput=True)

    with tile.TileContext(nc) as tc, ExitStack() as ctx:
        const = ctx.enter_context(tc.tile_pool(name="const", bufs=1))
        pe_pool = ctx.enter_context(tc.tile_pool(name="pe", bufs=8))
        out_pool = ctx.enter_context(tc.tile_pool(name="outp", bufs=4))

        sj_sb = const.tile([128, NSUP * 128], bf16, tag="sj")
        nc.sync.dma_start(sj_sb[:], sj_d[:])
        w5_sb = const.tile([128, E], bf16, tag="w5")
        nc.sync.dma_start(w5_sb[:], w5_d[:])
        l5_sb, egh_keep, egl_keep = [], [], []
        for r in range(RPC):
            l5t = const.tile([128, T // 4], bf16, tag=f"l5_{r}")
            nc.gpsimd.dma_start(l5t[:], l5_d[r])
            l5_sb.append(l5t)
            egh_keep.append(
                const.tile([128, NSUP, E], bf16, tag=f"egh{r}", name=f"egh{r}")
            )
            egl_keep.append(
                const.tile([128, NSUP, E], bf16, tag=f"egl{r}", name=f"egl{r}")
            )

        # ---- phase A: encG = enc @ (I+w_pos) as 3-term bf16-split matmuls;
        # result kept in SBUF as bf16 hi/lo.  psum layout [128p, m, e] ==
        # keep layout: row m*128+p at (partition p, block m).
        gh0 = const.tile([128, E], bf16, tag="gh0", name="gh0")
        gh1 = const.tile([128, E], bf16, tag="gh1", name="gh1")
        gl0 = const.tile([128, E], bf16, tag="gl0", name="gl0")
        gl1 = const.tile([128, E], bf16, tag="gl1", name="gl1")
        nc.sync.dma_start(gh0[:], g_hi[0:128, :])
        nc.sync.dma_start(gh1[:], g_hi[128:256, :])
        nc.sync.dma_start(gl0[:], g_lo[0:128, :])
        nc.sync.dma_start(gl1[:], g_lo[128:256, :])
        psum_a = ctx.enter_context(tc.tile_pool(name="psumA", bufs=1, space="PSUM"))
        psum_b = ctx.enter_context(tc.tile_pool(name="psumB", bufs=6, space="PSUM"))
        with tc.tile_pool(name="encT", bufs=2) as encT_pool:
            for r in range(RPC):
                eh0 = encT_pool.tile([128, P], bf16, tag="eh0")
                eh1 = encT_pool.tile([128, P], bf16, tag="eh1")
                el0 = encT_pool.tile([128, P], bf16, tag="el0")
                el1 = encT_pool.tile([128, P], bf16, tag="el1")
                nc.sync.dma_start(eh0[:], enc_hi[r, 0:128, :])
                nc.sync.dma_start(eh1[:], enc_hi[r, 128:256, :])
                nc.sync.dma_start(el0[:], enc_lo[r, 0:128, :])
                nc.sync.dma_start(el1[:], enc_lo[r, 128:256, :])
                for mh in range(2):
                    ps = psum_a.tile([128, 4 * E], f32, tag="psA", name="psA")
                    for mi in range(4):
                        m = mh * 4 + mi
                        sl = slice(m * 128, (m + 1) * 128)
                        terms = [
                            (eh0, gh0, True, False), (eh1, gh1, False, False),
                            (el0, gh0, False, False), (el1, gh1, False, False),
                            (eh0, gl0, False, False), (eh1, gl1, False, True),
                        ]
                        for lt, gt_, st, sp in terms:
                            nc.tensor.matmul(
                                ps[:, mi * E:(mi + 1) * E],
                                lhsT=lt[:, sl], rhs=gt_[:], start=st, stop=sp,
                            )
                        # drain per m-chunk so phase B super-chunk m can start
                        # before the rest of phase A finishes
                        hi = egh_keep[r][:, m, :]
                        nc.vector.tensor_copy(hi, ps[:, mi * E:(mi + 1) * E])
                        nc.vector.scalar_tensor_tensor(
                            out=egl_keep[r][:, m, :], in0=hi, scalar=-1.0,
                            in1=ps[:, mi * E:(mi + 1) * E], op0=mult, op1=add,
                        )

        # ---- phase B: per group, 5 bf16 matmuls into one PSUM group:
        # expansion hi/lo + pe hi/lo + the K=5 rank-1 stream matmul
        # (pitch_hi/lo x w_pitch_hi/lo cross terms + beats x demb_hi/lo).
        if True:
            for s in range(T // (NSUP * 128)):
                pe_t = pe_pool.tile([128, NSUP, E], f32, tag="pe")
                nc.sync.dma_start(pe_t[:], pe_w[:, s * NSUP:(s + 1) * NSUP, :])
                for r in range(RPC):
                    ot = out_pool.tile([128, NSUP, E], f32, tag="ot")
                    for j in range(NSUP):
                        g = s * NSUP + j
                        ps = psum_b.tile([128, E], f32, tag="ps")
                        sj_ap = sj_sb[:, j * 128:(j + 1) * 128]
                        nc.tensor.matmul(
                            ps[:], lhsT=sj_ap, rhs=egh_keep[r][:, s, :],
                            start=True, stop=False,
                        )
                        nc.tensor.matmul(
                            ps[:], lhsT=sj_ap, rhs=egl_keep[r][:, s, :],
                            start=False, stop=False,
                        )
                        cb = g // (NGRP // 4)          # token-chunk bank
                        u0 = (g % (NGRP // 4)) * 128
                        nc.tensor.matmul(
                            ps[:],
                            lhsT=l5_sb[r][32 * cb:32 * cb + 5, u0:u0 + 128],
                            rhs=w5_sb[32 * cb:32 * cb + 5, :],
                            start=False, stop=True,
                            tile_position=(32 * cb, 0),
                        )
                        # drain PSUM -> SBUF fused with the (pe@w_pos+bias) add
                        nc.vector.tensor_add(ot[:, j, :], ps[:], pe_t[:, j, :])
                    # out-stores go via the ACT HWDGE queue so they don't
                    # head-of-line-block SP's input loads
                    nc.scalar.dma_start(
                        out[r, s * NSUP * 128:(s + 1) * NSUP * 128, :].rearrange(
                            "(n p) e -> p n e", p=128
                        ),
                        ot[:],
                    )
    nc.compile()
    return nc


def build_nc_general():
    """Arbitrary-idx path: per-128-token indirect row gathers."""
    import concourse.bass as bass
    from contextlib import ExitStack
    import concourse.tile as tile
    from concourse import bacc, mybir
    from concourse._compat import get_trn_type

    f32 = mybir.dt.float32
    i32 = mybir.dt.int32

    nc = bacc.Bacc(get_trn_type() or "TRN2", target_bir_lowering=False, debug=False)
    enc_t = nc.declare_dram_parameter("enc_t", [RPC, E, P], f32, isOutput=False)
    g_mat = nc.declare_dram_parameter("g_mat", [E, E], f32, isOutput=False)
    pe_w = nc.declare_dram_parameter("pe_w", [128, NGRP, E], f32, isOutput=False)
    p3 = nc.declare_dram_parameter("p3", [RPC, 3, T], f32, isOutput=False)
    w3 = nc.declare_dram_parameter("w3", [3, E], f32, isOutput=False)
    idxo = nc.declare_dram_parameter(
        "idxo", [RPC, 128, NGRP], i32, isOutput=False
    )
    out = nc.declare_dram_parameter("out", [RPC, T, E], f32, isOutput=True)
    encg = nc.dram_tensor("encg", [RPC, P, E], f32)

    with tile.TileContext(nc) as tc, ExitStack() as ctx:
        const = ctx.enter_context(tc.tile_pool(name="const", bufs=1))
        encT_pool = ctx.enter_context(tc.tile_pool(name="encT", bufs=2))
        psum_pool = ctx.enter_context(tc.tile_pool(name="psum", bufs=2, space="PSUM"))
        eg_pool = ctx.enter_context(tc.tile_pool(name="eg", bufs=2))
        pe_pool = ctx.enter_context(tc.tile_pool(name="pe", bufs=2))
        gath_pool = ctx.enter_context(tc.tile_pool(name="gath", bufs=3))

        g0 = const.tile([128, E], f32, tag="g0")
        g1 = const.tile([128, E], f32, tag="g1")
        nc.sync.dma_start(g0[:], g_mat[0:128, :])
        nc.sync.dma_start(g1[:], g_mat[128:256, :])
        w3_sb = const.tile([3, E], f32, tag="w3")
        nc.sync.dma_start(w3_sb[:], w3[:, :])
        p3_sb = []
        ixo_sb = []
        for r in range(RPC):
            p3t = const.tile([3, T], f32, tag=f"p3_{r}")
            nc.sync.dma_start(p3t[:], p3[r])
            p3_sb.append(p3t)
            ixt = const.tile([128, NGRP], i32, tag=f"ixo_{r}")
            nc.sync.dma_start(ixt[:], idxo[r])
            ixo_sb.append(ixt)

        for r in range(RPC):
            et0 = encT_pool.tile([128, P], f32, tag="et0")
            et1 = encT_pool.tile([128, P], f32, tag="et1")
            nc.sync.dma_start(et0[:], enc_t[r, 0:128, :])
            nc.sync.dma_start(et1[:], enc_t[r, 128:256, :])
            ps = psum_pool.tile([128, 8 * E], f32, tag="ps")
            for m in range(8):
                nc.tensor.matmul(
                    ps[:, m * E:(m + 1) * E],
                    lhsT=et0[:, m * 128:(m + 1) * 128],
                    rhs=g0[:], start=True, stop=False,
                )
                nc.tensor.matmul(
                    ps[:, m * E:(m + 1) * E],
                    lhsT=et1[:, m * 128:(m + 1) * 128],
                    rhs=g1[:], start=False, stop=True,
                )
            eg = eg_pool.tile([128, 8 * E], f32, tag="eg")
            nc.vector.tensor_copy(eg[:], ps[:])
            nc.sync.dma_start(
                encg[r].rearrange("(m p) e -> p m e", p=128),
                eg[:].rearrange("q (m e) -> q m e", e=E),
            )

        encg_flat = encg[:].rearrange("r p e -> (r p) e")
        for s in range(T // (NSUP * 128)):
            pe_t = pe_pool.tile([128, NSUP, E], f32, tag="pe")
            nc.sync.dma_start(pe_t[:], pe_w[:, s * NSUP:(s + 1) * NSUP, :])
            for r in range(RPC):
                gt = gath_pool.tile([128, NSUP, E], f32, tag="gt")
                for g in range(NSUP):
                    gi = s * NSUP + g
                    nc.gpsimd.indirect_dma_start(
                        out=gt[:, g, :],
                        out_offset=None,
                        in_=encg_flat,
                        in_offset=bass.IndirectOffsetOnAxis(
                            ap=ixo_sb[r][:, gi:gi + 1], axis=0
                        ),
                    )
                nc.vector.tensor_add(gt[:], gt[:], pe_t[:])
                ps = psum_pool.tile([128, 8 * E], f32, tag="ps")
                for g in range(NSUP):
                    gi = s * NSUP + g
                    nc.tensor.matmul(
                        ps[:, g * E:(g + 1) * E],
                        lhsT=p3_sb[r][:, gi * 128:(gi + 1) * 128],
                        rhs=w3_sb[:],
                        start=True, stop=True,
                    )
                nc.vector.tensor_add(
                    gt[:], gt[:], ps[:].rearrange("q (n e) -> q n e", e=E)
                )
                nc.sync.dma_start(
                    out[r, s * NSUP * 128:(s + 1) * NSUP * 128, :].rearrange(
                        "(n p) e -> p n e", p=128
                    ),
                    gt[:],
                )
    nc.compile()
    return nc


def get_nc(fast):
    key = "nc_fast" if fast else "nc_gen"
    if key not in _CACHE:
        _CACHE[key] = build_nc_fast() if fast else build_nc_general()
    return _CACHE[key]


# --------------------------------------------------------------------------
# Host wrapper
# --------------------------------------------------------------------------
def make_in_maps(encoder_out, align_phone, text_phone, pitch, beats,
                 w_pitch, b_pitch, emb_beats, w_pos, b_pos):
    import ml_dtypes

    encoder_out = np.asarray(encoder_out, np.float32)
    pitch = np.asarray(pitch, np.float32)
    beats = np.asarray(beats)
    w_pitch = np.asarray(w_pitch, np.float32)
    w_pos = np.asarray(w_pos, np.float32)

    idx = compute_idx(np.asarray(align_phone), np.asarray(text_phone))
    fast = bool(np.all(idx == (np.arange(T, dtype=np.int32) // DUR)[None, :]))
    if FORCE_GENERAL:
        fast = False

    g_mat = (np.eye(E, dtype=np.float64) + w_pos.astype(np.float64)).astype(np.float32)
    pe = _positional_encoding_f64(T, E)
    pe_proj = pe @ w_pos.astype(np.float64)                          # [T, E]
    bias = (np.asarray(emb_beats[0], np.float64)
            + np.asarray(b_pitch, np.float64)
            + np.asarray(b_pos, np.float64))
    demb = (np.asarray(emb_beats[1], np.float64)
            - np.asarray(emb_beats[0], np.float64)).astype(np.float32)

    if fast:
        pe_tot = (pe_proj + bias[None, :]).astype(np.float32)
        pe_wrap = np.ascontiguousarray(pe_tot.reshape(NGRP, 128, E).swapaxes(0, 1))
        # S_j[k, t'] = 1 iff k == j*16 + t'//8
        rows = np.arange(128)[:, None]
        sj = np.concatenate(
            [(rows == (j * NW + np.arange(128) // DUR)[None, :]) for j in range(NSUP)],
            axis=1,
        ).astype(ml_dtypes.bfloat16)
        g_hi, g_lo = _bf16_split(g_mat)
        wp_hi, wp_lo = _bf16_split(w_pitch[0])
        db_hi, db_lo = _bf16_split(demb)
        w5_rows = np.stack([
            wp_hi, wp_lo, wp_hi,
            db_hi, db_lo,
        ]).astype(ml_dtypes.bfloat16)
        w5 = np.zeros((128, E), ml_dtypes.bfloat16)
        for cb in range(4):
            w5[32 * cb:32 * cb + 5] = w5_rows
        fast_common = {
            "pe_w": pe_wrap, "sj": sj,
            "g_hi": g_hi, "g_lo": g_lo, "w5": w5,
        }
    else:
        w3 = np.stack(
            [w_pitch[0].astype(np.float64), demb.astype(np.float64), bias]
        ).astype(np.float32)
        pe_wl = np.ascontiguousarray(
            pe_proj.astype(np.float32).reshape(NGRP, 128, E).swapaxes(0, 1)
        )

    in_maps = []
    for core in range(NCORES):
        rows_ = range(core * RPC, (core + 1) * RPC)
        enc_t = np.ascontiguousarray(
            encoder_out[core * RPC:(core + 1) * RPC].transpose(0, 2, 1)
        )
        if fast:
            import ml_dtypes as _md
            enc_hi, enc_lo = _bf16_split(enc_t)
            l5 = np.zeros((RPC, 128, T // 4), _md.bfloat16)
            for j, b in enumerate(rows_):
                p_hi, p_lo = _bf16_split(pitch[b, :, 0])
                bt = beats[b, :, 0].astype(_md.bfloat16)
                rows5 = np.stack([p_hi, p_hi, p_lo, bt, bt])     # [5, T]
                for cb in range(4):
                    l5[j, 32 * cb:32 * cb + 5] = rows5[
                        :, cb * (T // 4):(cb + 1) * (T // 4)
                    ]
            m = {"enc_hi": enc_hi, "enc_lo": enc_lo, "l5": l5, **fast_common}
        else:
            p3 = np.empty((RPC, 3, T), np.float32)
            idxo = np.empty((RPC, 128, NGRP), np.int32)
            for j, b in enumerate(rows_):
                p3[j, 0] = pitch[b, :, 0]
                p3[j, 1] = beats[b, :, 0].astype(np.float32)
                p3[j, 2] = 1.0
                idxo[j] = idx[b].reshape(NGRP, 128).T + j * P
            m = {"enc_t": enc_t, "g_mat": g_mat, "pe_w": pe_wl, "p3": p3,
                 "w3": w3, "idxo": idxo}
        in_maps.append(m)
    return fast, in_maps


def kernel(**inputs):
    from concourse.bass_utils import run_bass_kernel_spmd

    fast, in_maps = make_in_maps(**inputs)
    nc = get_nc(fast)
    res = run_bass_kernel_spmd(nc, in_maps, core_ids=list(range(NCORES)))
    out = np.concatenate([res.results[i]["out"] for i in range(NCORES)], axis=0)
    return np.ascontiguousarray(out.astype(np.float32))


# revision 44
# speedup vs baseline: 2.5530x; 1.0217x over previous
"""Trainium2 Bass kernel for nn_Encoder_Postnet (ragged_sequence).

Computation (reference):
    idx   = sequential aligner scan over (align_phone, text_phone)   [B,T]
    out   = enc[idx] + pitch @ w_pitch + b_pitch + emb_beats[beats]
            + (enc[idx] + pe) @ w_pos + b_pos

Key algebraic restructure: the duration-expansion gather commutes with the
E x E linear, so
    out[t] = encG[idx_t] + (pe@w_pos + bias)[t] + pitch[t]*w_pitch + beats[t]*demb
with encG = enc @ (I + w_pos) computed once per batch row ([P,E] not [T,E]),
collapsing the big [B*T,E]@[E,E] matmul 8x and making the kernel memory-bound.

Sharding: pure data parallel, 2 batch rows per core across 8 cores.

Fast path (the uniform duration-8 expansion this model's inputs produce,
idx == arange(T)//8 for every row):
  phase A: encG = enc_row @ (I+w_pos) on PE (fp32); the result stays resident
           in SBUF split into bf16 hi/lo pairs (hi+lo keeps ~17 mantissa
           bits, and bf16 matmuls run 4x faster than fp32 on the PE).
  phase B: per 128-token group, one PSUM accumulation group of 4 bf16
           matmuls: S_j-one-hot expansion of encG rows (hi+lo) + identity
           matmuls adding the (pe@w_pos + bias) tile (hi+lo).  Then the
           pitch rank-1 term rides the DVE op that drains PSUM->SBUF
           (scalar_tensor_tensor), and the beats term runs on the otherwise
           idle GPSIMD.  The only DMA traffic is inputs-once + outputs-once.

General path (arbitrary idx): per-128-token indirect-DMA row gathers
(production-shaped offset [128,1] DynamicAP descriptors) + K=3 stream matmul.

The aligner scan itself is index metadata ([B,T] int32); it is computed on
host with a run-compressed O(B*P) algorithm exactly equivalent to the
reference recurrence, then consumed either as a uniformity proof (fast path)
or as gather offsets (general path).
"""

import sys

for _p in ("/opt/trn_rl_repo",):
    if _p not in sys.path:
        sys.path.insert(0, _p)

import numpy as np

B, P, T, E = 16, 1024, 8192, 256
NCORES = 8
RPC = B // NCORES          # batch rows per core
NGRP = T // 128            # 64 groups of 128 tokens per row
NSUP = 8                   # groups per super-chunk
DUR = T // P               # uniform duration of the fast path (8)
NW = 128 // DUR            # encG rows per group (16)

FORCE_GENERAL = False      # test hook: force the arbitrary-idx path
_CACHE = {}


# --------------------------------------------------------------------------
# Host: aligner index computation (exact replica of the reference recurrence)
# --------------------------------------------------------------------------
def compute_idx(align, text):
    """idx[b,0]=0; idx[b,j] = idx[b,j-1] if align[b,j]==text[b,idx[b,j-1]]
    else min(idx[b,j-1]+1, P-1).   Vectorized over batch via segment starts:
    the pointer advances i->i+1 at s_{i+1} = first j >= s_i+1 with
    align[j] != text[i]; within a run of align values equal to text[i] the
    first mismatch is the run end."""
    align = np.asarray(align)
    text = np.asarray(text)
    Bn, Tn = align.shape
    Pn = text.shape[1]
    diff = align[:, 1:] != align[:, :-1]                       # [B, T-1]
    c = np.full((Bn, Tn), Tn, np.int64)
    c[:, :-1] = np.where(diff, np.arange(1, Tn)[None, :], Tn)
    re = np.flip(np.minimum.accumulate(np.flip(c, axis=1), axis=1), axis=1)

    s = np.full((Bn, Pn), Tn, np.int64)
    s[:, 0] = 0
    cur = np.zeros(Bn, np.int64)
    arB = np.arange(Bn)
    for i in range(Pn - 1):
        j0 = cur + 1
        active = j0 < Tn
        j0c = np.minimum(j0, Tn - 1)
        eq = (align[arB, j0c] == text[:, i]) & active
        nxt = np.where(active, np.where(eq, re[arB, j0c], j0), Tn)
        s[:, i + 1] = nxt
        cur = nxt
    idx = np.empty((Bn, Tn), np.int32)
    pos = np.arange(Tn)
    for b in range(Bn):
        idx[b] = (np.searchsorted(s[b], pos, side="right") - 1).astype(np.int32)
    return idx


def _positional_encoding_f64(t, e):
    pos = np.arange(t, dtype=np.float64)[:, None]
    div = np.exp(np.arange(0, e, 2, dtype=np.float64) * (-np.log(10000.0) / e))
    ang = pos * div[None, :]
    return np.stack([np.sin(ang), np.cos(ang)], axis=-1).reshape(t, e)


def _bf16_split(x):
    import ml_dtypes
    x = np.asarray(x, np.float32)
    hi = x.astype(ml_dtypes.bfloat16)
    lo = (x - hi.astype(np.float32)).astype(ml_dtypes.bfloat16)
    return hi, lo


# --------------------------------------------------------------------------
# Device programs
# --------------------------------------------------------------------------
def build_nc_fast():
    from contextlib import ExitStack
    import concourse.tile as tile
    from concourse import bacc, mybir
    from concourse._compat import get_trn_type

    f32 = mybir.dt.float32
    bf16 = mybir.dt.bfloat16
    mult = mybir.AluOpType.mult
    add = mybir.AluOpType.add

    nc = bacc.Bacc(get_trn_type() or "TRN2", target_bir_lowering=False, debug=False)
    enc_hi = nc.declare_dram_parameter("enc_hi", [RPC, E, P], bf16, isOutput=False)
    enc_lo = nc.declare_dram_parameter("enc_lo", [RPC, E, P], bf16, isOutput=False)
    g_hi = nc.declare_dram_parameter("g_hi", [E, E], bf16, isOutput=False)
    g_lo = nc.declare_dram_parameter("g_lo", [E, E], bf16, isOutput=False)
    pe_w = nc.declare_dram_parameter("pe_w", [128, NGRP, E], f32, isOutput=False)
    sj_d = nc.declare_dram_parameter("sj", [128, NSUP * 128], bf16, isOutput=False)
    # stream-term lhsT rows banked at 32-partition strides (4 token-chunks)
    # to keep the per-partition footprint small for the DMA
    l5_d = nc.declare_dram_parameter("l5", [RPC, 128, T // 4], bf16, isOutput=False)
    w5_d = nc.declare_dram_parameter("w5", [128, E], bf16, isOutput=False)
    out = nc.declare_dram_parameter("out", [RPC, T, E], f32, isOutput=True)

    with tile.TileContext(nc) as tc, ExitStack() as ctx:
        const = ctx.enter_context(tc.tile_pool(name="const", bufs=1))
        pe_pool = ctx.enter_context(tc.tile_pool(name="pe", bufs=8))
        out_pool = ctx.enter_context(tc.tile_pool(name="outp", bufs=4))

        sj_sb = const.tile([128, NSUP * 128], bf16, tag="sj")
        nc.sync.dma_start(sj_sb[:], sj_d[:])
        w5_sb = const.tile([128, E], bf16, tag="w5")
        nc.sync.dma_start(w5_sb[:], w5_d[:])
        l5_sb, egh_keep, egl_keep = [], [], []
        for r in range(RPC):
            l5t = const.tile([128, T // 4], bf16, tag=f"l5_{r}")
            for cb in range(4):
                nc.gpsimd.dma_start(
                    l5t[32 * cb:32 * cb + 5, :], l5_d[r, 32 * cb:32 * cb + 5, :]
                )
            l5_sb.append(l5t)
            egh_keep.append(
                const.tile([128, NSUP, E], bf16, tag=f"egh{r}", name=f"egh{r}")
            )
            egl_keep.append(
                const.tile([128, NSUP, E], bf16, tag=f"egl{r}", name=f"egl{r}")
            )

        # ---- phase A: encG = enc @ (I+w_pos) as 3-term bf16-split matmuls;
        # result kept in SBUF as bf16 hi/lo.  psum layout [128p, m, e] ==
        # keep layout: row m*128+p at (partition p, block m).
        gh0 = const.tile([128, E], bf16, tag="gh0", name="gh0")
        gh1 = const.tile([128, E], bf16, tag="gh1", name="gh1")
        gl0 = const.tile([128, E], bf16, tag="gl0", name="gl0")
        gl1 = const.tile([128, E], bf16, tag="gl1", name="gl1")
        nc.sync.dma_start(gh0[:], g_hi[0:128, :])
        nc.sync.dma_start(gh1[:], g_hi[128:256, :])
        nc.sync.dma_start(gl0[:], g_lo[0:128, :])
        nc.sync.dma_start(gl1[:], g_lo[128:256, :])
        psum_a = ctx.enter_context(tc.tile_pool(name="psumA", bufs=1, space="PSUM"))
        psum_b = ctx.enter_context(tc.tile_pool(name="psumB", bufs=6, space="PSUM"))
        with tc.tile_pool(name="encT", bufs=2) as encT_pool:
            for r in range(RPC):
                eh0 = encT_pool.tile([128, P], bf16, tag="eh0")
                eh1 = encT_pool.tile([128, P], bf16, tag="eh1")
                el0 = encT_pool.tile([128, P], bf16, tag="el0")
                el1 = encT_pool.tile([128, P], bf16, tag="el1")
                nc.sync.dma_start(eh0[:], enc_hi[r, 0:128, :])
                nc.sync.dma_start(eh1[:], enc_hi[r, 128:256, :])
                nc.sync.dma_start(el0[:], enc_lo[r, 0:128, :])
                nc.sync.dma_start(el1[:], enc_lo[r, 128:256, :])
                for mh in range(2):
                    ps = psum_a.tile([128, 4 * E], f32, tag="psA", name="psA")
                    for mi in range(4):
                        m = mh * 4 + mi
                        sl = slice(m * 128, (m + 1) * 128)
                        terms = [
                            (eh0, gh0, True, False), (eh1, gh1, False, False),
                            (el0, gh0, False, False), (el1, gh1, False, False),
                            (eh0, gl0, False, False), (eh1, gl1, False, True),
                        ]
                        for lt, gt_, st, sp in terms:
                            nc.tensor.matmul(
                                ps[:, mi * E:(mi + 1) * E],
                                lhsT=lt[:, sl], rhs=gt_[:], start=st, stop=sp,
                            )
                        # drain per m-chunk so phase B super-chunk m can start
                        # before the rest of phase A finishes
                        hi = egh_keep[r][:, m, :]
                        nc.vector.tensor_copy(hi, ps[:, mi * E:(mi + 1) * E])
                        nc.vector.scalar_tensor_tensor(
                            out=egl_keep[r][:, m, :], in0=hi, scalar=-1.0,
                            in1=ps[:, mi * E:(mi + 1) * E], op0=mult, op1=add,
                        )

        # ---- phase B: per group, 5 bf16 matmuls into one PSUM group:
        # expansion hi/lo + pe hi/lo + the K=5 rank-1 stream matmul
        # (pitch_hi/lo x w_pitch_hi/lo cross terms + beats x demb_hi/lo).
        if True:
            for s in range(T // (NSUP * 128)):
                pe_t = pe_pool.tile([128, NSUP, E], f32, tag="pe")
                nc.sync.dma_start(pe_t[:], pe_w[:, s * NSUP:(s + 1) * NSUP, :])
                for r in range(RPC):
                    ot = out_pool.tile([128, NSUP, E], f32, tag="ot")
                    for j in range(NSUP):
                        g = s * NSUP + j
                        ps = psum_b.tile([128, E], f32, tag="ps")
                        sj_ap = sj_sb[:, j * 128:(j + 1) * 128]
                        nc.tensor.matmul(
                            ps[:], lhsT=sj_ap, rhs=egh_keep[r][:, s, :],
                            start=True, stop=False,
                        )
                        nc.tensor.matmul(
                            ps[:], lhsT=sj_ap, rhs=egl_keep[r][:, s, :],
                            start=False, stop=False,
                        )
                        cb = g // (NGRP // 4)          # token-chunk bank
                        u0 = (g % (NGRP // 4)) * 128
                        nc.tensor.matmul(
                            ps[:],
                            lhsT=l5_sb[r][32 * cb:32 * cb + 5, u0:u0 + 128],
                            rhs=w5_sb[32 * cb:32 * cb + 5, :],
                            start=False, stop=True,
                            tile_position=(32 * cb, 0),
                        )
                        # drain PSUM -> SBUF fused with the (pe@w_pos+bias) add
                        nc.vector.tensor_add(ot[:, j, :], ps[:], pe_t[:, j, :])
                    # out-stores go via the ACT HWDGE queue so they don't
                    # head-of-line-block SP's input loads; two half-stores so
                    # the first half streams while the second half computes
                    for h in range(2):
                        t0 = (s * NSUP + h * (NSUP // 2)) * 128
                        nc.scalar.dma_start(
                            out[r, t0:t0 + (NSUP // 2) * 128, :].rearrange(
                                "(n p) e -> p n e", p=128
                            ),
                            ot[:, h * (NSUP // 2):(h + 1) * (NSUP // 2), :],
                        )
    nc.compile()
    return nc


def build_nc_general():
    """Arbitrary-idx path: per-128-token indirect row gathers."""
    import concourse.bass as bass
    from contextlib import ExitStack
    import concourse.tile as tile
    from concourse import bacc, mybir
    from concourse._compat import get_trn_type

    f32 = mybir.dt.float32
    i32 = mybir.dt.int32

    nc = bacc.Bacc(get_trn_type() or "TRN2", target_bir_lowering=False, debug=False)
    enc_t = nc.declare_dram_parameter("enc_t", [RPC, E, P], f32, isOutput=False)
    g_mat = nc.declare_dram_parameter("g_mat", [E, E], f32, isOutput=False)
    pe_w = nc.declare_dram_parameter("pe_w", [128, NGRP, E], f32, isOutput=False)
    p3 = nc.declare_dram_parameter("p3", [RPC, 3, T], f32, isOutput=False)
    w3 = nc.declare_dram_parameter("w3", [3, E], f32, isOutput=False)
    idxo = nc.declare_dram_parameter(
        "idxo", [RPC, 128, NGRP], i32, isOutput=False
    )
    out = nc.declare_dram_parameter("out", [RPC, T, E], f32, isOutput=True)
    encg = nc.dram_tensor("encg", [RPC, P, E], f32)

    with tile.TileContext(nc) as tc, ExitStack() as ctx:
        const = ctx.enter_context(tc.tile_pool(name="const", bufs=1))
        encT_pool = ctx.enter_context(tc.tile_pool(name="encT", bufs=2))
        psum_pool = ctx.enter_context(tc.tile_pool(name="psum", bufs=2, space="PSUM"))
        eg_pool = ctx.enter_context(tc.tile_pool(name="eg", bufs=2))
        pe_pool = ctx.enter_context(tc.tile_pool(name="pe", bufs=2))
        gath_pool = ctx.enter_context(tc.tile_pool(name="gath", bufs=3))

        g0 = const.tile([128, E], f32, tag="g0")
        g1 = const.tile([128, E], f32, tag="g1")
        nc.sync.dma_start(g0[:], g_mat[0:128, :])
        nc.sync.dma_start(g1[:], g_mat[128:256, :])
        w3_sb = const.tile([3, E], f32, tag="w3")
        nc.sync.dma_start(w3_sb[:], w3[:, :])
        p3_sb = []
        ixo_sb = []
        for r in range(RPC):
            p3t = const.tile([3, T], f32, tag=f"p3_{r}")
            nc.sync.dma_start(p3t[:], p3[r])
            p3_sb.append(p3t)
            ixt = const.tile([128, NGRP], i32, tag=f"ixo_{r}")
            nc.sync.dma_start(ixt[:], idxo[r])
            ixo_sb.append(ixt)

        for r in range(RPC):
            et0 = encT_pool.tile([128, P], f32, tag="et0")
            et1 = encT_pool.tile([128, P], f32, tag="et1")
            nc.sync.dma_start(et0[:], enc_t[r, 0:128, :])
            nc.sync.dma_start(et1[:], enc_t[r, 128:256, :])
            ps = psum_pool.tile([128, 8 * E], f32, tag="ps")
            for m in range(8):
                nc.tensor.matmul(
                    ps[:, m * E:(m + 1) * E],
                    lhsT=et0[:, m * 128:(m + 1) * 128],
                    rhs=g0[:], start=True, stop=False,
                )
                nc.tensor.matmul(
                    ps[:, m * E:(m + 1) * E],
                    lhsT=et1[:, m * 128:(m + 1) * 128],
                    rhs=g1[:], start=False, stop=True,
                )
            eg = eg_pool.tile([128, 8 * E], f32, tag="eg")
            nc.vector.tensor_copy(eg[:], ps[:])
            nc.sync.dma_start(
                encg[r].rearrange("(m p) e -> p m e", p=128),
                eg[:].rearrange("q (m e) -> q m e", e=E),
            )

        encg_flat = encg[:].rearrange("r p e -> (r p) e")
        for s in range(T // (NSUP * 128)):
            pe_t = pe_pool.tile([128, NSUP, E], f32, tag="pe")
            nc.sync.dma_start(pe_t[:], pe_w[:, s * NSUP:(s + 1) * NSUP, :])
            for r in range(RPC):
                gt = gath_pool.tile([128, NSUP, E], f32, tag="gt")
                for g in range(NSUP):
                    gi = s * NSUP + g
                    nc.gpsimd.indirect_dma_start(
                        out=gt[:, g, :],
                        out_offset=None,
                        in_=encg_flat,
                        in_offset=bass.IndirectOffsetOnAxis(
                            ap=ixo_sb[r][:, gi:gi + 1], axis=0
                        ),
                    )
                nc.vector.tensor_add(gt[:], gt[:], pe_t[:])
                ps = psum_pool.tile([128, 8 * E], f32, tag="ps")
                for g in range(NSUP):
                    gi = s * NSUP + g
                    nc.tensor.matmul(
                        ps[:, g * E:(g + 1) * E],
                        lhsT=p3_sb[r][:, gi * 128:(gi + 1) * 128],
                        rhs=w3_sb[:],
                        start=True, stop=True,
                    )
                nc.vector.tensor_add(
                    gt[:], gt[:], ps[:].rearrange("q (n e) -> q n e", e=E)
                )
                nc.sync.dma_start(
                    out[r, s * NSUP * 128:(s + 1) * NSUP * 128, :].rearrange(
                        "(n p) e -> p n e", p=128
                    ),
                    gt[:],
                )
    nc.compile()
    return nc


def get_nc(fast):
    key = "nc_fast" if fast else "nc_gen"
    if key not in _CACHE:
        _CACHE[key] = build_nc_fast() if fast else build_nc_general()
    return _CACHE[key]


# --------------------------------------------------------------------------
# Host wrapper
# --------------------------------------------------------------------------
def make_in_maps(encoder_out, align_phone, text_phone, pitch, beats,
                 w_pitch, b_pitch, emb_beats, w_pos, b_pos):
    import ml_dtypes

    encoder_out = np.asarray(encoder_out, np.float32)
    pitch = np.asarray(pitch, np.float32)
    beats = np.asarray(beats)
    w_pitch = np.asarray(w_pitch, np.float32)
    w_pos = np.asarray(w_pos, np.float32)

    idx = compute_idx(np.asarray(align_phone), np.asarray(text_phone))
    fast = bool(np.all(idx == (np.arange(T, dtype=np.int32) // DUR)[None, :]))
    if FORCE_GENERAL:
        fast = False

    g_mat = (np.eye(E, dtype=np.float64) + w_pos.astype(np.float64)).astype(np.float32)
    pe = _positional_encoding_f64(T, E)
    pe_proj = pe @ w_pos.astype(np.float64)                          # [T, E]
    bias = (np.asarray(emb_beats[0], np.float64)
            + np.asarray(b_pitch, np.float64)
            + np.asarray(b_pos, np.float64))
    demb = (np.asarray(emb_beats[1], np.float64)
            - np.asarray(emb_beats[0], np.float64)).astype(np.float32)

    if fast:
        pe_tot = (pe_proj + bias[None, :]).astype(np.float32)
        pe_wrap = np.ascontiguousarray(pe_tot.reshape(NGRP, 128, E).swapaxes(0, 1))
        # S_j[k, t'] = 1 iff k == j*16 + t'//8
        rows = np.arange(128)[:, None]
        sj = np.concatenate(
            [(rows == (j * NW + np.arange(128) // DUR)[None, :]) for j in range(NSUP)],
            axis=1,
        ).astype(ml_dtypes.bfloat16)
        g_hi, g_lo = _bf16_split(g_mat)
        wp_hi, wp_lo = _bf16_split(w_pitch[0])
        db_hi, db_lo = _bf16_split(demb)
        w5_rows = np.stack([
            wp_hi, wp_lo, wp_hi,
            db_hi, db_lo,
        ]).astype(ml_dtypes.bfloat16)
        w5 = np.zeros((128, E), ml_dtypes.bfloat16)
        for cb in range(4):
            w5[32 * cb:32 * cb + 5] = w5_rows
        fast_common = {
            "pe_w": pe_wrap, "sj": sj,
            "g_hi": g_hi, "g_lo": g_lo, "w5": w5,
        }
    else:
        w3 = np.stack(
            [w_pitch[0].astype(np.float64), demb.astype(np.float64), bias]
        ).astype(np.float32)
        pe_wl = np.ascontiguousarray(
            pe_proj.astype(np.float32).reshape(NGRP, 128, E).swapaxes(0, 1)
        )

    in_maps = []
    for core in range(NCORES):
        rows_ = range(core * RPC, (core + 1) * RPC)
        enc_t = np.ascontiguousarray(
            encoder_out[core * RPC:(core + 1) * RPC].transpose(0, 2, 1)
        )
        if fast:
            import ml_dtypes as _md
            enc_hi, enc_lo = _bf16_split(enc_t)
            l5 = np.zeros((RPC, 128, T // 4), _md.bfloat16)
            for j, b in enumerate(rows_):
                p_hi, p_lo = _bf16_split(pitch[b, :, 0])
                bt = beats[b, :, 0].astype(_md.bfloat16)
                rows5 = np.stack([p_hi, p_hi, p_lo, bt, bt])     # [5, T]
                for cb in range(4):
                    l5[j, 32 * cb:32 * cb + 5] = rows5[
                        :, cb * (T // 4):(cb + 1) * (T // 4)
                    ]
            m = {"enc_hi": enc_hi, "enc_lo": enc_lo, "l5": l5, **fast_common}
        else:
            p3 = np.empty((RPC, 3, T), np.float32)
            idxo = np.empty((RPC, 128, NGRP), np.int32)
            for j, b in enumerate(rows_):
                p3[j, 0] = pitch[b, :, 0]
                p3[j, 1] = beats[b, :, 0].astype(np.float32)
                p3[j, 2] = 1.0
                idxo[j] = idx[b].reshape(NGRP, 128).T + j * P
            m = {"enc_t": enc_t, "g_mat": g_mat, "pe_w": pe_wl, "p3": p3,
                 "w3": w3, "idxo": idxo}
        in_maps.append(m)
    return fast, in_maps


def kernel(**inputs):
    from concourse.bass_utils import run_bass_kernel_spmd

    fast, in_maps = make_in_maps(**inputs)
    nc = get_nc(fast)
    res = run_bass_kernel_spmd(nc, in_maps, core_ids=list(range(NCORES)))
    out = np.concatenate([res.results[i]["out"] for i in range(NCORES)], axis=0)
    return np.ascontiguousarray(out.astype(np.float32))
